# revision 1
# baseline (speedup 1.0000x reference)
"""AFNO block kernel for 8 Trainium2 NeuronCores.

Sharding: token-shard (H rows, 23 per core padded) for LN/MLP phases;
AllToAll to channel-shard (core i = spectral block i, 96 channels) for the
2D-FFT filter, computed as matmuls against precomputed DFT matrices;
AllToAll back; small AllGather for the column-sharded 6144x6144 scale-shift
MLP weight.
"""

import os
import numpy as np
import ml_dtypes

import concourse.bass as bass
import concourse.bacc as bacc
import concourse.mybir as mybir
import concourse.tile as tile
from concourse.bass_utils import run_bass_kernel_spmd
from concourse.masks import make_identity

f32 = mybir.dt.float32
f32r = mybir.dt.float32r
bf16 = mybir.dt.bfloat16
fp16 = mybir.dt.float16
fp8 = mybir.dt.float8e4
FT = mybir.ActivationFunctionType
OP = mybir.AluOpType

H, W, C = 180, 360, 768
NB, BS, KW = 8, 96, 91
HP = 23                 # rows per shard (8*23 = 184 >= 180)
TOKR = HP * W           # 8280 real token slots per shard
NT2 = 33                # phase-2 tiles of 256
TOKP = NT2 * 256        # 8448 padded tokens per shard
MODD, LAT, LAT2 = 64, 3072, 6144
LAM = 0.01
EPS = 1e-5
N = 8


def rap(t, offset, dims):
    a = t[:] if not isinstance(t, bass.AP) else t
    return bass.AP(tensor=a.tensor, offset=a.offset + offset, ap=[list(d) for d in dims])


def _build():
    nc = bacc.Bacc("TRN2", target_bir_lowering=False, debug=False, num_devices=N)

    def P(name, shp, dt=f32):
        return nc.declare_dram_parameter(name, list(shp), dt, isOutput=False)

    xs = P("xs", [TOKP, C])
    modT = P("modT", [MODD, 2])
    n1w = P("n1w", [C]); n1b = P("n1b", [C])
    n2w = P("n2w", [C]); n2b = P("n2b", [C])
    fwr_p = P("fwr", [W, KW], fp16); fwi_p = P("fwi", [W, KW], fp16)
    fhs_p = P("fhs", [2 * H, 2 * H], fp16)
    ifhs_p = P("ifhs", [2 * H, 2 * H], fp16)
    ifwr_p = P("ifwr", [KW, W], fp16); ifwi_p = P("ifwi", [KW, W], fp16)
    w1r_p = P("w1r", [BS, BS], fp16); w1i_p = P("w1i", [BS, BS], fp16)
    w1in_p = P("w1in", [BS, BS], fp16)
    w2cr_p = P("w2cr", [BS, 2 * BS], fp16)   # [W2r | W2i]
    w2ci_p = P("w2ci", [BS, 2 * BS], fp16)   # [-W2i | W2r]
    b1r_p = P("b1r", [BS, 1]); b1i_p = P("b1i", [BS, 1])
    b2c_p = P("b2c", [2 * BS])               # concat(b2r, b2i)
    fs_w0_p = P("fs_w0", [MODD, 2 * C])
    fs_b0T_p = P("fs_b0T", [128, 12])
    fs_w1s_p = P("fs_w1s", [2 * C, 2 * BS])
    fs_b1s_p = P("fs_b1s", [1, 2 * BS])
    ms_w0_p = P("ms_w0", [MODD, LAT2])
    ms_b0T_p = P("ms_b0T", [128, 48])
    ms_w1s_p = P("ms_w1s", [LAT2, C], bf16)
    ms_b1s_p = P("ms_b1s", [1, C])
    fc1w_p = P("fc1w", [C, LAT], fp8)
    fc1bT_p = P("fc1bT", [128, 24])
    fc2w_p = P("fc2w", [LAT, C], fp8)
    fc2b_p = P("fc2b", [C])
    out_p = nc.declare_dram_parameter("out", [TOKP, C], f32, isOutput=True)
    DBG = False

    # internal DRAM
    a1i = nc.dram_tensor("a1i", [N, TOKR * BS], fp16)
    a1o = nc.dram_tensor("a1o", [N, TOKR * BS], fp16)
    a2i = nc.dram_tensor("a2i", [N, TOKR * BS], fp16)
    a2o = nc.dram_tensor("a2o", [N, TOKR * BS], fp16)
    t1d = nc.dram_tensor("t1d", [KW, 2, H, BS], fp16)   # [kw][ri][h][c]
    ud = nc.dram_tensor("ud", [KW, BS, 2 * H], fp16)
    sfd = nc.dram_tensor("sfd", [1, 2 * BS], f32)
    agi = nc.dram_tensor("agi", [1, C], f32)
    ago = nc.dram_tensor("ago", [N, C], f32)

    RG = [list(range(N))]

    with tile.TileContext(nc) as tc:
        with (
            tc.tile_pool(name="const", bufs=1) as cpool,
            tc.tile_pool(name="ssb", bufs=1) as ssb,
        ):
            # ---- broadcast constants ----
            def bcast(p, n, name):
                t = cpool.tile([128, n], f32, tag=name)
                nc.sync.dma_start(out=t[:], in_=rap(p, 0, [[0, 128], [1, n]]))
                return t

            n1w_b = bcast(n1w, C, "n1w"); n1b_b = bcast(n1b, C, "n1b")
            n2w_b = bcast(n2w, C, "n2w"); n2b_b = bcast(n2b, C, "n2b")
            fc2b_b = bcast(fc2b_p, C, "fc2b")
            b2c_b = cpool.tile([128, 2, 2 * BS], f32, tag="b2c")
            for bh in range(2):
                nc.sync.dma_start(out=b2c_b[:, bh, :],
                                  in_=rap(b2c_p, 0, [[0, 128], [1, 2 * BS]]))
            eps_sb = cpool.tile([128, 1], f32, tag="eps")
            nc.vector.memset(eps_sb[:], EPS)
            nlam_sb = cpool.tile([128, 1], f32, tag="nlam")
            nc.vector.memset(nlam_sb[:], -LAM)
            ident = cpool.tile([128, 128], f32, tag="ident")
            make_identity(nc, ident[:])
            identb = cpool.tile([128, 128], bf16, tag="identb")
            nc.vector.tensor_copy(out=identb[:], in_=ident[:])

            # ---- scale-shift MLPs (tiny, overlap with phase 0) ----
            ss_ctx = tc.tile_pool(name="ssw", bufs=1)
            ssw = ss_ctx.__enter__()
            ssp_ctx = tc.tile_pool(name="ssp", bufs=1, space="PSUM")
            ssp = ssp_ctx.__enter__()
            modT_sb = ssw.tile([MODD, 2], f32r)
            nc.sync.dma_start(out=modT_sb[:], in_=modT[:].bitcast(f32r))
            fs_w0_sb = ssw.tile([MODD, 2 * C], f32r)
            nc.sync.dma_start(out=fs_w0_sb[:], in_=fs_w0_p[:].bitcast(f32r))
            fs_b0T_sb = ssw.tile([128, 12], f32)
            nc.sync.dma_start(out=fs_b0T_sb[:], in_=fs_b0T_p[:])
            e0T = ssw.tile([128, 12], f32r)
            for j in range(12):
                pt = ssp.tile([128, 2], f32, tag="ss1")
                nc.tensor.matmul(pt[:], fs_w0_sb[:, 128 * j : 128 * (j + 1)],
                                 modT_sb[:], start=True, stop=True)
                nc.scalar.activation(out=e0T[:, j : j + 1], in_=pt[:, 0:1], func=FT.Gelu,
                                     bias=fs_b0T_sb[:, j : j + 1], scale=1.0)
            fs_w1s_sb = ssw.tile([128, 12, 2 * BS], f32r)
            nc.sync.dma_start(
                out=fs_w1s_sb[:],
                in_=rap(fs_w1s_p, 0, [[2 * BS, 128], [128 * 2 * BS, 12], [1, 2 * BS]]).bitcast(f32r),
            )
            fs_b1s_sb = ssw.tile([1, 2 * BS], f32)
            nc.sync.dma_start(out=fs_b1s_sb[:], in_=fs_b1s_p[:])
            p2 = ssp.tile([1, 2 * BS], f32, tag="ss2")
            for j in range(12):
                nc.tensor.matmul(p2[:], e0T[:, j : j + 1], fs_w1s_sb[:, j, :],
                                 start=(j == 0), stop=(j == 11))
            sfo = ssw.tile([1, 2 * BS], f32)
            nc.vector.tensor_add(out=sfo[:], in0=p2[:], in1=fs_b1s_sb[:])
            nc.sync.dma_start(out=sfd[:], in_=sfo[:])
            sfT = ssw.tile([BS, 2], f32)
            nc.sync.dma_start(out=sfT[:], in_=rap(sfd, 0, [[1, BS], [BS, 2]]))
            sfv = ssb.tile([BS, 1], f32)
            nc.vector.tensor_scalar_add(out=sfv[:], in0=sfT[:, 0:1], scalar1=1.0)
            b1r_sb = ssw.tile([BS, 1], f32)
            nc.sync.dma_start(out=b1r_sb[:], in_=b1r_p[:])
            b1i_sb = ssw.tile([BS, 1], f32)
            nc.sync.dma_start(out=b1i_sb[:], in_=b1i_p[:])
            Br = ssb.tile([BS, 1], f32)
            nc.vector.tensor_mul(out=Br[:], in0=b1r_sb[:], in1=sfv[:])
            nc.vector.tensor_add(out=Br[:], in0=Br[:], in1=sfT[:, 1:2])
            Bi = ssb.tile([BS, 1], f32)
            nc.vector.tensor_mul(out=Bi[:], in0=b1i_sb[:], in1=sfv[:])
            nc.vector.tensor_add(out=Bi[:], in0=Bi[:], in1=sfT[:, 1:2])

            # ms MLP: e1T then column-sharded 6144->768, AllGather
            ms_w0_sb = ssw.tile([MODD, LAT2], f32r)
            nc.sync.dma_start(out=ms_w0_sb[:], in_=ms_w0_p[:].bitcast(f32r))
            ms_b0T_sb = ssw.tile([128, 48], f32)
            nc.sync.dma_start(out=ms_b0T_sb[:], in_=ms_b0T_p[:])
            e1T = ssw.tile([128, 48], bf16)
            for j in range(48):
                pt = ssp.tile([128, 2], f32, tag="ss1")
                nc.tensor.matmul(pt[:], ms_w0_sb[:, 128 * j : 128 * (j + 1)],
                                 modT_sb[:], start=True, stop=True)
                nc.scalar.activation(out=e1T[:, j : j + 1], in_=pt[:, 0:1], func=FT.Gelu,
                                     bias=ms_b0T_sb[:, j : j + 1], scale=1.0)
            p3 = ssp.tile([1, 2, 512], f32, tag="ss3")
            with tc.tile_pool(name="msw", bufs=3) as mswp:
                for j in range(48):
                    wt = mswp.tile([128, C], bf16)
                    nc.sync.dma_start(
                        out=wt[:], in_=ms_w1s_p[128 * j : 128 * (j + 1), :])
                    for h2 in range(2):
                        nc.tensor.matmul(
                            p3[:, h2, 0:384], e1T[:, j : j + 1],
                            wt[:, 384 * h2 : 384 * (h2 + 1)],
                            start=(j == 0), stop=(j == 47))
            ms_b1s_sb = ssw.tile([1, C], f32)
            nc.sync.dma_start(out=ms_b1s_sb[:], in_=ms_b1s_p[:])
            mso = ssw.tile([1, C], f32)
            nc.vector.tensor_add(out=mso[:].rearrange("p (a b) -> p a b", a=2),
                                 in0=p3[:, :, 0:384],
                                 in1=ms_b1s_sb[:].rearrange("p (a b) -> p a b", a=2))
            nc.sync.dma_start(out=agi[:], in_=mso[:])
            nc.gpsimd.collective_compute(
                "AllGather", OP.bypass, replica_groups=RG, ins=[agi[:]], outs=[ago[:]])
            sM = ssb.tile([128, 24], f32)
            nc.sync.dma_start(out=sM[:], in_=rap(ago, 0, [[1, 128], [128, 24]]))
            nc.vector.tensor_scalar_add(out=sM[:], in0=sM[:], scalar1=1.0)
            tM = ssb.tile([128, 24], f32)
            nc.sync.dma_start(out=tM[:], in_=rap(ago, LAT, [[1, 128], [128, 24]]))
            fc1bT_sb = ssw.tile([128, 24], f32)
            nc.sync.dma_start(out=fc1bT_sb[:], in_=fc1bT_p[:])
            B1 = ssb.tile([128, 24], f32)
            nc.vector.tensor_mul(out=B1[:], in0=fc1bT_sb[:], in1=sM[:])
            nc.vector.tensor_add(out=B1[:], in0=B1[:], in1=tM[:])
            sM16 = ssb.tile([128, 24], f32)
            nc.vector.tensor_scalar_mul(out=sM16[:], in0=sM[:], scalar1=1.0 / 16.0)

            ssp_ctx.__exit__(None, None, None)
            ss_ctx.__exit__(None, None, None)

            # ---- phase 0: LN1 + scatter into A2A-1 send buffer ----
            with (
                tc.tile_pool(name="p0", bufs=6) as p0,
                tc.tile_pool(name="p0s", bufs=8) as p0s,
            ):
                for it in range(65):
                    t0 = it * 128
                    nrow = min(128, TOKR - t0)
                    xt = p0.tile([128, C], f32, tag="xt")
                    nc.sync.dma_start(out=xt[:], in_=xs[t0 : t0 + 128, :])
                    st = p0s.tile([128, 2, 6], f32, tag="st")
                    for g in range(2):
                        nc.vector.bn_stats(out=st[:, g, :], in_=xt[:, 384 * g : 384 * (g + 1)])
                    mv = p0s.tile([128, 2], f32, tag="mv")
                    nc.vector.bn_aggr(out=mv[:], in_=st[:])
                    rstd = p0s.tile([128, 1], f32, tag="rstd")
                    nc.scalar.activation(out=rstd[:], in_=mv[:, 1:2], func=FT.Sqrt,
                                         bias=eps_sb[:], scale=1.0)
                    nc.vector.reciprocal(out=rstd[:], in_=rstd[:])
                    ln = p0.tile([128, C], f32, tag="ln")
                    nc.vector.tensor_scalar(out=ln[:], in0=xt[:], scalar1=mv[:, 0:1],
                                            scalar2=rstd[:], op0=OP.subtract, op1=OP.mult)
                    nc.gpsimd.tensor_mul(out=ln[:], in0=ln[:], in1=n1w_b[:])
                    lnh = p0.tile([128, C], fp16, tag="lnh")
                    nc.gpsimd.tensor_add(out=lnh[:], in0=ln[:], in1=n1b_b[:])
                    nc.scalar.dma_start(
                        out=rap(a1i, t0 * BS, [[BS, nrow], [TOKR * BS, N], [1, BS]]),
                        in_=lnh[:nrow].rearrange("p (j c) -> p j c", j=N),
                    )

            nc.gpsimd.collective_compute(
                "AllToAll", OP.bypass, replica_groups=RG, ins=[a1i[:]], outs=[a1o[:]])

            # ---- phase 1 stage A: W-DFT  (X[h,w,c] -> t1d[kw,ri,h,c]) ----
            with (
                tc.tile_pool(name="sa", bufs=1) as sa,
                tc.tile_pool(name="sac", bufs=3) as sac,
                tc.tile_pool(name="sap", bufs=2, space="PSUM") as sap,
            ):
                fw_sb = []
                for ri, p in enumerate([fwr_p, fwi_p]):
                    t = sa.tile([120, 3, KW], fp16, tag=f"fw{ri}")
                    nc.sync.dma_start(
                        out=t[:], in_=rap(p, 0, [[KW, 120], [120 * KW, 3], [1, KW]]))
                    fw_sb.append(t)
                X_sb = sa.tile([120, 3, H, BS], fp16, tag="xsb")
                for ch in range(4):
                    for k in range(3):
                        eng = nc.scalar if (ch + k) % 2 else nc.sync
                        eng.dma_start(
                            out=X_sb[:, k, 45 * ch : 45 * (ch + 1), :],
                            in_=rap(a1o, (45 * ch * W + 120 * k) * BS,
                                    [[BS, 120], [W * BS, 45], [1, BS]]))
                for hs in range(36):
                    hh0 = 5 * hs
                    cp = sac.tile([KW, 2, 5, BS], fp16, tag="cpa")
                    for ri in range(2):
                        ps = sap.tile([KW, 5, BS], f32, tag="pa")
                        for k in range(3):
                            nc.tensor.matmul(ps[:], fw_sb[ri][:, k, :],
                                             X_sb[:, k, hh0 : hh0 + 5, :],
                                             start=(k == 0), stop=(k == 2))
                        nc.vector.tensor_copy(out=cp[:, ri, :, :], in_=ps[:])
                    nc.scalar.dma_start(
                        out=rap(t1d, hh0 * BS,
                                [[2 * H * BS, KW], [H * BS, 2], [BS, 5], [1, BS]]),
                        in_=cp[:])

            # ---- stages B+C+D fused in SBUF, then E ----
            with tc.tile_pool(name="fb", bufs=1) as fb:
                fhs_sb = fb.tile([90, 4, 2 * H], fp16)
                nc.sync.dma_start(
                    out=fhs_sb[:],
                    in_=rap(fhs_p, 0, [[2 * H, 90], [90 * 2 * H, 4], [1, 2 * H]]))
                ifhs_sb = fb.tile([90, 4, 2 * H], fp16)
                nc.sync.dma_start(
                    out=ifhs_sb[:],
                    in_=rap(ifhs_p, 0, [[2 * H, 90], [90 * 2 * H, 4], [1, 2 * H]]))
                w1r_sb = fb.tile([BS, BS], fp16)
                nc.sync.dma_start(out=w1r_sb[:], in_=w1r_p[:])
                w1i_sb = fb.tile([BS, BS], fp16)
                nc.sync.dma_start(out=w1i_sb[:], in_=w1i_p[:])
                w1in_sb = fb.tile([BS, BS], fp16)
                nc.sync.dma_start(out=w1in_sb[:], in_=w1in_p[:])
                w2cr_sb = fb.tile([BS, 2 * BS], fp16)
                nc.sync.dma_start(out=w2cr_sb[:], in_=w2cr_p[:])
                w2ci_sb = fb.tile([BS, 2 * BS], fp16)
                nc.sync.dma_start(out=w2ci_sb[:], in_=w2ci_p[:])
                ifw_sb = []
                for ri, p in enumerate([ifwr_p, ifwi_p]):
                    t = fb.tile([KW, 3, 120], fp16, tag=f"ifw{ri}")
                    nc.sync.dma_start(
                        out=t[:], in_=rap(p, 0, [[W, KW], [120, 3], [1, 120]]))
                    ifw_sb.append(t)
                # T1 resident: [h(90), half, ri, kw, c]
                T1_sb = fb.tile([90, 2, 2, KW, BS], fp16, tag="t1sb")
                for half in range(2):
                    for ri in range(2):
                        eng = nc.scalar if ri else nc.sync
                        eng.dma_start(
                            out=T1_sb[:, half, ri, :, :],
                            in_=rap(t1d, (ri * H + half * 90) * BS,
                                    [[BS, 90], [2 * H * BS, KW], [1, BS]]))
                U_sb = fb.tile([KW, BS, 2 * H], fp16, tag="usb")

                with (
                    tc.tile_pool(name="bcw", bufs=4) as bcw,
                    tc.tile_pool(name="bct", bufs=4) as bct,
                    tc.tile_pool(name="bco", bufs=3) as bco,
                    tc.tile_pool(name="bcp1", bufs=1, space="PSUM") as bcps,
                    tc.tile_pool(name="bcp2", bufs=1, space="PSUM") as bcps2,
                    tc.tile_pool(name="bcp3", bufs=2, space="PSUM") as bcps3,
                    tc.tile_pool(name="bcp4", bufs=1, space="PSUM") as bcps4,
                ):
                    for pr in range(46):
                        kw0 = 2 * pr
                        G = 2 if kw0 + 1 < KW else 1
                        psF = bcps.tile([BS, 2, 512], f32, tag="psF")
                        for g in range(G):
                            kw = kw0 + g
                            for q in range(4):
                                ri, half = q // 2, q % 2
                                nc.tensor.matmul(
                                    psF[:, g, 0 : 2 * H], T1_sb[:, half, ri, kw, :],
                                    fhs_sb[:, q, :], start=(q == 0), stop=(q == 3))
                        fsb = bcw.tile([BS, 2, 2 * H], fp16, tag="fsb")
                        nc.vector.tensor_copy(out=fsb[:, :G, :], in_=psF[:, :G, 0 : 2 * H])
                        ps1r = bcps2.tile([BS, 2, H], f32, tag="ps1r")
                        ps1i = bcps2.tile([BS, 2, H], f32, tag="ps1i")
                        nc.tensor.matmul(ps1r[:, :G, :], w1r_sb[:], fsb[:, :G, 0:H],
                                         start=True, stop=False)
                        nc.tensor.matmul(ps1r[:, :G, :], w1in_sb[:], fsb[:, :G, H : 2 * H],
                                         start=False, stop=True)
                        nc.tensor.matmul(ps1i[:, :G, :], w1i_sb[:], fsb[:, :G, 0:H],
                                         start=True, stop=False)
                        nc.tensor.matmul(ps1i[:, :G, :], w1r_sb[:], fsb[:, :G, H : 2 * H],
                                         start=False, stop=True)
                        o1r = bcw.tile([BS, 2, H], fp16, tag="o1r")
                        o1i = bcw.tile([BS, 2, H], fp16, tag="o1i")
                        nc.scalar.activation(out=o1r[:, :G, :], in_=ps1r[:, :G, :],
                                             func=FT.Relu, bias=Br[:], scale=sfv[:])
                        nc.scalar.activation(out=o1i[:, :G, :], in_=ps1i[:, :G, :],
                                             func=FT.Relu, bias=Bi[:], scale=sfv[:])
                        o1rf = o1r[:].rearrange("p g k -> p (g k)")
                        o1if = o1i[:].rearrange("p g k -> p (g k)")
                        O2t = bco.tile([90, 2, 2, 2 * BS], fp16, tag="o2sb")
                        for g in range(G):
                            ps2 = bcps3.tile([90, 2, 2 * BS], f32, tag="ps2")
                            for half in range(2):
                                sl = slice(90 * (2 * g + half), 90 * (2 * g + half + 1))
                                nc.tensor.matmul(ps2[:, half, :], o1rf[:, sl], w2cr_sb[:],
                                                 start=True, stop=False)
                                nc.tensor.matmul(ps2[:, half, :], o1if[:, sl], w2ci_sb[:],
                                                 start=False, stop=True)
                            tmp = bct.tile([90, 2, 2 * BS], f32, tag="tmp")
                            nc.vector.tensor_add(out=tmp[:], in0=ps2[:], in1=b2c_b[:90])
                            r1 = bct.tile([90, 2, 2 * BS], f32, tag="r1")
                            nc.scalar.activation(out=r1[:], in_=tmp[:], func=FT.Relu,
                                                 bias=nlam_sb[:90], scale=1.0)
                            r2 = bct.tile([90, 2, 2 * BS], f32, tag="r2")
                            nc.scalar.activation(out=r2[:], in_=tmp[:], func=FT.Relu,
                                                 bias=nlam_sb[:90], scale=-1.0)
                            nc.gpsimd.tensor_sub(out=O2t[:, g, :, :],
                                                 in0=r1[:], in1=r2[:])
                        # stage D (inverse H-DFT) fused per kw pair
                        psU = bcps4.tile([BS, 2, 512], f32, tag="psU")
                        for g in range(G):
                            for q in range(4):
                                ri, half = q // 2, q % 2
                                nc.tensor.matmul(
                                    psU[:, g, 0 : 2 * H],
                                    O2t[:, g, half, ri * BS : (ri + 1) * BS],
                                    ifhs_sb[:, q, :], start=(q == 0), stop=(q == 3))
                        ucp = bcw.tile([BS, 2, 2 * H], fp16, tag="ucp")
                        nc.vector.tensor_copy(out=ucp[:, :G, :], in_=psU[:, :G, 0 : 2 * H])
                        nc.sync.dma_start(
                            out=rap(ud, kw0 * BS * 2 * H,
                                    [[2 * H, BS], [BS * 2 * H, G], [1, 2 * H]]),
                            in_=ucp[:, :G, :])
                        # (ud write stays on SP: Act is busy with o1/r1/r2 here)

                # U back to kw-partitioned SBUF, then stage E (inverse W-DFT)
                for chv in range(2):
                    k0 = 46 * chv
                    kn = min(46, KW - k0)
                    eng = nc.scalar if chv else nc.sync
                    eng.dma_start(
                        out=U_sb[k0 : k0 + kn, :, :],
                        in_=rap(ud, k0 * BS * 2 * H,
                                [[BS * 2 * H, kn], [2 * H, BS], [1, 2 * H]]))
                with (
                    tc.tile_pool(name="sec", bufs=4) as sec,
                    tc.tile_pool(name="sep", bufs=2, space="PSUM") as sep,
                ):
                    for wk in range(3):
                        for ht in range(45):
                            h0 = 4 * ht
                            psE = sep.tile([120, 4, BS], f32, tag="psE")
                            for ri in range(2):
                                nc.tensor.matmul(
                                    psE[:], ifw_sb[ri][:, wk, :],
                                    U_sb[:, :, ri * H + h0 : ri * H + h0 + 4]
                                        .rearrange("p c h -> p h c"),
                                    start=(ri == 0), stop=(ri == 1))
                            ecp = sec.tile([120, 4, BS], fp16, tag="ecp")
                            nc.vector.tensor_copy(out=ecp[:], in_=psE[:])
                            nc.scalar.dma_start(
                                out=rap(a2i, h0 * W * BS + wk * 120 * BS,
                                        [[BS, 120], [W * BS, 4], [1, BS]]),
                                in_=ecp[:])

            nc.gpsimd.collective_compute(
                "AllToAll", OP.bypass, replica_groups=RG, ins=[a2i[:]], outs=[a2o[:]])

            # ---- phase 2: h1 = F2 + ln1x + x; LN2; modulated MLP; + h1 ----
            with (
                tc.tile_pool(name="p2w", bufs=1) as p2w,
                tc.tile_pool(name="p2", bufs=4) as p2,
                tc.tile_pool(name="p2h", bufs=4) as p2h,
                tc.tile_pool(name="p2s", bufs=4) as p2s,
                tc.tile_pool(name="p2m", bufs=2) as p2m,
                tc.tile_pool(name="ptp", bufs=2, space="PSUM") as ptp,
                tc.tile_pool(name="php", bufs=2, space="PSUM") as php,
                tc.tile_pool(name="pop", bufs=2, space="PSUM") as pop,
            ):
                PM = mybir.MatmulPerfMode.DoubleRow
                fc1w_sb = p2w.tile([128, 6, LAT], fp8)
                nc.sync.dma_start(
                    out=fc1w_sb[:], in_=rap(fc1w_p, 0, [[LAT, 128], [128 * LAT, 6], [1, LAT]]))
                fc2w_sb = p2w.tile([128, 24, C], fp8)
                nc.sync.dma_start(
                    out=fc2w_sb[:], in_=rap(fc2w_p, 0, [[C, 128], [128 * C, 24], [1, C]]))

                def p2_prep(it):
                    T0 = it * 256
                    ln2T = p2m.tile([128, 6, 2, 128], fp8, tag="ln2T")
                    h1s = []
                    for hf in range(2):
                        t0 = T0 + 128 * hf
                        nload = max(0, min(128, TOKR - t0))
                        xt = p2.tile([128, C], f32, tag="xt2")
                        nc.sync.dma_start(out=xt[:], in_=xs[t0 : t0 + 128, :])
                        f2t = p2.tile([128, N, BS], fp16, tag="f2t")
                        l1t = p2.tile([128, N, BS], fp16, tag="l1t")
                        if nload < 128:
                            nc.vector.memset(f2t[:], 0.0)
                            nc.vector.memset(l1t[:], 0.0)
                        if nload > 0:
                            nc.sync.dma_start(
                                out=f2t[:nload],
                                in_=rap(a2o, t0 * BS, [[BS, nload], [TOKR * BS, N], [1, BS]]))
                            nc.sync.dma_start(
                                out=l1t[:nload],
                                in_=rap(a1i, t0 * BS, [[BS, nload], [TOKR * BS, N], [1, BS]]))
                        h1 = p2h.tile([128, C], f32, tag="h1")
                        nc.gpsimd.tensor_add(out=h1[:], in0=xt[:],
                                             in1=f2t[:].rearrange("p j c -> p (j c)"))
                        nc.gpsimd.tensor_add(out=h1[:], in0=h1[:],
                                             in1=l1t[:].rearrange("p j c -> p (j c)"))
                        h1s.append(h1)
                        st = p2s.tile([128, 2, 6], f32, tag="st2")
                        for g in range(2):
                            nc.vector.bn_stats(out=st[:, g, :], in_=h1[:, 384 * g : 384 * (g + 1)])
                        mv = p2s.tile([128, 2], f32, tag="mv2")
                        nc.vector.bn_aggr(out=mv[:], in_=st[:])
                        rstd = p2s.tile([128, 1], f32, tag="rstd2")
                        nc.scalar.activation(out=rstd[:], in_=mv[:, 1:2], func=FT.Sqrt,
                                             bias=eps_sb[:], scale=1.0)
                        nc.vector.reciprocal(out=rstd[:], in_=rstd[:])
                        ln2 = p2.tile([128, C], bf16, tag="ln2")
                        nc.vector.tensor_scalar(out=ln2[:], in0=h1[:], scalar1=mv[:, 0:1],
                                                scalar2=rstd[:], op0=OP.subtract, op1=OP.mult)
                        nc.gpsimd.tensor_mul(out=ln2[:], in0=ln2[:], in1=n2w_b[:])
                        nc.gpsimd.tensor_add(out=ln2[:], in0=ln2[:], in1=n2b_b[:])
                        for jb in range(2):
                            pst = ptp.tile([128, 3, 128], bf16, tag="pst")
                            for jj in range(3):
                                j = 3 * jb + jj
                                nc.tensor.transpose(pst[:, jj, :],
                                                    ln2[:, 128 * j : 128 * (j + 1)],
                                                    identb[:])
                            nc.vector.tensor_copy(out=ln2T[:, 3 * jb : 3 * jb + 3, hf, :],
                                                  in_=pst[:])
                    return T0, ln2T, h1s

                def p2_mm(T0, ln2T, h1s):
                    hmidT = p2m.tile([128, 24, 256], fp8, tag="hmidT")
                    for l in range(24):
                        psH = php.tile([128, 256], f32, tag="psH")
                        for jp in range(3):
                            nc.tensor.matmul(
                                psH[:],
                                fc1w_sb[:, 2 * jp : 2 * jp + 2, 128 * l : 128 * (l + 1)],
                                ln2T[:, 2 * jp : 2 * jp + 2, :, :]
                                    .rearrange("p j h t -> p j (h t)"),
                                start=(jp == 0), stop=(jp == 2), perf_mode=PM)
                        nc.scalar.activation(out=hmidT[:, l, :], in_=psH[:], func=FT.Gelu,
                                             bias=B1[:, l : l + 1], scale=sM16[:, l : l + 1])
                    for hf in range(2):
                        t0 = T0 + 128 * hf
                        psO = pop.tile([128, 2, 512], f32, tag="psO")
                        for lp in range(12):
                            for h2 in range(2):
                                nc.tensor.matmul(
                                    psO[:, h2, 0:384],
                                    hmidT[:, 2 * lp : 2 * lp + 2, 128 * hf : 128 * (hf + 1)],
                                    fc2w_sb[:, 2 * lp : 2 * lp + 2, 384 * h2 : 384 * (h2 + 1)],
                                    start=(lp == 0), stop=(lp == 11), perf_mode=PM)
                        ot = p2.tile([128, C], f32, tag="ot")
                        nc.vector.scalar_tensor_tensor(
                            out=ot[:].rearrange("p (a b) -> p a b", a=2),
                            in0=psO[:, :, 0:384], scalar=1.0 / 16.0,
                            in1=h1s[hf][:].rearrange("p (a b) -> p a b", a=2),
                            op0=OP.mult, op1=OP.add)
                        nc.gpsimd.tensor_add(out=ot[:], in0=ot[:], in1=fc2b_b[:])
                        nc.sync.dma_start(out=out_p[t0 : t0 + 128, :], in_=ot[:])

                # pairs of token tiles: LN2 (Sqrt) for both, then MLP (Gelu)
                # for both — halves Act table swaps
                for itp in range(0, NT2, 2):
                    states = [p2_prep(it) for it in range(itp, min(itp + 2, NT2))]
                    for s in states:
                        p2_mm(*s)

    nc.compile()
    return nc


_NC = None


def _get_nc():
    global _NC
    if _NC is None:
        _NC = _build()
    return _NC


def _dft_mats():
    w = np.arange(W); kw = np.arange(KW)
    ang = 2 * np.pi * np.outer(w, kw) / W
    fwr = (np.cos(ang) / np.sqrt(W)).astype(np.float32)
    fwi = (-np.sin(ang) / np.sqrt(W)).astype(np.float32)
    kh = np.arange(H); h = np.arange(H)
    angh = 2 * np.pi * np.outer(kh, h) / H        # [kh, h]
    fhr = np.cos(angh) / np.sqrt(H)
    fhi = -np.sin(angh) / np.sqrt(H)
    fhs = np.zeros((2 * H, 2 * H))
    fhs[:H, :H] = fhr.T; fhs[:H, H:] = fhi.T
    fhs[H:, :H] = -fhi.T; fhs[H:, H:] = fhr.T
    ci = np.cos(angh) / np.sqrt(H)                # [kh, h] for inverse
    si = np.sin(angh) / np.sqrt(H)
    ifhs = np.zeros((2 * H, 2 * H))
    ifhs[:H, :H] = ci; ifhs[:H, H:] = si
    ifhs[H:, :H] = -si; ifhs[H:, H:] = ci
    ckw = np.where(kw == 0, 1.0, 2.0)
    angi = 2 * np.pi * np.outer(kw, np.arange(W)) / W    # [kw, w]
    ifwr = (ckw[:, None] * np.cos(angi) / np.sqrt(W)).astype(np.float32)
    ifwi = (-ckw[:, None] * np.sin(angi) / np.sqrt(W)).astype(np.float32)
    return fwr, fwi, fhs.astype(np.float32), ifhs.astype(np.float32), ifwr, ifwi


def _prepare_in_maps(x, mod_embed, n1w, n1b, n2w, n2b, w1, b1, w2, b2,
                     fs_w0, fs_b0, fs_w1, fs_b1, fc1w, fc1b, fc2w, fc2b,
                     ms_w0, ms_b0, ms_w1, ms_b1):
    f = np.asarray
    x = f(x, dtype=np.float32)
    grid = x.reshape(H, W, C)
    fwr, fwi, fhs, ifhs, ifwr, ifwi = _dft_mats()
    bf = ml_dtypes.bfloat16

    in_maps = []
    for b in range(N):
        r0, r1 = HP * b, min(HP * (b + 1), H)
        xsb = np.zeros((TOKP, C), np.float32)
        xsb[: (r1 - r0) * W] = grid[r0:r1].reshape(-1, C)
        sl = slice(BS * b, BS * (b + 1))
        w2r = f(w2[0, b], np.float32); w2i = f(w2[1, b], np.float32)
        im = {
            "xs": xsb,
            "modT": np.repeat(f(mod_embed, np.float32).reshape(MODD, 1), 2, axis=1).copy(),
            "n1w": f(n1w, np.float32), "n1b": f(n1b, np.float32),
            "n2w": f(n2w, np.float32), "n2b": f(n2b, np.float32),
            "fwr": fwr.astype(np.float16), "fwi": fwi.astype(np.float16),
            "fhs": fhs.astype(np.float16), "ifhs": ifhs.astype(np.float16),
            "ifwr": ifwr.astype(np.float16), "ifwi": ifwi.astype(np.float16),
            "w1r": f(w1[0, b], np.float16).copy(),
            "w1i": f(w1[1, b], np.float16).copy(),
            "w1in": (-f(w1[1, b], np.float16)).copy(),
            "w2cr": np.concatenate([w2r, w2i], axis=1).astype(np.float16),
            "w2ci": np.concatenate([-w2i, w2r], axis=1).astype(np.float16),
            "b1r": f(b1[0, b], np.float32).reshape(BS, 1).copy(),
            "b1i": f(b1[1, b], np.float32).reshape(BS, 1).copy(),
            "b2c": np.concatenate([f(b2[0, b], np.float32), f(b2[1, b], np.float32)]),
            "fs_w0": f(fs_w0, np.float32),
            "fs_b0T": f(fs_b0, np.float32).reshape(12, 128).T.copy(),
            "fs_w1s": np.concatenate(
                [f(fs_w1, np.float32)[:, sl], f(fs_w1, np.float32)[:, C + BS * b : C + BS * (b + 1)]],
                axis=1),
            "fs_b1s": np.concatenate(
                [f(fs_b1, np.float32)[sl], f(fs_b1, np.float32)[C + BS * b : C + BS * (b + 1)]]
            ).reshape(1, -1),
            "ms_w0": f(ms_w0, np.float32),
            "ms_b0T": f(ms_b0, np.float32).reshape(48, 128).T.copy(),
            "ms_w1s": f(ms_w1, np.float32)[:, C * b : C * (b + 1)].astype(bf),
            "ms_b1s": f(ms_b1, np.float32)[C * b : C * (b + 1)].reshape(1, -1),
            "fc1w": (16.0 * f(fc1w, np.float32)).astype(ml_dtypes.float8_e4m3),
            "fc1bT": f(fc1b, np.float32).reshape(24, 128).T.copy(),
            "fc2w": (16.0 * f(fc2w, np.float32)).astype(ml_dtypes.float8_e4m3),
            "fc2b": f(fc2b, np.float32),
        }
        in_maps.append(im)
    return in_maps


def kernel(x, mod_embed, n1w, n1b, n2w, n2b, w1, b1, w2, b2,
           fs_w0, fs_b0, fs_w1, fs_b1, fc1w, fc1b, fc2w, fc2b,
           ms_w0, ms_b0, ms_w1, ms_b1):
    nc = _get_nc()
    in_maps = _prepare_in_maps(
        x, mod_embed, n1w, n1b, n2w, n2b, w1, b1, w2, b2,
        fs_w0, fs_b0, fs_w1, fs_b1, fc1w, fc1b, fc2w, fc2b,
        ms_w0, ms_b0, ms_w1, ms_b1)

    res = run_bass_kernel_spmd(nc, in_maps, core_ids=list(range(N)))
    globals()["last_results"] = res
    out = np.zeros((H, W, C), np.float32)
    for b in range(N):
        r0, r1 = HP * b, min(HP * (b + 1), H)
        out[r0:r1] = res.results[b]["out"][: (r1 - r0) * W].reshape(r1 - r0, W, C)
    return out.reshape(1, H, W, C)



# revision 9
# speedup vs baseline: 1.0753x; 1.0753x over previous
"""AFNO block kernel for 8 Trainium2 NeuronCores.

Sharding: token-shard (H rows, 23 per core padded) for LN/MLP phases;
AllToAll to channel-shard (core i = spectral block i, 96 channels) for the
2D-FFT filter, computed as matmuls against precomputed DFT matrices;
AllToAll back; small AllGather for the column-sharded 6144x6144 scale-shift
MLP weight.
"""

import os
import numpy as np
import ml_dtypes

import concourse.bass as bass
import concourse.bacc as bacc
import concourse.mybir as mybir
import concourse.tile as tile
from concourse.bass_utils import run_bass_kernel_spmd
from concourse.masks import make_identity

f32 = mybir.dt.float32
f32r = mybir.dt.float32r
bf16 = mybir.dt.bfloat16
fp16 = mybir.dt.float16
fp8 = mybir.dt.float8e4
FT = mybir.ActivationFunctionType
OP = mybir.AluOpType

H, W, C = 180, 360, 768
NB, BS, KW = 8, 96, 91
HP = 23                 # rows per shard (8*23 = 184 >= 180)
TOKR = HP * W           # 8280 real token slots per shard
NT2 = 33                # phase-2 tiles of 256
TOKP = NT2 * 256        # 8448 padded tokens per shard
MODD, LAT, LAT2 = 64, 3072, 6144
LAM = 0.01
EPS = 1e-5
N = 8


def rap(t, offset, dims):
    a = t[:] if not isinstance(t, bass.AP) else t
    return bass.AP(tensor=a.tensor, offset=a.offset + offset, ap=[list(d) for d in dims])


def _build():
    nc = bacc.Bacc("TRN2", target_bir_lowering=False, debug=False, num_devices=N)

    def P(name, shp, dt=f32):
        return nc.declare_dram_parameter(name, list(shp), dt, isOutput=False)

    xs = P("xs", [TOKP, C])
    modT = P("modT", [MODD, 2])
    n1w = P("n1w", [C]); n1b = P("n1b", [C])
    n2w = P("n2w", [C]); n2b = P("n2b", [C])
    fwr_p = P("fwr", [W, KW], fp16); fwi_p = P("fwi", [W, KW], fp16)
    fhs_p = P("fhs", [2 * H, 2 * H], fp16)
    ifhs_p = P("ifhs", [2 * H, 2 * H], fp16)
    ifwr_p = P("ifwr", [KW, W], fp16); ifwi_p = P("ifwi", [KW, W], fp16)
    w1r_p = P("w1r", [BS, BS], fp16); w1i_p = P("w1i", [BS, BS], fp16)
    w1in_p = P("w1in", [BS, BS], fp16)
    w2cr_p = P("w2cr", [BS, 2 * BS], fp16)   # [W2r | W2i]
    w2ci_p = P("w2ci", [BS, 2 * BS], fp16)   # [-W2i | W2r]
    b1r_p = P("b1r", [BS, 1]); b1i_p = P("b1i", [BS, 1])
    b2c_p = P("b2c", [2 * BS])               # concat(b2r, b2i)
    fs_w0_p = P("fs_w0", [MODD, 2 * C])
    fs_b0T_p = P("fs_b0T", [128, 12])
    fs_w1s_p = P("fs_w1s", [2 * C, 2 * BS])
    fs_b1s_p = P("fs_b1s", [1, 2 * BS])
    ms_w0_p = P("ms_w0", [MODD, LAT2])
    ms_b0T_p = P("ms_b0T", [128, 48])
    ms_w1s_p = P("ms_w1s", [LAT2, C], bf16)
    ms_b1s_p = P("ms_b1s", [1, C])
    fc1w_p = P("fc1w", [C, LAT], fp8)
    fc1bT_p = P("fc1bT", [128, 24])
    fc2w_p = P("fc2w", [LAT, C], fp8)
    fc2b_p = P("fc2b", [C])
    out_p = nc.declare_dram_parameter("out", [TOKP, C], f32, isOutput=True)
    DBG = False

    # internal DRAM
    a1i = nc.dram_tensor("a1i", [N, TOKR * BS], fp16)
    a1o = nc.dram_tensor("a1o", [N, TOKR * BS], fp16)
    a2i = nc.dram_tensor("a2i", [N, TOKR * BS], fp16)
    a2o = nc.dram_tensor("a2o", [N, TOKR * BS], fp16)
    t1d = nc.dram_tensor("t1d", [KW, 2, H, BS], fp16)   # [kw][ri][h][c]
    ud = nc.dram_tensor("ud", [KW, BS, 2 * H], fp16)
    sfd = nc.dram_tensor("sfd", [1, 2 * BS], f32)
    agi = nc.dram_tensor("agi", [1, C], f32)
    ago = nc.dram_tensor("ago", [N, C], f32)

    RG = [list(range(N))]

    with tile.TileContext(nc) as tc:
        with (
            tc.tile_pool(name="const", bufs=1) as cpool,
            tc.tile_pool(name="ssb", bufs=1) as ssb,
        ):
            # ---- broadcast constants ----
            def bcast(p, n, name):
                t = cpool.tile([128, n], f32, tag=name)
                nc.sync.dma_start(out=t[:], in_=rap(p, 0, [[0, 128], [1, n]]))
                return t

            n1w_b = bcast(n1w, C, "n1w"); n1b_b = bcast(n1b, C, "n1b")
            n2w_b = bcast(n2w, C, "n2w"); n2b_b = bcast(n2b, C, "n2b")
            fc2b_b = bcast(fc2b_p, C, "fc2b")
            b2c_b = cpool.tile([128, 2, 2 * BS], f32, tag="b2c")
            for bh in range(2):
                nc.sync.dma_start(out=b2c_b[:, bh, :],
                                  in_=rap(b2c_p, 0, [[0, 128], [1, 2 * BS]]))
            eps_sb = cpool.tile([128, 1], f32, tag="eps")
            nc.vector.memset(eps_sb[:], EPS)
            nlam_sb = cpool.tile([128, 1], f32, tag="nlam")
            nc.vector.memset(nlam_sb[:], -LAM)
            ident = cpool.tile([128, 128], f32, tag="ident")
            make_identity(nc, ident[:])
            identb = cpool.tile([128, 128], bf16, tag="identb")
            nc.vector.tensor_copy(out=identb[:], in_=ident[:])


            # ---- phase 0: LN1 + scatter into A2A-1 send buffer ----
            with (
                tc.tile_pool(name="p0", bufs=6) as p0,
                tc.tile_pool(name="p0s", bufs=8) as p0s,
            ):
                for it in range(65):
                    t0 = it * 128
                    nrow = min(128, TOKR - t0)
                    xt = p0.tile([128, C], f32, tag="xt")
                    nc.sync.dma_start(out=xt[:], in_=xs[t0 : t0 + 128, :])
                    st = p0s.tile([128, 2, 6], f32, tag="st")
                    for g in range(2):
                        nc.vector.bn_stats(out=st[:, g, :], in_=xt[:, 384 * g : 384 * (g + 1)])
                    mv = p0s.tile([128, 2], f32, tag="mv")
                    nc.vector.bn_aggr(out=mv[:], in_=st[:])
                    rstd = p0s.tile([128, 1], f32, tag="rstd")
                    nc.scalar.activation(out=rstd[:], in_=mv[:, 1:2], func=FT.Sqrt,
                                         bias=eps_sb[:], scale=1.0)
                    nc.vector.reciprocal(out=rstd[:], in_=rstd[:])
                    ln = p0.tile([128, C], f32, tag="ln")
                    nc.vector.tensor_scalar(out=ln[:], in0=xt[:], scalar1=mv[:, 0:1],
                                            scalar2=rstd[:], op0=OP.subtract, op1=OP.mult)
                    nc.vector.tensor_mul(out=ln[:], in0=ln[:], in1=n1w_b[:])
                    lnh = p0.tile([128, C], fp16, tag="lnh")
                    nc.vector.tensor_add(out=lnh[:], in0=ln[:], in1=n1b_b[:])
                    nc.scalar.dma_start(
                        out=rap(a1i, t0 * BS, [[BS, nrow], [TOKR * BS, N], [1, BS]]),
                        in_=lnh[:nrow].rearrange("p (j c) -> p j c", j=N),
                    )

            nc.gpsimd.collective_compute(
                "AllToAll", OP.bypass, replica_groups=RG, ins=[a1i[:]], outs=[a1o[:]])

            # ---- scale-shift MLPs (overlap with A2A1 window) ----
            ss_ctx = tc.tile_pool(name="ssw", bufs=1)
            ssw = ss_ctx.__enter__()
            ssp_ctx = tc.tile_pool(name="ssp", bufs=1, space="PSUM")
            ssp = ssp_ctx.__enter__()
            modT_sb = ssw.tile([MODD, 2], f32r)
            nc.sync.dma_start(out=modT_sb[:], in_=modT[:].bitcast(f32r))
            fs_w0_sb = ssw.tile([MODD, 2 * C], f32r)
            nc.sync.dma_start(out=fs_w0_sb[:], in_=fs_w0_p[:].bitcast(f32r))
            fs_b0T_sb = ssw.tile([128, 12], f32)
            nc.sync.dma_start(out=fs_b0T_sb[:], in_=fs_b0T_p[:])
            e0T = ssw.tile([128, 12], f32r)
            for j in range(12):
                pt = ssp.tile([128, 2], f32, tag="ss1")
                nc.tensor.matmul(pt[:], fs_w0_sb[:, 128 * j : 128 * (j + 1)],
                                 modT_sb[:], start=True, stop=True)
                nc.scalar.activation(out=e0T[:, j : j + 1], in_=pt[:, 0:1], func=FT.Gelu,
                                     bias=fs_b0T_sb[:, j : j + 1], scale=1.0)
            fs_w1s_sb = ssw.tile([128, 12, 2 * BS], f32r)
            nc.sync.dma_start(
                out=fs_w1s_sb[:],
                in_=rap(fs_w1s_p, 0, [[2 * BS, 128], [128 * 2 * BS, 12], [1, 2 * BS]]).bitcast(f32r),
            )
            fs_b1s_sb = ssw.tile([1, 2 * BS], f32)
            nc.sync.dma_start(out=fs_b1s_sb[:], in_=fs_b1s_p[:])
            p2 = ssp.tile([1, 2 * BS], f32, tag="ss2")
            for j in range(12):
                nc.tensor.matmul(p2[:], e0T[:, j : j + 1], fs_w1s_sb[:, j, :],
                                 start=(j == 0), stop=(j == 11))
            sfo = ssw.tile([1, 2 * BS], f32)
            nc.vector.tensor_add(out=sfo[:], in0=p2[:], in1=fs_b1s_sb[:])
            nc.sync.dma_start(out=sfd[:], in_=sfo[:])
            sfT = ssw.tile([BS, 2], f32)
            nc.sync.dma_start(out=sfT[:], in_=rap(sfd, 0, [[1, BS], [BS, 2]]))
            sfv = ssb.tile([BS, 1], f32)
            nc.vector.tensor_scalar_add(out=sfv[:], in0=sfT[:, 0:1], scalar1=1.0)
            b1r_sb = ssw.tile([BS, 1], f32)
            nc.sync.dma_start(out=b1r_sb[:], in_=b1r_p[:])
            b1i_sb = ssw.tile([BS, 1], f32)
            nc.sync.dma_start(out=b1i_sb[:], in_=b1i_p[:])
            Br = ssb.tile([BS, 1], f32)
            nc.vector.tensor_mul(out=Br[:], in0=b1r_sb[:], in1=sfv[:])
            nc.vector.tensor_add(out=Br[:], in0=Br[:], in1=sfT[:, 1:2])
            Bi = ssb.tile([BS, 1], f32)
            nc.vector.tensor_mul(out=Bi[:], in0=b1i_sb[:], in1=sfv[:])
            nc.vector.tensor_add(out=Bi[:], in0=Bi[:], in1=sfT[:, 1:2])

            # ms MLP: e1T then column-sharded 6144->768, AllGather
            ms_w0_sb = ssw.tile([MODD, LAT2], f32r)
            nc.sync.dma_start(out=ms_w0_sb[:], in_=ms_w0_p[:].bitcast(f32r))
            ms_b0T_sb = ssw.tile([128, 48], f32)
            nc.sync.dma_start(out=ms_b0T_sb[:], in_=ms_b0T_p[:])
            e1T = ssw.tile([128, 48], bf16)
            for j in range(48):
                pt = ssp.tile([128, 2], f32, tag="ss1")
                nc.tensor.matmul(pt[:], ms_w0_sb[:, 128 * j : 128 * (j + 1)],
                                 modT_sb[:], start=True, stop=True)
                nc.scalar.activation(out=e1T[:, j : j + 1], in_=pt[:, 0:1], func=FT.Gelu,
                                     bias=ms_b0T_sb[:, j : j + 1], scale=1.0)
            p3 = ssp.tile([1, 2, 512], f32, tag="ss3")
            with tc.tile_pool(name="msw", bufs=3) as mswp:
                for j in range(48):
                    wt = mswp.tile([128, C], bf16)
                    nc.sync.dma_start(
                        out=wt[:], in_=ms_w1s_p[128 * j : 128 * (j + 1), :])
                    for h2 in range(2):
                        nc.tensor.matmul(
                            p3[:, h2, 0:384], e1T[:, j : j + 1],
                            wt[:, 384 * h2 : 384 * (h2 + 1)],
                            start=(j == 0), stop=(j == 47))
            ms_b1s_sb = ssw.tile([1, C], f32)
            nc.sync.dma_start(out=ms_b1s_sb[:], in_=ms_b1s_p[:])
            mso = ssw.tile([1, C], f32)
            nc.vector.tensor_add(out=mso[:].rearrange("p (a b) -> p a b", a=2),
                                 in0=p3[:, :, 0:384],
                                 in1=ms_b1s_sb[:].rearrange("p (a b) -> p a b", a=2))
            nc.sync.dma_start(out=agi[:], in_=mso[:])
            nc.gpsimd.collective_compute(
                "AllGather", OP.bypass, replica_groups=RG, ins=[agi[:]], outs=[ago[:]])
            sM = ssb.tile([128, 24], f32)
            nc.sync.dma_start(out=sM[:], in_=rap(ago, 0, [[1, 128], [128, 24]]))
            nc.vector.tensor_scalar_add(out=sM[:], in0=sM[:], scalar1=1.0)
            tM = ssb.tile([128, 24], f32)
            nc.sync.dma_start(out=tM[:], in_=rap(ago, LAT, [[1, 128], [128, 24]]))
            fc1bT_sb = ssw.tile([128, 24], f32)
            nc.sync.dma_start(out=fc1bT_sb[:], in_=fc1bT_p[:])
            B1 = ssb.tile([128, 24], f32)
            nc.vector.tensor_mul(out=B1[:], in0=fc1bT_sb[:], in1=sM[:])
            nc.vector.tensor_add(out=B1[:], in0=B1[:], in1=tM[:])
            sM16 = ssb.tile([128, 24], f32)
            nc.vector.tensor_scalar_mul(out=sM16[:], in0=sM[:], scalar1=1.0 / 16.0)

            ssp_ctx.__exit__(None, None, None)
            ss_ctx.__exit__(None, None, None)

            # ---- phase 1 stage A: W-DFT  (X[h,w,c] -> t1d[kw,ri,h,c]) ----
            with (
                tc.tile_pool(name="sa", bufs=1) as sa,
                tc.tile_pool(name="sac", bufs=3) as sac,
                tc.tile_pool(name="sap", bufs=2, space="PSUM") as sap,
            ):
                fw_sb = []
                for ri, p in enumerate([fwr_p, fwi_p]):
                    t = sa.tile([120, 3, KW], fp16, tag=f"fw{ri}")
                    nc.sync.dma_start(
                        out=t[:], in_=rap(p, 0, [[KW, 120], [120 * KW, 3], [1, KW]]))
                    fw_sb.append(t)
                X_sb = sa.tile([120, 3, H, BS], fp16, tag="xsb")
                for ch in range(4):
                    for k in range(3):
                        eng = nc.scalar if (ch + k) % 2 else nc.sync
                        eng.dma_start(
                            out=X_sb[:, k, 45 * ch : 45 * (ch + 1), :],
                            in_=rap(a1o, (45 * ch * W + 120 * k) * BS,
                                    [[BS, 120], [W * BS, 45], [1, BS]]))
                for hs in range(36):
                    hh0 = 5 * hs
                    cp = sac.tile([KW, 2, 5, BS], fp16, tag="cpa")
                    for ri in range(2):
                        ps = sap.tile([KW, 5, BS], f32, tag="pa")
                        for k in range(3):
                            nc.tensor.matmul(ps[:], fw_sb[ri][:, k, :],
                                             X_sb[:, k, hh0 : hh0 + 5, :],
                                             start=(k == 0), stop=(k == 2))
                        nc.vector.tensor_copy(out=cp[:, ri, :, :], in_=ps[:])
                    nc.scalar.dma_start(
                        out=rap(t1d, hh0 * BS,
                                [[2 * H * BS, KW], [H * BS, 2], [BS, 5], [1, BS]]),
                        in_=cp[:])

            # ---- stages B+C+D fused in SBUF, then E ----
            with tc.tile_pool(name="fb", bufs=1) as fb:
                fhs_sb = fb.tile([90, 4, 2 * H], fp16)
                nc.sync.dma_start(
                    out=fhs_sb[:],
                    in_=rap(fhs_p, 0, [[2 * H, 90], [90 * 2 * H, 4], [1, 2 * H]]))
                ifhs_sb = fb.tile([90, 4, 2 * H], fp16)
                nc.sync.dma_start(
                    out=ifhs_sb[:],
                    in_=rap(ifhs_p, 0, [[2 * H, 90], [90 * 2 * H, 4], [1, 2 * H]]))
                w1r_sb = fb.tile([BS, BS], fp16)
                nc.sync.dma_start(out=w1r_sb[:], in_=w1r_p[:])
                w1i_sb = fb.tile([BS, BS], fp16)
                nc.sync.dma_start(out=w1i_sb[:], in_=w1i_p[:])
                w1in_sb = fb.tile([BS, BS], fp16)
                nc.sync.dma_start(out=w1in_sb[:], in_=w1in_p[:])
                w2cr_sb = fb.tile([BS, 2 * BS], fp16)
                nc.sync.dma_start(out=w2cr_sb[:], in_=w2cr_p[:])
                w2ci_sb = fb.tile([BS, 2 * BS], fp16)
                nc.sync.dma_start(out=w2ci_sb[:], in_=w2ci_p[:])
                ifw_sb = []
                for ri, p in enumerate([ifwr_p, ifwi_p]):
                    t = fb.tile([KW, 3, 120], fp16, tag=f"ifw{ri}")
                    nc.sync.dma_start(
                        out=t[:], in_=rap(p, 0, [[W, KW], [120, 3], [1, 120]]))
                    ifw_sb.append(t)
                # T1 resident: [h(90), half, ri, kw, c]
                T1_sb = fb.tile([90, 2, 2, KW, BS], fp16, tag="t1sb")
                for half in range(2):
                    for ri in range(2):
                        eng = nc.scalar if ri else nc.sync
                        eng.dma_start(
                            out=T1_sb[:, half, ri, :, :],
                            in_=rap(t1d, (ri * H + half * 90) * BS,
                                    [[BS, 90], [2 * H * BS, KW], [1, BS]]))
                U_sb = fb.tile([KW, BS, 2 * H], fp16, tag="usb")

                with (
                    tc.tile_pool(name="bcw", bufs=4) as bcw,
                    tc.tile_pool(name="bct", bufs=4) as bct,
                    tc.tile_pool(name="bco", bufs=3) as bco,
                    tc.tile_pool(name="bcp1", bufs=1, space="PSUM") as bcps,
                    tc.tile_pool(name="bcp2", bufs=1, space="PSUM") as bcps2,
                    tc.tile_pool(name="bcp3", bufs=2, space="PSUM") as bcps3,
                    tc.tile_pool(name="bcp4", bufs=1, space="PSUM") as bcps4,
                ):
                    for pr in range(46):
                        kw0 = 2 * pr
                        G = 2 if kw0 + 1 < KW else 1
                        psF = bcps.tile([BS, 2, 512], f32, tag="psF")
                        for g in range(G):
                            kw = kw0 + g
                            for q in range(4):
                                ri, half = q // 2, q % 2
                                nc.tensor.matmul(
                                    psF[:, g, 0 : 2 * H], T1_sb[:, half, ri, kw, :],
                                    fhs_sb[:, q, :], start=(q == 0), stop=(q == 3))
                        fsb = bcw.tile([BS, 2, 2 * H], fp16, tag="fsb")
                        nc.vector.tensor_copy(out=fsb[:, :G, :], in_=psF[:, :G, 0 : 2 * H])
                        ps1r = bcps2.tile([BS, 2, H], f32, tag="ps1r")
                        ps1i = bcps2.tile([BS, 2, H], f32, tag="ps1i")
                        nc.tensor.matmul(ps1r[:, :G, :], w1r_sb[:], fsb[:, :G, 0:H],
                                         start=True, stop=False)
                        nc.tensor.matmul(ps1r[:, :G, :], w1in_sb[:], fsb[:, :G, H : 2 * H],
                                         start=False, stop=True)
                        nc.tensor.matmul(ps1i[:, :G, :], w1i_sb[:], fsb[:, :G, 0:H],
                                         start=True, stop=False)
                        nc.tensor.matmul(ps1i[:, :G, :], w1r_sb[:], fsb[:, :G, H : 2 * H],
                                         start=False, stop=True)
                        o1r = bcw.tile([BS, 2, H], fp16, tag="o1r")
                        o1i = bcw.tile([BS, 2, H], fp16, tag="o1i")
                        nc.scalar.activation(out=o1r[:, :G, :], in_=ps1r[:, :G, :],
                                             func=FT.Relu, bias=Br[:], scale=sfv[:])
                        nc.scalar.activation(out=o1i[:, :G, :], in_=ps1i[:, :G, :],
                                             func=FT.Relu, bias=Bi[:], scale=sfv[:])
                        o1rf = o1r[:].rearrange("p g k -> p (g k)")
                        o1if = o1i[:].rearrange("p g k -> p (g k)")
                        O2t = bco.tile([90, 2, 2, 2 * BS], fp16, tag="o2sb")
                        for g in range(G):
                            ps2 = bcps3.tile([90, 2, 2 * BS], f32, tag="ps2")
                            for half in range(2):
                                sl = slice(90 * (2 * g + half), 90 * (2 * g + half + 1))
                                nc.tensor.matmul(ps2[:, half, :], o1rf[:, sl], w2cr_sb[:],
                                                 start=True, stop=False)
                                nc.tensor.matmul(ps2[:, half, :], o1if[:, sl], w2ci_sb[:],
                                                 start=False, stop=True)
                            tmp = bct.tile([90, 2, 2 * BS], f32, tag="tmp")
                            nc.vector.tensor_add(out=tmp[:], in0=ps2[:], in1=b2c_b[:90])
                            r1 = bct.tile([90, 2, 2 * BS], f32, tag="r1")
                            nc.vector.tensor_scalar(out=r1[:], in0=tmp[:], scalar1=-LAM,
                                                    scalar2=LAM, op0=OP.max, op1=OP.min)
                            nc.vector.tensor_sub(out=O2t[:, g, :, :],
                                                 in0=tmp[:], in1=r1[:])
                        # stage D (inverse H-DFT) fused per kw pair
                        psU = bcps4.tile([BS, 2, 512], f32, tag="psU")
                        for g in range(G):
                            for q in range(4):
                                ri, half = q // 2, q % 2
                                nc.tensor.matmul(
                                    psU[:, g, 0 : 2 * H],
                                    O2t[:, g, half, ri * BS : (ri + 1) * BS],
                                    ifhs_sb[:, q, :], start=(q == 0), stop=(q == 3))
                        ucp = bcw.tile([BS, 2, 2 * H], fp16, tag="ucp")
                        nc.vector.tensor_copy(out=ucp[:, :G, :], in_=psU[:, :G, 0 : 2 * H])
                        nc.sync.dma_start(
                            out=rap(ud, kw0 * BS * 2 * H,
                                    [[2 * H, BS], [BS * 2 * H, G], [1, 2 * H]]),
                            in_=ucp[:, :G, :])
                        # (ud write stays on SP: Act is busy with o1/r1/r2 here)

                # U back to kw-partitioned SBUF, then stage E (inverse W-DFT)
                for chv in range(2):
                    k0 = 46 * chv
                    kn = min(46, KW - k0)
                    eng = nc.scalar if chv else nc.sync
                    eng.dma_start(
                        out=U_sb[k0 : k0 + kn, :, :],
                        in_=rap(ud, k0 * BS * 2 * H,
                                [[BS * 2 * H, kn], [2 * H, BS], [1, 2 * H]]))
                with (
                    tc.tile_pool(name="sec", bufs=4) as sec,
                    tc.tile_pool(name="sep", bufs=2, space="PSUM") as sep,
                ):
                    for wk in range(3):
                        for ht in range(45):
                            h0 = 4 * ht
                            psE = sep.tile([120, 4, BS], f32, tag="psE")
                            for ri in range(2):
                                nc.tensor.matmul(
                                    psE[:], ifw_sb[ri][:, wk, :],
                                    U_sb[:, :, ri * H + h0 : ri * H + h0 + 4]
                                        .rearrange("p c h -> p h c"),
                                    start=(ri == 0), stop=(ri == 1))
                            ecp = sec.tile([120, 4, BS], fp16, tag="ecp")
                            nc.vector.tensor_copy(out=ecp[:], in_=psE[:])
                            nc.scalar.dma_start(
                                out=rap(a2i, h0 * W * BS + wk * 120 * BS,
                                        [[BS, 120], [W * BS, 4], [1, BS]]),
                                in_=ecp[:])

            nc.gpsimd.collective_compute(
                "AllToAll", OP.bypass, replica_groups=RG, ins=[a2i[:]], outs=[a2o[:]])

            # ---- phase 2: h1 = F2 + ln1x + x; LN2; modulated MLP; + h1 ----
            with (
                tc.tile_pool(name="p2w", bufs=1) as p2w,
                tc.tile_pool(name="p2", bufs=4) as p2,
                tc.tile_pool(name="p2h", bufs=4) as p2h,
                tc.tile_pool(name="p2s", bufs=4) as p2s,
                tc.tile_pool(name="p2m", bufs=2) as p2m,
                tc.tile_pool(name="ptp", bufs=2, space="PSUM") as ptp,
                tc.tile_pool(name="php", bufs=2, space="PSUM") as php,
                tc.tile_pool(name="pop", bufs=2, space="PSUM") as pop,
            ):
                PM = mybir.MatmulPerfMode.DoubleRow
                fc1w_sb = p2w.tile([128, 6, LAT], fp8)
                nc.sync.dma_start(
                    out=fc1w_sb[:], in_=rap(fc1w_p, 0, [[LAT, 128], [128 * LAT, 6], [1, LAT]]))
                fc2w_sb = p2w.tile([128, 24, C], fp8)
                nc.sync.dma_start(
                    out=fc2w_sb[:], in_=rap(fc2w_p, 0, [[C, 128], [128 * C, 24], [1, C]]))

                def p2_prep(it):
                    T0 = it * 256
                    ln2T = p2m.tile([128, 6, 2, 128], fp8, tag="ln2T")
                    h1s = []
                    for hf in range(2):
                        t0 = T0 + 128 * hf
                        nload = max(0, min(128, TOKR - t0))
                        xt = p2.tile([128, C], f32, tag="xt2")
                        nc.sync.dma_start(out=xt[:], in_=xs[t0 : t0 + 128, :])
                        f2t = p2.tile([128, N, BS], fp16, tag="f2t")
                        l1t = p2.tile([128, N, BS], fp16, tag="l1t")
                        if nload < 128:
                            nc.vector.memset(f2t[:], 0.0)
                            nc.vector.memset(l1t[:], 0.0)
                        if nload > 0:
                            nc.sync.dma_start(
                                out=f2t[:nload],
                                in_=rap(a2o, t0 * BS, [[BS, nload], [TOKR * BS, N], [1, BS]]))
                            nc.sync.dma_start(
                                out=l1t[:nload],
                                in_=rap(a1i, t0 * BS, [[BS, nload], [TOKR * BS, N], [1, BS]]))
                        h1 = p2h.tile([128, C], f32, tag="h1")
                        nc.vector.tensor_add(out=h1[:], in0=xt[:],
                                             in1=f2t[:].rearrange("p j c -> p (j c)"))
                        nc.vector.tensor_add(out=h1[:], in0=h1[:],
                                             in1=l1t[:].rearrange("p j c -> p (j c)"))
                        h1s.append(h1)
                        st = p2s.tile([128, 2, 6], f32, tag="st2")
                        for g in range(2):
                            nc.vector.bn_stats(out=st[:, g, :], in_=h1[:, 384 * g : 384 * (g + 1)])
                        mv = p2s.tile([128, 2], f32, tag="mv2")
                        nc.vector.bn_aggr(out=mv[:], in_=st[:])
                        rstd = p2s.tile([128, 1], f32, tag="rstd2")
                        nc.scalar.activation(out=rstd[:], in_=mv[:, 1:2], func=FT.Sqrt,
                                             bias=eps_sb[:], scale=1.0)
                        nc.vector.reciprocal(out=rstd[:], in_=rstd[:])
                        ln2 = p2.tile([128, C], bf16, tag="ln2")
                        nc.vector.tensor_scalar(out=ln2[:], in0=h1[:], scalar1=mv[:, 0:1],
                                                scalar2=rstd[:], op0=OP.subtract, op1=OP.mult)
                        nc.vector.tensor_mul(out=ln2[:], in0=ln2[:], in1=n2w_b[:])
                        nc.vector.tensor_add(out=ln2[:], in0=ln2[:], in1=n2b_b[:])
                        for jb in range(2):
                            pst = ptp.tile([128, 3, 128], bf16, tag="pst")
                            for jj in range(3):
                                j = 3 * jb + jj
                                nc.tensor.transpose(pst[:, jj, :],
                                                    ln2[:, 128 * j : 128 * (j + 1)],
                                                    identb[:])
                            nc.vector.tensor_copy(out=ln2T[:, 3 * jb : 3 * jb + 3, hf, :],
                                                  in_=pst[:])
                    return T0, ln2T, h1s

                def p2_mm(T0, ln2T, h1s):
                    hmidT = p2m.tile([128, 24, 256], fp8, tag="hmidT")
                    for l in range(24):
                        psH = php.tile([128, 256], f32, tag="psH")
                        for jp in range(3):
                            nc.tensor.matmul(
                                psH[:],
                                fc1w_sb[:, 2 * jp : 2 * jp + 2, 128 * l : 128 * (l + 1)],
                                ln2T[:, 2 * jp : 2 * jp + 2, :, :]
                                    .rearrange("p j h t -> p j (h t)"),
                                start=(jp == 0), stop=(jp == 2), perf_mode=PM)
                        nc.scalar.activation(out=hmidT[:, l, :], in_=psH[:], func=FT.Gelu,
                                             bias=B1[:, l : l + 1], scale=sM16[:, l : l + 1])
                    for hf in range(2):
                        t0 = T0 + 128 * hf
                        psO = pop.tile([128, 2, 512], f32, tag="psO")
                        for lp in range(12):
                            for h2 in range(2):
                                nc.tensor.matmul(
                                    psO[:, h2, 0:384],
                                    hmidT[:, 2 * lp : 2 * lp + 2, 128 * hf : 128 * (hf + 1)],
                                    fc2w_sb[:, 2 * lp : 2 * lp + 2, 384 * h2 : 384 * (h2 + 1)],
                                    start=(lp == 0), stop=(lp == 11), perf_mode=PM)
                        ot = p2.tile([128, C], f32, tag="ot")
                        nc.vector.scalar_tensor_tensor(
                            out=ot[:].rearrange("p (a b) -> p a b", a=2),
                            in0=psO[:, :, 0:384], scalar=1.0 / 16.0,
                            in1=h1s[hf][:].rearrange("p (a b) -> p a b", a=2),
                            op0=OP.mult, op1=OP.add)
                        nc.vector.tensor_add(out=ot[:], in0=ot[:], in1=fc2b_b[:])
                        nc.sync.dma_start(out=out_p[t0 : t0 + 128, :], in_=ot[:])

                # pairs of token tiles: LN2 (Sqrt) for both, then MLP (Gelu)
                # for both — halves Act table swaps
                for itp in range(0, NT2, 2):
                    states = [p2_prep(it) for it in range(itp, min(itp + 2, NT2))]
                    for s in states:
                        p2_mm(*s)

    nc.compile()
    return nc


_NC = None


def _get_nc():
    global _NC
    if _NC is None:
        _NC = _build()
    return _NC


def _dft_mats():
    w = np.arange(W); kw = np.arange(KW)
    ang = 2 * np.pi * np.outer(w, kw) / W
    fwr = (np.cos(ang) / np.sqrt(W)).astype(np.float32)
    fwi = (-np.sin(ang) / np.sqrt(W)).astype(np.float32)
    kh = np.arange(H); h = np.arange(H)
    angh = 2 * np.pi * np.outer(kh, h) / H        # [kh, h]
    fhr = np.cos(angh) / np.sqrt(H)
    fhi = -np.sin(angh) / np.sqrt(H)
    fhs = np.zeros((2 * H, 2 * H))
    fhs[:H, :H] = fhr.T; fhs[:H, H:] = fhi.T
    fhs[H:, :H] = -fhi.T; fhs[H:, H:] = fhr.T
    ci = np.cos(angh) / np.sqrt(H)                # [kh, h] for inverse
    si = np.sin(angh) / np.sqrt(H)
    ifhs = np.zeros((2 * H, 2 * H))
    ifhs[:H, :H] = ci; ifhs[:H, H:] = si
    ifhs[H:, :H] = -si; ifhs[H:, H:] = ci
    ckw = np.where(kw == 0, 1.0, 2.0)
    angi = 2 * np.pi * np.outer(kw, np.arange(W)) / W    # [kw, w]
    ifwr = (ckw[:, None] * np.cos(angi) / np.sqrt(W)).astype(np.float32)
    ifwi = (-ckw[:, None] * np.sin(angi) / np.sqrt(W)).astype(np.float32)
    return fwr, fwi, fhs.astype(np.float32), ifhs.astype(np.float32), ifwr, ifwi


def _prepare_in_maps(x, mod_embed, n1w, n1b, n2w, n2b, w1, b1, w2, b2,
                     fs_w0, fs_b0, fs_w1, fs_b1, fc1w, fc1b, fc2w, fc2b,
                     ms_w0, ms_b0, ms_w1, ms_b1):
    f = np.asarray
    x = f(x, dtype=np.float32)
    grid = x.reshape(H, W, C)
    fwr, fwi, fhs, ifhs, ifwr, ifwi = _dft_mats()
    bf = ml_dtypes.bfloat16

    in_maps = []
    for b in range(N):
        r0, r1 = HP * b, min(HP * (b + 1), H)
        xsb = np.zeros((TOKP, C), np.float32)
        xsb[: (r1 - r0) * W] = grid[r0:r1].reshape(-1, C)
        sl = slice(BS * b, BS * (b + 1))
        w2r = f(w2[0, b], np.float32); w2i = f(w2[1, b], np.float32)
        im = {
            "xs": xsb,
            "modT": np.repeat(f(mod_embed, np.float32).reshape(MODD, 1), 2, axis=1).copy(),
            "n1w": f(n1w, np.float32), "n1b": f(n1b, np.float32),
            "n2w": f(n2w, np.float32), "n2b": f(n2b, np.float32),
            "fwr": fwr.astype(np.float16), "fwi": fwi.astype(np.float16),
            "fhs": fhs.astype(np.float16), "ifhs": ifhs.astype(np.float16),
            "ifwr": ifwr.astype(np.float16), "ifwi": ifwi.astype(np.float16),
            "w1r": f(w1[0, b], np.float16).copy(),
            "w1i": f(w1[1, b], np.float16).copy(),
            "w1in": (-f(w1[1, b], np.float16)).copy(),
            "w2cr": np.concatenate([w2r, w2i], axis=1).astype(np.float16),
            "w2ci": np.concatenate([-w2i, w2r], axis=1).astype(np.float16),
            "b1r": f(b1[0, b], np.float32).reshape(BS, 1).copy(),
            "b1i": f(b1[1, b], np.float32).reshape(BS, 1).copy(),
            "b2c": np.concatenate([f(b2[0, b], np.float32), f(b2[1, b], np.float32)]),
            "fs_w0": f(fs_w0, np.float32),
            "fs_b0T": f(fs_b0, np.float32).reshape(12, 128).T.copy(),
            "fs_w1s": np.concatenate(
                [f(fs_w1, np.float32)[:, sl], f(fs_w1, np.float32)[:, C + BS * b : C + BS * (b + 1)]],
                axis=1),
            "fs_b1s": np.concatenate(
                [f(fs_b1, np.float32)[sl], f(fs_b1, np.float32)[C + BS * b : C + BS * (b + 1)]]
            ).reshape(1, -1),
            "ms_w0": f(ms_w0, np.float32),
            "ms_b0T": f(ms_b0, np.float32).reshape(48, 128).T.copy(),
            "ms_w1s": f(ms_w1, np.float32)[:, C * b : C * (b + 1)].astype(bf),
            "ms_b1s": f(ms_b1, np.float32)[C * b : C * (b + 1)].reshape(1, -1),
            "fc1w": (16.0 * f(fc1w, np.float32)).astype(ml_dtypes.float8_e4m3),
            "fc1bT": f(fc1b, np.float32).reshape(24, 128).T.copy(),
            "fc2w": (16.0 * f(fc2w, np.float32)).astype(ml_dtypes.float8_e4m3),
            "fc2b": f(fc2b, np.float32),
        }
        in_maps.append(im)
    return in_maps


def kernel(x, mod_embed, n1w, n1b, n2w, n2b, w1, b1, w2, b2,
           fs_w0, fs_b0, fs_w1, fs_b1, fc1w, fc1b, fc2w, fc2b,
           ms_w0, ms_b0, ms_w1, ms_b1):
    nc = _get_nc()
    in_maps = _prepare_in_maps(
        x, mod_embed, n1w, n1b, n2w, n2b, w1, b1, w2, b2,
        fs_w0, fs_b0, fs_w1, fs_b1, fc1w, fc1b, fc2w, fc2b,
        ms_w0, ms_b0, ms_w1, ms_b1)

    res = run_bass_kernel_spmd(nc, in_maps, core_ids=list(range(N)))
    globals()["last_results"] = res
    out = np.zeros((H, W, C), np.float32)
    for b in range(N):
        r0, r1 = HP * b, min(HP * (b + 1), H)
        out[r0:r1] = res.results[b]["out"][: (r1 - r0) * W].reshape(r1 - r0, W, C)
    return out.reshape(1, H, W, C)



# revision 23
# speedup vs baseline: 1.1143x; 1.0362x over previous
"""AFNO block kernel for 8 Trainium2 NeuronCores.

Sharding: token-shard (H rows, 23 per core padded) for LN/MLP phases;
AllToAll to channel-shard (core i = spectral block i, 96 channels) for the
2D-FFT filter, computed as matmuls against precomputed DFT matrices;
AllToAll back; small AllGather for the column-sharded 6144x6144 scale-shift
MLP weight.
"""

import os
import numpy as np
import ml_dtypes

import concourse.bass as bass
import concourse.bacc as bacc
import concourse.mybir as mybir
import concourse.tile as tile
from concourse.bass_utils import run_bass_kernel_spmd
from concourse.masks import make_identity

f32 = mybir.dt.float32
f32r = mybir.dt.float32r
bf16 = mybir.dt.bfloat16
fp16 = mybir.dt.float16
fp8 = mybir.dt.float8e4
FT = mybir.ActivationFunctionType
OP = mybir.AluOpType

H, W, C = 180, 360, 768
NB, BS, KW = 8, 96, 91
HP = 23                 # rows per shard (8*23 = 184 >= 180)
TOKR = HP * W           # 8280 real token slots per shard
NT2 = 17                # phase-2 tiles of 512
TOKP = NT2 * 512        # 8704 padded tokens per shard
MODD, LAT, LAT2 = 64, 3072, 6144
LAM = 0.01
EPS = 1e-5
N = 8


def rap(t, offset, dims):
    a = t[:] if not isinstance(t, bass.AP) else t
    return bass.AP(tensor=a.tensor, offset=a.offset + offset, ap=[list(d) for d in dims])


def _build():
    nc = bacc.Bacc("TRN2", target_bir_lowering=False, debug=False, num_devices=N)

    def P(name, shp, dt=f32):
        return nc.declare_dram_parameter(name, list(shp), dt, isOutput=False)

    xs = P("xs", [TOKP, C])
    modT = P("modT", [MODD, 2])
    n1w = P("n1w", [C]); n1b = P("n1b", [C])
    n2w = P("n2w", [C]); n2b = P("n2b", [C])
    fwr_p = P("fwr", [W, KW], fp16); fwi_p = P("fwi", [W, KW], fp16)
    fhs_p = P("fhs", [2 * H, 2 * H], fp16)
    ifhs_p = P("ifhs", [2 * H, 2 * H], fp16)
    ifwr_p = P("ifwr", [KW, W], fp16); ifwi_p = P("ifwi", [KW, W], fp16)
    w1r_p = P("w1r", [BS, BS], fp16); w1i_p = P("w1i", [BS, BS], fp16)
    w1in_p = P("w1in", [BS, BS], fp16)
    w2cr_p = P("w2cr", [BS, 2 * BS], fp16)   # [W2r | W2i]
    w2ci_p = P("w2ci", [BS, 2 * BS], fp16)   # [-W2i | W2r]
    b1r_p = P("b1r", [BS, 1]); b1i_p = P("b1i", [BS, 1])
    b2c_p = P("b2c", [2 * BS])               # concat(b2r, b2i)
    fs_w0_p = P("fs_w0", [MODD, 2 * C])
    fs_b0T_p = P("fs_b0T", [128, 12])
    fs_w1s_p = P("fs_w1s", [2 * C, 2 * BS])
    fs_b1s_p = P("fs_b1s", [1, 2 * BS])
    ms_w0_p = P("ms_w0", [MODD, LAT2])
    ms_b0T_p = P("ms_b0T", [128, 48])
    ms_w1s_p = P("ms_w1s", [LAT2, C], bf16)
    ms_b1s_p = P("ms_b1s", [1, C])
    fc1w_p = P("fc1w", [C, LAT], fp8)
    fc1bT_p = P("fc1bT", [128, 24])
    fc2w_p = P("fc2w", [LAT, C], fp8)
    fc2b_p = P("fc2b", [C])
    out_p = nc.declare_dram_parameter("out", [TOKP, C], f32, isOutput=True)
    DBG = False

    # internal DRAM
    a1i = nc.dram_tensor("a1i", [N, TOKR * BS], fp16)
    a1o = nc.dram_tensor("a1o", [N, TOKR * BS], fp16)
    MA = 12 * W * BS        # chunk A: local rows 0-11 per dest
    MB = 11 * W * BS        # chunk B: local rows 12-22
    TA = 12 * W             # tokens per dest covered by chunk A
    a2iA = nc.dram_tensor("a2iA", [N, MA], fp16)
    a2oA = nc.dram_tensor("a2oA", [N, MA], fp16)
    a2iB = nc.dram_tensor("a2iB", [N, MB], fp16)
    a2oB = nc.dram_tensor("a2oB", [N, MB], fp16)
    t1d = nc.dram_tensor("t1d", [KW, 2, H, BS], fp16)   # [kw][ri][h][c]
    ud = nc.dram_tensor("ud", [KW, BS, 2 * H], fp16)
    sfd = nc.dram_tensor("sfd", [1, 2 * BS], f32)
    agi = nc.dram_tensor("agi", [1, C], f32)
    ago = nc.dram_tensor("ago", [N, C], f32)

    RG = [list(range(N))]

    with tile.TileContext(nc) as tc:
        with (
            tc.tile_pool(name="const", bufs=1) as cpool,
            tc.tile_pool(name="ssb", bufs=1) as ssb,
        ):
            # ---- broadcast constants ----
            def bcast(p, n, name):
                t = cpool.tile([128, n], f32, tag=name)
                nc.sync.dma_start(out=t[:], in_=rap(p, 0, [[0, 128], [1, n]]))
                return t

            n1w_b = bcast(n1w, C, "n1w"); n1b_b = bcast(n1b, C, "n1b")
            n2w_b = bcast(n2w, C, "n2w"); n2b_b = bcast(n2b, C, "n2b")
            fc2b_b = bcast(fc2b_p, C, "fc2b")
            b2c_b = cpool.tile([128, 2, 2 * BS], f32, tag="b2c")
            for bh in range(2):
                nc.sync.dma_start(out=b2c_b[:, bh, :],
                                  in_=rap(b2c_p, 0, [[0, 128], [1, 2 * BS]]))
            eps_sb = cpool.tile([128, 1], f32, tag="eps")
            nc.vector.memset(eps_sb[:], EPS)
            nlam_sb = cpool.tile([128, 1], f32, tag="nlam")
            nc.vector.memset(nlam_sb[:], -LAM)
            ident = cpool.tile([128, 128], f32, tag="ident")
            make_identity(nc, ident[:])
            identb = cpool.tile([128, 128], bf16, tag="identb")
            nc.vector.tensor_copy(out=identb[:], in_=ident[:])


            # ---- phase 0: LN1 + scatter into A2A-1 send buffer ----
            with (
                tc.tile_pool(name="p0", bufs=6) as p0,
                tc.tile_pool(name="p0s", bufs=8) as p0s,
            ):
                for it in range(65):
                    t0 = it * 128
                    nrow = min(128, TOKR - t0)
                    xt = p0.tile([128, C], f32, tag="xt")
                    nc.sync.dma_start(out=xt[:], in_=xs[t0 : t0 + 128, :])
                    st = p0s.tile([128, 2, 6], f32, tag="st")
                    for g in range(2):
                        nc.vector.bn_stats(out=st[:, g, :], in_=xt[:, 384 * g : 384 * (g + 1)])
                    mv = p0s.tile([128, 2], f32, tag="mv")
                    nc.vector.bn_aggr(out=mv[:], in_=st[:])
                    rstd = p0s.tile([128, 1], f32, tag="rstd")
                    nc.scalar.activation(out=rstd[:], in_=mv[:, 1:2], func=FT.Sqrt,
                                         bias=eps_sb[:], scale=1.0)
                    nc.vector.reciprocal(out=rstd[:], in_=rstd[:])
                    ln = p0.tile([128, C], f32, tag="ln")
                    nc.vector.tensor_scalar(out=ln[:], in0=xt[:], scalar1=mv[:, 0:1],
                                            scalar2=rstd[:], op0=OP.subtract, op1=OP.mult)
                    nc.vector.tensor_mul(out=ln[:], in0=ln[:], in1=n1w_b[:])
                    lnh = p0.tile([128, C], fp16, tag="lnh")
                    nc.vector.tensor_add(out=lnh[:], in0=ln[:], in1=n1b_b[:])
                    nc.scalar.dma_start(
                        out=rap(a1i, t0 * BS, [[BS, nrow], [TOKR * BS, N], [1, BS]]),
                        in_=lnh[:nrow].rearrange("p (j c) -> p j c", j=N),
                    )

            nc.gpsimd.collective_compute(
                "AllToAll", OP.bypass, replica_groups=RG, ins=[a1i[:]], outs=[a1o[:]])

            # ---- scale-shift MLPs (overlap with A2A1 window) ----
            ss_ctx = tc.tile_pool(name="ssw", bufs=1)
            ssw = ss_ctx.__enter__()
            ssp_ctx = tc.tile_pool(name="ssp", bufs=1, space="PSUM")
            ssp = ssp_ctx.__enter__()
            modT_sb = ssw.tile([MODD, 2], f32r)
            nc.sync.dma_start(out=modT_sb[:], in_=modT[:].bitcast(f32r))
            fs_w0_sb = ssw.tile([MODD, 2 * C], f32r)
            nc.sync.dma_start(out=fs_w0_sb[:], in_=fs_w0_p[:].bitcast(f32r))
            fs_b0T_sb = ssw.tile([128, 12], f32)
            nc.sync.dma_start(out=fs_b0T_sb[:], in_=fs_b0T_p[:])
            e0T = ssw.tile([128, 12], f32r)
            for j in range(12):
                pt = ssp.tile([128, 2], f32, tag="ss1")
                nc.tensor.matmul(pt[:], fs_w0_sb[:, 128 * j : 128 * (j + 1)],
                                 modT_sb[:], start=True, stop=True)
                nc.scalar.activation(out=e0T[:, j : j + 1], in_=pt[:, 0:1], func=FT.Gelu,
                                     bias=fs_b0T_sb[:, j : j + 1], scale=1.0)
            fs_w1s_sb = ssw.tile([128, 12, 2 * BS], f32r)
            nc.sync.dma_start(
                out=fs_w1s_sb[:],
                in_=rap(fs_w1s_p, 0, [[2 * BS, 128], [128 * 2 * BS, 12], [1, 2 * BS]]).bitcast(f32r),
            )
            fs_b1s_sb = ssw.tile([1, 2 * BS], f32)
            nc.sync.dma_start(out=fs_b1s_sb[:], in_=fs_b1s_p[:])
            p2 = ssp.tile([1, 2 * BS], f32, tag="ss2")
            for j in range(12):
                nc.tensor.matmul(p2[:], e0T[:, j : j + 1], fs_w1s_sb[:, j, :],
                                 start=(j == 0), stop=(j == 11))
            sfo = ssw.tile([1, 2 * BS], f32)
            nc.vector.tensor_add(out=sfo[:], in0=p2[:], in1=fs_b1s_sb[:])
            nc.sync.dma_start(out=sfd[:], in_=sfo[:])
            sfT = ssw.tile([BS, 2], f32)
            nc.sync.dma_start(out=sfT[:], in_=rap(sfd, 0, [[1, BS], [BS, 2]]))
            sfv = ssb.tile([BS, 1], f32)
            nc.vector.tensor_scalar_add(out=sfv[:], in0=sfT[:, 0:1], scalar1=1.0)
            b1r_sb = ssw.tile([BS, 1], f32)
            nc.sync.dma_start(out=b1r_sb[:], in_=b1r_p[:])
            b1i_sb = ssw.tile([BS, 1], f32)
            nc.sync.dma_start(out=b1i_sb[:], in_=b1i_p[:])
            Br = ssb.tile([BS, 1], f32)
            nc.vector.tensor_mul(out=Br[:], in0=b1r_sb[:], in1=sfv[:])
            nc.vector.tensor_add(out=Br[:], in0=Br[:], in1=sfT[:, 1:2])
            Bi = ssb.tile([BS, 1], f32)
            nc.vector.tensor_mul(out=Bi[:], in0=b1i_sb[:], in1=sfv[:])
            nc.vector.tensor_add(out=Bi[:], in0=Bi[:], in1=sfT[:, 1:2])

            # ms MLP: e1T then column-sharded 6144->768, AllGather
            ms_w0_sb = ssw.tile([MODD, LAT2], f32r)
            nc.gpsimd.dma_start(out=ms_w0_sb[:], in_=ms_w0_p[:].bitcast(f32r))
            ms_b0T_sb = ssw.tile([128, 48], f32)
            nc.sync.dma_start(out=ms_b0T_sb[:], in_=ms_b0T_p[:])
            e1T = ssw.tile([128, 48], bf16)
            for j in range(48):
                pt = ssp.tile([128, 2], f32, tag="ss1")
                nc.tensor.matmul(pt[:], ms_w0_sb[:, 128 * j : 128 * (j + 1)],
                                 modT_sb[:], start=True, stop=True)
                nc.scalar.activation(out=e1T[:, j : j + 1], in_=pt[:, 0:1], func=FT.Gelu,
                                     bias=ms_b0T_sb[:, j : j + 1], scale=1.0)
            p3 = ssp.tile([1, 2, 512], f32, tag="ss3")
            with tc.tile_pool(name="msw", bufs=3) as mswp:
                for j in range(48):
                    wt = mswp.tile([128, C], bf16)
                    nc.gpsimd.dma_start(
                        out=wt[:], in_=ms_w1s_p[128 * j : 128 * (j + 1), :])
                    for h2 in range(2):
                        nc.tensor.matmul(
                            p3[:, h2, 0:384], e1T[:, j : j + 1],
                            wt[:, 384 * h2 : 384 * (h2 + 1)],
                            start=(j == 0), stop=(j == 47))
            ms_b1s_sb = ssw.tile([1, C], f32)
            nc.sync.dma_start(out=ms_b1s_sb[:], in_=ms_b1s_p[:])
            mso = ssw.tile([1, C], f32)
            nc.vector.tensor_add(out=mso[:].rearrange("p (a b) -> p a b", a=2),
                                 in0=p3[:, :, 0:384],
                                 in1=ms_b1s_sb[:].rearrange("p (a b) -> p a b", a=2))
            nc.sync.dma_start(out=agi[:], in_=mso[:])
            nc.gpsimd.collective_compute(
                "AllGather", OP.bypass, replica_groups=RG, ins=[agi[:]], outs=[ago[:]])
            sM = ssb.tile([128, 24], f32)
            nc.sync.dma_start(out=sM[:], in_=rap(ago, 0, [[1, 128], [128, 24]]))
            nc.vector.tensor_scalar_add(out=sM[:], in0=sM[:], scalar1=1.0)
            tM = ssb.tile([128, 24], f32)
            nc.sync.dma_start(out=tM[:], in_=rap(ago, LAT, [[1, 128], [128, 24]]))
            fc1bT_sb = ssw.tile([128, 24], f32)
            nc.sync.dma_start(out=fc1bT_sb[:], in_=fc1bT_p[:])
            B1 = ssb.tile([128, 24], f32)
            nc.vector.tensor_mul(out=B1[:], in0=fc1bT_sb[:], in1=sM[:])
            nc.vector.tensor_add(out=B1[:], in0=B1[:], in1=tM[:])
            sM16 = ssb.tile([128, 24], f32)
            nc.vector.tensor_scalar_mul(out=sM16[:], in0=sM[:], scalar1=1.0 / 16.0)

            ssp_ctx.__exit__(None, None, None)
            ss_ctx.__exit__(None, None, None)

            # ---- phase 1 stage A: W-DFT  (X[h,w,c] -> t1d[kw,ri,h,c]) ----
            with (
                tc.tile_pool(name="sa", bufs=1) as sa,
                tc.tile_pool(name="sac", bufs=3) as sac,
                tc.tile_pool(name="sap", bufs=2, space="PSUM") as sap,
            ):
                fw_sb = []
                for ri, p in enumerate([fwr_p, fwi_p]):
                    t = sa.tile([120, 3, KW], fp16, tag=f"fw{ri}")
                    nc.sync.dma_start(
                        out=t[:], in_=rap(p, 0, [[KW, 120], [120 * KW, 3], [1, KW]]))
                    fw_sb.append(t)
                X_sb = sa.tile([120, 3, H, BS], fp16, tag="xsb")
                for ch in range(4):
                    for k in range(3):
                        eng = nc.scalar if (ch + k) % 2 else nc.sync
                        eng.dma_start(
                            out=X_sb[:, k, 45 * ch : 45 * (ch + 1), :],
                            in_=rap(a1o, (45 * ch * W + 120 * k) * BS,
                                    [[BS, 120], [W * BS, 45], [1, BS]]))
                for hs in range(36):
                    hh0 = 5 * hs
                    cp = sac.tile([KW, 2, 5, BS], fp16, tag="cpa")
                    for ri in range(2):
                        ps = sap.tile([KW, 5, BS], f32, tag="pa")
                        for k in range(3):
                            nc.tensor.matmul(ps[:], fw_sb[ri][:, k, :],
                                             X_sb[:, k, hh0 : hh0 + 5, :],
                                             start=(k == 0), stop=(k == 2))
                        nc.vector.tensor_copy(out=cp[:, ri, :, :], in_=ps[:])
                    nc.scalar.dma_start(
                        out=rap(t1d, hh0 * BS,
                                [[2 * H * BS, KW], [H * BS, 2], [BS, 5], [1, BS]]),
                        in_=cp[:])

            # ---- stages B+C+D fused in SBUF, then E ----
            with tc.tile_pool(name="fb", bufs=1) as fb:
                fhs_sb = fb.tile([90, 4, 2 * H], fp16)
                nc.sync.dma_start(
                    out=fhs_sb[:],
                    in_=rap(fhs_p, 0, [[2 * H, 90], [90 * 2 * H, 4], [1, 2 * H]]))
                ifhs_sb = fb.tile([90, 4, 2 * H], fp16)
                nc.sync.dma_start(
                    out=ifhs_sb[:],
                    in_=rap(ifhs_p, 0, [[2 * H, 90], [90 * 2 * H, 4], [1, 2 * H]]))
                w1r_sb = fb.tile([BS, BS], fp16)
                nc.sync.dma_start(out=w1r_sb[:], in_=w1r_p[:])
                w1i_sb = fb.tile([BS, BS], fp16)
                nc.sync.dma_start(out=w1i_sb[:], in_=w1i_p[:])
                w1in_sb = fb.tile([BS, BS], fp16)
                nc.sync.dma_start(out=w1in_sb[:], in_=w1in_p[:])
                w2cr_sb = fb.tile([BS, 2 * BS], fp16)
                nc.sync.dma_start(out=w2cr_sb[:], in_=w2cr_p[:])
                w2ci_sb = fb.tile([BS, 2 * BS], fp16)
                nc.sync.dma_start(out=w2ci_sb[:], in_=w2ci_p[:])
                ifw_sb = []
                for ri, p in enumerate([ifwr_p, ifwi_p]):
                    t = fb.tile([KW, 3, 120], fp16, tag=f"ifw{ri}")
                    nc.sync.dma_start(
                        out=t[:], in_=rap(p, 0, [[W, KW], [120, 3], [1, 120]]))
                    ifw_sb.append(t)
                # T1 resident: [h(90), half, ri, kw, c]
                T1_sb = fb.tile([90, 2, 2, KW, BS], fp16, tag="t1sb")
                for half in range(2):
                    for ri in range(2):
                        eng = nc.scalar if ri else nc.sync
                        eng.dma_start(
                            out=T1_sb[:, half, ri, :, :],
                            in_=rap(t1d, (ri * H + half * 90) * BS,
                                    [[BS, 90], [2 * H * BS, KW], [1, BS]]))
                U_sb = fb.tile([KW, BS, 2 * H], fp16, tag="usb")

                with (
                    tc.tile_pool(name="bcw", bufs=4) as bcw,
                    tc.tile_pool(name="bct", bufs=4) as bct,
                    tc.tile_pool(name="bco", bufs=3) as bco,
                    tc.tile_pool(name="bcp1", bufs=1, space="PSUM") as bcps,
                    tc.tile_pool(name="bcp2", bufs=1, space="PSUM") as bcps2,
                    tc.tile_pool(name="bcp3", bufs=2, space="PSUM") as bcps3,
                    tc.tile_pool(name="bcp4", bufs=1, space="PSUM") as bcps4,
                ):
                    for pr in range(46):
                        kw0 = 2 * pr
                        G = 2 if kw0 + 1 < KW else 1
                        psF = bcps.tile([BS, 2, 512], f32, tag="psF")
                        for g in range(G):
                            kw = kw0 + g
                            for q in range(4):
                                ri, half = q // 2, q % 2
                                nc.tensor.matmul(
                                    psF[:, g, 0 : 2 * H], T1_sb[:, half, ri, kw, :],
                                    fhs_sb[:, q, :], start=(q == 0), stop=(q == 3))
                        fsb = bcw.tile([BS, 2, 2 * H], fp16, tag="fsb")
                        nc.vector.tensor_copy(out=fsb[:, :G, :], in_=psF[:, :G, 0 : 2 * H])
                        ps1r = bcps2.tile([BS, 2, H], f32, tag="ps1r")
                        ps1i = bcps2.tile([BS, 2, H], f32, tag="ps1i")
                        nc.tensor.matmul(ps1r[:, :G, :], w1r_sb[:], fsb[:, :G, 0:H],
                                         start=True, stop=False)
                        nc.tensor.matmul(ps1r[:, :G, :], w1in_sb[:], fsb[:, :G, H : 2 * H],
                                         start=False, stop=True)
                        nc.tensor.matmul(ps1i[:, :G, :], w1i_sb[:], fsb[:, :G, 0:H],
                                         start=True, stop=False)
                        nc.tensor.matmul(ps1i[:, :G, :], w1r_sb[:], fsb[:, :G, H : 2 * H],
                                         start=False, stop=True)
                        o1r = bcw.tile([BS, 2, H], fp16, tag="o1r")
                        o1i = bcw.tile([BS, 2, H], fp16, tag="o1i")
                        nc.scalar.activation(out=o1r[:, :G, :], in_=ps1r[:, :G, :],
                                             func=FT.Relu, bias=Br[:], scale=sfv[:])
                        nc.scalar.activation(out=o1i[:, :G, :], in_=ps1i[:, :G, :],
                                             func=FT.Relu, bias=Bi[:], scale=sfv[:])
                        o1rf = o1r[:].rearrange("p g k -> p (g k)")
                        o1if = o1i[:].rearrange("p g k -> p (g k)")
                        O2t = bco.tile([90, 2, 2, 2 * BS], fp16, tag="o2sb")
                        for g in range(G):
                            ps2 = bcps3.tile([90, 2, 2 * BS], f32, tag="ps2")
                            for half in range(2):
                                sl = slice(90 * (2 * g + half), 90 * (2 * g + half + 1))
                                nc.tensor.matmul(ps2[:, half, :], o1rf[:, sl], w2cr_sb[:],
                                                 start=True, stop=False)
                                nc.tensor.matmul(ps2[:, half, :], o1if[:, sl], w2ci_sb[:],
                                                 start=False, stop=True)
                            tmp = bct.tile([90, 2, 2 * BS], f32, tag="tmp")
                            nc.vector.tensor_add(out=tmp[:], in0=ps2[:], in1=b2c_b[:90])
                            r1 = bct.tile([90, 2, 2 * BS], f32, tag="r1")
                            nc.vector.tensor_scalar(out=r1[:], in0=tmp[:], scalar1=-LAM,
                                                    scalar2=LAM, op0=OP.max, op1=OP.min)
                            nc.vector.tensor_sub(out=O2t[:, g, :, :],
                                                 in0=tmp[:], in1=r1[:])
                        # stage D (inverse H-DFT) fused per kw pair
                        psU = bcps4.tile([BS, 2, 512], f32, tag="psU")
                        for g in range(G):
                            for q in range(4):
                                ri, half = q // 2, q % 2
                                nc.tensor.matmul(
                                    psU[:, g, 0 : 2 * H],
                                    O2t[:, g, half, ri * BS : (ri + 1) * BS],
                                    ifhs_sb[:, q, :], start=(q == 0), stop=(q == 3))
                        ucp = bcw.tile([BS, 2, 2 * H], fp16, tag="ucp")
                        nc.vector.tensor_copy(out=ucp[:, :G, :], in_=psU[:, :G, 0 : 2 * H])
                        nc.sync.dma_start(
                            out=rap(ud, kw0 * BS * 2 * H,
                                    [[2 * H, BS], [BS * 2 * H, G], [1, 2 * H]]),
                            in_=ucp[:, :G, :])
                        # (ud write stays on SP: Act is busy with o1/r1/r2 here)

                # U back to kw-partitioned SBUF, then stage E (inverse W-DFT)
                for chv in range(2):
                    k0 = 46 * chv
                    kn = min(46, KW - k0)
                    eng = nc.scalar if chv else nc.sync
                    eng.dma_start(
                        out=U_sb[k0 : k0 + kn, :, :],
                        in_=rap(ud, k0 * BS * 2 * H,
                                [[BS * 2 * H, kn], [2 * H, BS], [1, 2 * H]]))
                with (
                    tc.tile_pool(name="sec", bufs=4) as sec,
                    tc.tile_pool(name="sep", bufs=2, space="PSUM") as sep,
                ):
                    # chunk A = local rows 0-11 per dest, chunk B = rows 12-22;
                    # A2A for chunk A overlaps stage-E compute of chunk B.
                    for part in range(2):
                        rts = [(0, 4), (4, 4), (8, 4)] if part == 0 else \
                              [(12, 4), (16, 4), (20, 3)]
                        for d in range(N):
                            for roff, nr in rts:
                                h0 = HP * d + roff
                                nr = min(nr, H - h0)
                                if nr <= 0:
                                    continue
                                for wk in range(3):
                                    psE = sep.tile([120, 4, BS], f32, tag="psE")
                                    for ri in range(2):
                                        nc.tensor.matmul(
                                            psE[:, :nr, :], ifw_sb[ri][:, wk, :],
                                            U_sb[:, :, ri * H + h0 : ri * H + h0 + nr]
                                                .rearrange("p c h -> p h c"),
                                            start=(ri == 0), stop=(ri == 1))
                                    ecp = sec.tile([120, 4, BS], fp16, tag="ecp")
                                    nc.vector.tensor_copy(out=ecp[:, :nr, :],
                                                          in_=psE[:, :nr, :])
                                    if part == 0:
                                        dst, off = a2iA, d * MA + (roff * W + wk * 120) * BS
                                    else:
                                        dst, off = a2iB, d * MB + ((roff - 12) * W + wk * 120) * BS
                                    nc.scalar.dma_start(
                                        out=rap(dst, off, [[BS, 120], [W * BS, nr], [1, BS]]),
                                        in_=ecp[:, :nr, :])
                        if part == 0:
                            nc.gpsimd.collective_compute(
                                "AllToAll", OP.bypass, replica_groups=RG,
                                ins=[a2iA[:]], outs=[a2oA[:]])

            nc.gpsimd.collective_compute(
                "AllToAll", OP.bypass, replica_groups=RG, ins=[a2iB[:]], outs=[a2oB[:]])

            # ---- phase 2: h1 = F2 + ln1x + x; LN2; modulated MLP; + h1 ----
            with (
                tc.tile_pool(name="p2w", bufs=1) as p2w,
                tc.tile_pool(name="p2", bufs=4) as p2,
                tc.tile_pool(name="p2h", bufs=12) as p2h,
                tc.tile_pool(name="p2s", bufs=4) as p2s,
                tc.tile_pool(name="p2m", bufs=3) as p2m,
                tc.tile_pool(name="ptp", bufs=2, space="PSUM") as ptp,
                tc.tile_pool(name="php", bufs=2, space="PSUM") as php,
                tc.tile_pool(name="pop", bufs=2, space="PSUM") as pop,
            ):
                PM = mybir.MatmulPerfMode.DoubleRow
                fc1w_sb = p2w.tile([128, 6, LAT], fp8)
                nc.sync.dma_start(
                    out=fc1w_sb[:], in_=rap(fc1w_p, 0, [[LAT, 128], [128 * LAT, 6], [1, LAT]]))
                fc2w_sb = p2w.tile([128, 24, C], fp8)
                nc.sync.dma_start(
                    out=fc2w_sb[:], in_=rap(fc2w_p, 0, [[C, 128], [128 * C, 24], [1, C]]))

                def p2_prep(it):
                    T0 = it * 512
                    ln2T = p2m.tile([128, 6, 4, 128], fp8, tag="ln2T")
                    h1s = []
                    for hf in range(4):
                        t0 = T0 + 128 * hf
                        nload = max(0, min(128, TOKR - t0))
                        xt = p2.tile([128, C], f32, tag="xt2")
                        nc.sync.dma_start(out=xt[:], in_=xs[t0 : t0 + 128, :])
                        f2t = p2.tile([128, N, BS], fp16, tag="f2t")
                        l1t = p2.tile([128, N, BS], fp16, tag="l1t")
                        if nload < 128:
                            nc.vector.memset(f2t[:], 0.0)
                            nc.vector.memset(l1t[:], 0.0)
                        if nload > 0:
                            ta = min(nload, max(0, TA - t0))
                            if ta > 0:
                                nc.sync.dma_start(
                                    out=f2t[:ta],
                                    in_=rap(a2oA, t0 * BS, [[BS, ta], [MA, N], [1, BS]]))
                            if ta < nload:
                                t0b = t0 + ta - TA
                                nc.sync.dma_start(
                                    out=f2t[ta:nload],
                                    in_=rap(a2oB, t0b * BS, [[BS, nload - ta], [MB, N], [1, BS]]))
                            nc.sync.dma_start(
                                out=l1t[:nload],
                                in_=rap(a1i, t0 * BS, [[BS, nload], [TOKR * BS, N], [1, BS]]))
                        h1 = p2h.tile([128, C], f32, tag="h1")
                        nc.vector.tensor_add(out=h1[:], in0=xt[:],
                                             in1=f2t[:].rearrange("p j c -> p (j c)"))
                        nc.vector.tensor_add(out=h1[:], in0=h1[:],
                                             in1=l1t[:].rearrange("p j c -> p (j c)"))
                        h1s.append(h1)
                        st = p2s.tile([128, 2, 6], f32, tag="st2")
                        for g in range(2):
                            nc.vector.bn_stats(out=st[:, g, :], in_=h1[:, 384 * g : 384 * (g + 1)])
                        mv = p2s.tile([128, 2], f32, tag="mv2")
                        nc.vector.bn_aggr(out=mv[:], in_=st[:])
                        rstd = p2s.tile([128, 1], f32, tag="rstd2")
                        nc.scalar.activation(out=rstd[:], in_=mv[:, 1:2], func=FT.Sqrt,
                                             bias=eps_sb[:], scale=1.0)
                        nc.vector.reciprocal(out=rstd[:], in_=rstd[:])
                        ln2 = p2.tile([128, C], bf16, tag="ln2")
                        nc.vector.tensor_scalar(out=ln2[:], in0=h1[:], scalar1=mv[:, 0:1],
                                                scalar2=rstd[:], op0=OP.subtract, op1=OP.mult)
                        nc.vector.tensor_mul(out=ln2[:], in0=ln2[:], in1=n2w_b[:])
                        nc.vector.tensor_add(out=ln2[:], in0=ln2[:], in1=n2b_b[:])
                        for jb in range(2):
                            pst = ptp.tile([128, 3, 128], bf16, tag="pst")
                            for jj in range(3):
                                j = 3 * jb + jj
                                nc.tensor.transpose(pst[:, jj, :],
                                                    ln2[:, 128 * j : 128 * (j + 1)],
                                                    identb[:])
                            nc.vector.tensor_copy(out=ln2T[:, 3 * jb : 3 * jb + 3, hf, :],
                                                  in_=pst[:])
                    return T0, ln2T, h1s

                def p2_mm(T0, ln2T, h1s):
                    hmidT = p2m.tile([128, 24, 512], fp8, tag="hmidT")
                    for l in range(24):
                        psH = php.tile([128, 512], f32, tag="psH")
                        for jp in range(3):
                            nc.tensor.matmul(
                                psH[:],
                                fc1w_sb[:, 2 * jp : 2 * jp + 2, 128 * l : 128 * (l + 1)],
                                ln2T[:, 2 * jp : 2 * jp + 2, :, :]
                                    .rearrange("p j h t -> p j (h t)"),
                                start=(jp == 0), stop=(jp == 2), perf_mode=PM)
                        nc.scalar.activation(out=hmidT[:, l, :], in_=psH[:], func=FT.Gelu,
                                             bias=B1[:, l : l + 1], scale=sM16[:, l : l + 1])
                    for hf in range(4):
                        t0 = T0 + 128 * hf
                        psO = pop.tile([128, 2, 512], f32, tag="psO")
                        for lp in range(12):
                            for h2 in range(2):
                                nc.tensor.matmul(
                                    psO[:, h2, 0:384],
                                    hmidT[:, 2 * lp : 2 * lp + 2, 128 * hf : 128 * (hf + 1)],
                                    fc2w_sb[:, 2 * lp : 2 * lp + 2, 384 * h2 : 384 * (h2 + 1)],
                                    start=(lp == 0), stop=(lp == 11), perf_mode=PM)
                        mo = p2.tile([128, C], f32, tag="mo")
                        nc.scalar.activation(
                            out=mo[:].rearrange("p (a b) -> p a b", a=2),
                            in_=psO[:, :, 0:384], func=FT.Copy, scale=1.0 / 16.0)
                        ot = p2.tile([128, C], f32, tag="ot")
                        nc.gpsimd.tensor_add(out=ot[:], in0=mo[:], in1=h1s[hf][:])
                        nc.gpsimd.tensor_add(out=ot[:], in0=ot[:], in1=fc2b_b[:])
                        nc.sync.dma_start(out=out_p[t0 : t0 + 128, :], in_=ot[:])

                # depth-2 software pipeline: prep(i+2) issues between mm(i)
                # and mm(i+1) so LN2/transposes never stall the PE stream
                sq = [p2_prep(0), p2_prep(1)]
                for it in range(NT2):
                    if it + 2 < NT2:
                        sq.append(p2_prep(it + 2))
                    p2_mm(*sq[it])

    nc.compile()
    return nc


_NC = None


def _get_nc():
    global _NC
    if _NC is None:
        _NC = _build()
    return _NC


def _dft_mats():
    w = np.arange(W); kw = np.arange(KW)
    ang = 2 * np.pi * np.outer(w, kw) / W
    fwr = (np.cos(ang) / np.sqrt(W)).astype(np.float32)
    fwi = (-np.sin(ang) / np.sqrt(W)).astype(np.float32)
    kh = np.arange(H); h = np.arange(H)
    angh = 2 * np.pi * np.outer(kh, h) / H        # [kh, h]
    fhr = np.cos(angh) / np.sqrt(H)
    fhi = -np.sin(angh) / np.sqrt(H)
    fhs = np.zeros((2 * H, 2 * H))
    fhs[:H, :H] = fhr.T; fhs[:H, H:] = fhi.T
    fhs[H:, :H] = -fhi.T; fhs[H:, H:] = fhr.T
    ci = np.cos(angh) / np.sqrt(H)                # [kh, h] for inverse
    si = np.sin(angh) / np.sqrt(H)
    ifhs = np.zeros((2 * H, 2 * H))
    ifhs[:H, :H] = ci; ifhs[:H, H:] = si
    ifhs[H:, :H] = -si; ifhs[H:, H:] = ci
    ckw = np.where(kw == 0, 1.0, 2.0)
    angi = 2 * np.pi * np.outer(kw, np.arange(W)) / W    # [kw, w]
    ifwr = (ckw[:, None] * np.cos(angi) / np.sqrt(W)).astype(np.float32)
    ifwi = (-ckw[:, None] * np.sin(angi) / np.sqrt(W)).astype(np.float32)
    return fwr, fwi, fhs.astype(np.float32), ifhs.astype(np.float32), ifwr, ifwi


def _prepare_in_maps(x, mod_embed, n1w, n1b, n2w, n2b, w1, b1, w2, b2,
                     fs_w0, fs_b0, fs_w1, fs_b1, fc1w, fc1b, fc2w, fc2b,
                     ms_w0, ms_b0, ms_w1, ms_b1):
    f = np.asarray
    x = f(x, dtype=np.float32)
    grid = x.reshape(H, W, C)
    fwr, fwi, fhs, ifhs, ifwr, ifwi = _dft_mats()
    bf = ml_dtypes.bfloat16

    in_maps = []
    for b in range(N):
        r0, r1 = HP * b, min(HP * (b + 1), H)
        xsb = np.zeros((TOKP, C), np.float32)
        xsb[: (r1 - r0) * W] = grid[r0:r1].reshape(-1, C)
        sl = slice(BS * b, BS * (b + 1))
        w2r = f(w2[0, b], np.float32); w2i = f(w2[1, b], np.float32)
        im = {
            "xs": xsb,
            "modT": np.repeat(f(mod_embed, np.float32).reshape(MODD, 1), 2, axis=1).copy(),
            "n1w": f(n1w, np.float32), "n1b": f(n1b, np.float32),
            "n2w": f(n2w, np.float32), "n2b": f(n2b, np.float32),
            "fwr": fwr.astype(np.float16), "fwi": fwi.astype(np.float16),
            "fhs": fhs.astype(np.float16), "ifhs": ifhs.astype(np.float16),
            "ifwr": ifwr.astype(np.float16), "ifwi": ifwi.astype(np.float16),
            "w1r": f(w1[0, b], np.float16).copy(),
            "w1i": f(w1[1, b], np.float16).copy(),
            "w1in": (-f(w1[1, b], np.float16)).copy(),
            "w2cr": np.concatenate([w2r, w2i], axis=1).astype(np.float16),
            "w2ci": np.concatenate([-w2i, w2r], axis=1).astype(np.float16),
            "b1r": f(b1[0, b], np.float32).reshape(BS, 1).copy(),
            "b1i": f(b1[1, b], np.float32).reshape(BS, 1).copy(),
            "b2c": np.concatenate([f(b2[0, b], np.float32), f(b2[1, b], np.float32)]),
            "fs_w0": f(fs_w0, np.float32),
            "fs_b0T": f(fs_b0, np.float32).reshape(12, 128).T.copy(),
            "fs_w1s": np.concatenate(
                [f(fs_w1, np.float32)[:, sl], f(fs_w1, np.float32)[:, C + BS * b : C + BS * (b + 1)]],
                axis=1),
            "fs_b1s": np.concatenate(
                [f(fs_b1, np.float32)[sl], f(fs_b1, np.float32)[C + BS * b : C + BS * (b + 1)]]
            ).reshape(1, -1),
            "ms_w0": f(ms_w0, np.float32),
            "ms_b0T": f(ms_b0, np.float32).reshape(48, 128).T.copy(),
            "ms_w1s": f(ms_w1, np.float32)[:, C * b : C * (b + 1)].astype(bf),
            "ms_b1s": f(ms_b1, np.float32)[C * b : C * (b + 1)].reshape(1, -1),
            "fc1w": (16.0 * f(fc1w, np.float32)).astype(ml_dtypes.float8_e4m3),
            "fc1bT": f(fc1b, np.float32).reshape(24, 128).T.copy(),
            "fc2w": (16.0 * f(fc2w, np.float32)).astype(ml_dtypes.float8_e4m3),
            "fc2b": f(fc2b, np.float32),
        }
        in_maps.append(im)
    return in_maps


def kernel(x, mod_embed, n1w, n1b, n2w, n2b, w1, b1, w2, b2,
           fs_w0, fs_b0, fs_w1, fs_b1, fc1w, fc1b, fc2w, fc2b,
           ms_w0, ms_b0, ms_w1, ms_b1):
    nc = _get_nc()
    in_maps = _prepare_in_maps(
        x, mod_embed, n1w, n1b, n2w, n2b, w1, b1, w2, b2,
        fs_w0, fs_b0, fs_w1, fs_b1, fc1w, fc1b, fc2w, fc2b,
        ms_w0, ms_b0, ms_w1, ms_b1)

    res = run_bass_kernel_spmd(nc, in_maps, core_ids=list(range(N)))
    globals()["last_results"] = res
    out = np.zeros((H, W, C), np.float32)
    for b in range(N):
        r0, r1 = HP * b, min(HP * (b + 1), H)
        out[r0:r1] = res.results[b]["out"][: (r1 - r0) * W].reshape(r1 - r0, W, C)
    return out.reshape(1, H, W, C)



# revision 26
# speedup vs baseline: 1.1238x; 1.0085x over previous
"""AFNO block kernel for 8 Trainium2 NeuronCores.

Sharding: token-shard (H rows, 23 per core padded) for LN/MLP phases;
AllToAll to channel-shard (core i = spectral block i, 96 channels) for the
2D-FFT filter, computed as matmuls against precomputed DFT matrices;
AllToAll back; small AllGather for the column-sharded 6144x6144 scale-shift
MLP weight.
"""

import os
import numpy as np
import ml_dtypes

import concourse.bass as bass
import concourse.bacc as bacc
import concourse.mybir as mybir
import concourse.tile as tile
from concourse.bass_utils import run_bass_kernel_spmd
from concourse.masks import make_identity

f32 = mybir.dt.float32
f32r = mybir.dt.float32r
bf16 = mybir.dt.bfloat16
fp16 = mybir.dt.float16
fp8 = mybir.dt.float8e4
FT = mybir.ActivationFunctionType
OP = mybir.AluOpType

H, W, C = 180, 360, 768
NB, BS, KW = 8, 96, 91
HP = 23                 # rows per shard (8*23 = 184 >= 180)
TOKR = HP * W           # 8280 real token slots per shard
NT2 = 17                # phase-2 tiles of 512
TOKP = NT2 * 512        # 8704 padded tokens per shard
MODD, LAT, LAT2 = 64, 3072, 6144
LAM = 0.01
EPS = 1e-5
N = 8


def rap(t, offset, dims):
    a = t[:] if not isinstance(t, bass.AP) else t
    return bass.AP(tensor=a.tensor, offset=a.offset + offset, ap=[list(d) for d in dims])


def _build():
    nc = bacc.Bacc("TRN2", target_bir_lowering=False, debug=False, num_devices=N)

    def P(name, shp, dt=f32):
        return nc.declare_dram_parameter(name, list(shp), dt, isOutput=False)

    xs = P("xs", [TOKP, C])
    modT = P("modT", [MODD, 2])
    n1w = P("n1w", [C]); n1b = P("n1b", [C])
    n2w = P("n2w", [C]); n2b = P("n2b", [C])
    fwr_p = P("fwr", [W, KW], fp16); fwi_p = P("fwi", [W, KW], fp16)
    fhs_p = P("fhs", [2 * H, 2 * H], fp16)
    ifhs_p = P("ifhs", [2 * H, 2 * H], fp16)
    ifwr_p = P("ifwr", [KW, W], fp16); ifwi_p = P("ifwi", [KW, W], fp16)
    w1r_p = P("w1r", [BS, BS], fp16); w1i_p = P("w1i", [BS, BS], fp16)
    w1in_p = P("w1in", [BS, BS], fp16)
    w2cr_p = P("w2cr", [BS, 2 * BS], fp16)   # [W2r | W2i]
    w2ci_p = P("w2ci", [BS, 2 * BS], fp16)   # [-W2i | W2r]
    b1r_p = P("b1r", [BS, 1]); b1i_p = P("b1i", [BS, 1])
    b2c_p = P("b2c", [2 * BS])               # concat(b2r, b2i)
    fs_w0_p = P("fs_w0", [MODD, 2 * C])
    fs_b0T_p = P("fs_b0T", [128, 12])
    fs_w1s_p = P("fs_w1s", [2 * C, 2 * BS])
    fs_b1s_p = P("fs_b1s", [1, 2 * BS])
    ms_w0_p = P("ms_w0", [MODD, LAT2])
    ms_b0T_p = P("ms_b0T", [128, 48])
    ms_w1s_p = P("ms_w1s", [LAT2, C], bf16)
    ms_b1s_p = P("ms_b1s", [1, C])
    fc1w_p = P("fc1w", [C, LAT], fp8)
    fc1bT_p = P("fc1bT", [128, 24])
    fc2w_p = P("fc2w", [LAT, C], fp8)
    fc2b_p = P("fc2b", [C])
    out_p = nc.declare_dram_parameter("out", [TOKP, C], f32, isOutput=True)
    DBG = False

    # internal DRAM
    a1i = nc.dram_tensor("a1i", [N, TOKR * BS], fp16)
    a1o = nc.dram_tensor("a1o", [N, TOKR * BS], fp16)
    MA = 12 * W * BS        # chunk A: local rows 0-11 per dest
    MB = 11 * W * BS        # chunk B: local rows 12-22
    TA = 12 * W             # tokens per dest covered by chunk A
    a2iA = nc.dram_tensor("a2iA", [N, MA], fp16)
    a2oA = nc.dram_tensor("a2oA", [N, MA], fp16)
    a2iB = nc.dram_tensor("a2iB", [N, MB], fp16)
    a2oB = nc.dram_tensor("a2oB", [N, MB], fp16)
    t1d = nc.dram_tensor("t1d", [KW, 2, H, BS], fp16)   # [kw][ri][h][c]
    ud = nc.dram_tensor("ud", [KW, BS, 2 * H], fp16)
    sfd = nc.dram_tensor("sfd", [1, 2 * BS], f32)
    agi = nc.dram_tensor("agi", [1, C], f32)
    ago = nc.dram_tensor("ago", [N, C], f32)

    RG = [list(range(N))]

    with tile.TileContext(nc) as tc:
        with (
            tc.tile_pool(name="const", bufs=1) as cpool,
            tc.tile_pool(name="ssb", bufs=1) as ssb,
        ):
            # ---- broadcast constants ----
            def bcast(p, n, name):
                t = cpool.tile([128, n], f32, tag=name)
                nc.sync.dma_start(out=t[:], in_=rap(p, 0, [[0, 128], [1, n]]))
                return t

            n1w_b = bcast(n1w, C, "n1w"); n1b_b = bcast(n1b, C, "n1b")
            n2w_b = bcast(n2w, C, "n2w"); n2b_b = bcast(n2b, C, "n2b")
            fc2b_b = bcast(fc2b_p, C, "fc2b")
            b2c_b = cpool.tile([128, 2, 2 * BS], f32, tag="b2c")
            for bh in range(2):
                nc.sync.dma_start(out=b2c_b[:, bh, :],
                                  in_=rap(b2c_p, 0, [[0, 128], [1, 2 * BS]]))
            eps_sb = cpool.tile([128, 1], f32, tag="eps")
            nc.vector.memset(eps_sb[:], EPS)
            nlam_sb = cpool.tile([128, 1], f32, tag="nlam")
            nc.vector.memset(nlam_sb[:], -LAM)
            ident = cpool.tile([128, 128], f32, tag="ident")
            make_identity(nc, ident[:])
            identb = cpool.tile([128, 128], bf16, tag="identb")
            nc.vector.tensor_copy(out=identb[:], in_=ident[:])


            # ---- phase 0: LN1 + scatter into A2A-1 send buffer ----
            with (
                tc.tile_pool(name="p0", bufs=6) as p0,
                tc.tile_pool(name="p0s", bufs=8) as p0s,
            ):
                for it in range(65):
                    t0 = it * 128
                    nrow = min(128, TOKR - t0)
                    xt = p0.tile([128, C], f32, tag="xt")
                    nc.sync.dma_start(out=xt[:], in_=xs[t0 : t0 + 128, :])
                    st = p0s.tile([128, 2, 6], f32, tag="st")
                    for g in range(2):
                        nc.vector.bn_stats(out=st[:, g, :], in_=xt[:, 384 * g : 384 * (g + 1)])
                    mv = p0s.tile([128, 2], f32, tag="mv")
                    nc.vector.bn_aggr(out=mv[:], in_=st[:])
                    rstd = p0s.tile([128, 1], f32, tag="rstd")
                    nc.scalar.activation(out=rstd[:], in_=mv[:, 1:2], func=FT.Sqrt,
                                         bias=eps_sb[:], scale=1.0)
                    nc.vector.reciprocal(out=rstd[:], in_=rstd[:])
                    ln = p0.tile([128, C], f32, tag="ln")
                    nc.vector.tensor_scalar(out=ln[:], in0=xt[:], scalar1=mv[:, 0:1],
                                            scalar2=rstd[:], op0=OP.subtract, op1=OP.mult)
                    aeng = nc.gpsimd if it % 3 == 2 else nc.vector
                    aeng.tensor_mul(out=ln[:], in0=ln[:], in1=n1w_b[:])
                    lnh = p0.tile([128, C], fp16, tag="lnh")
                    aeng.tensor_add(out=lnh[:], in0=ln[:], in1=n1b_b[:])
                    nc.scalar.dma_start(
                        out=rap(a1i, t0 * BS, [[BS, nrow], [TOKR * BS, N], [1, BS]]),
                        in_=lnh[:nrow].rearrange("p (j c) -> p j c", j=N),
                    )

            nc.gpsimd.collective_compute(
                "AllToAll", OP.bypass, replica_groups=RG, ins=[a1i[:]], outs=[a1o[:]])

            # ---- scale-shift MLPs (overlap with A2A1 window) ----
            ss_ctx = tc.tile_pool(name="ssw", bufs=1)
            ssw = ss_ctx.__enter__()
            ssp_ctx = tc.tile_pool(name="ssp", bufs=1, space="PSUM")
            ssp = ssp_ctx.__enter__()
            modT_sb = ssw.tile([MODD, 2], f32r)
            nc.sync.dma_start(out=modT_sb[:], in_=modT[:].bitcast(f32r))
            fs_w0_sb = ssw.tile([MODD, 2 * C], f32r)
            nc.sync.dma_start(out=fs_w0_sb[:], in_=fs_w0_p[:].bitcast(f32r))
            fs_b0T_sb = ssw.tile([128, 12], f32)
            nc.sync.dma_start(out=fs_b0T_sb[:], in_=fs_b0T_p[:])
            e0T = ssw.tile([128, 12], f32r)
            for j in range(12):
                pt = ssp.tile([128, 2], f32, tag="ss1")
                nc.tensor.matmul(pt[:], fs_w0_sb[:, 128 * j : 128 * (j + 1)],
                                 modT_sb[:], start=True, stop=True)
                nc.scalar.activation(out=e0T[:, j : j + 1], in_=pt[:, 0:1], func=FT.Gelu,
                                     bias=fs_b0T_sb[:, j : j + 1], scale=1.0)
            fs_w1s_sb = ssw.tile([128, 12, 2 * BS], f32r)
            nc.sync.dma_start(
                out=fs_w1s_sb[:],
                in_=rap(fs_w1s_p, 0, [[2 * BS, 128], [128 * 2 * BS, 12], [1, 2 * BS]]).bitcast(f32r),
            )
            fs_b1s_sb = ssw.tile([1, 2 * BS], f32)
            nc.sync.dma_start(out=fs_b1s_sb[:], in_=fs_b1s_p[:])
            p2 = ssp.tile([1, 2 * BS], f32, tag="ss2")
            for j in range(12):
                nc.tensor.matmul(p2[:], e0T[:, j : j + 1], fs_w1s_sb[:, j, :],
                                 start=(j == 0), stop=(j == 11))
            sfo = ssw.tile([1, 2 * BS], f32)
            nc.vector.tensor_add(out=sfo[:], in0=p2[:], in1=fs_b1s_sb[:])
            nc.sync.dma_start(out=sfd[:], in_=sfo[:])
            sfT = ssw.tile([BS, 2], f32)
            nc.sync.dma_start(out=sfT[:], in_=rap(sfd, 0, [[1, BS], [BS, 2]]))
            sfv = ssb.tile([BS, 1], f32)
            nc.vector.tensor_scalar_add(out=sfv[:], in0=sfT[:, 0:1], scalar1=1.0)
            b1r_sb = ssw.tile([BS, 1], f32)
            nc.sync.dma_start(out=b1r_sb[:], in_=b1r_p[:])
            b1i_sb = ssw.tile([BS, 1], f32)
            nc.sync.dma_start(out=b1i_sb[:], in_=b1i_p[:])
            Br = ssb.tile([BS, 1], f32)
            nc.vector.tensor_mul(out=Br[:], in0=b1r_sb[:], in1=sfv[:])
            nc.vector.tensor_add(out=Br[:], in0=Br[:], in1=sfT[:, 1:2])
            Bi = ssb.tile([BS, 1], f32)
            nc.vector.tensor_mul(out=Bi[:], in0=b1i_sb[:], in1=sfv[:])
            nc.vector.tensor_add(out=Bi[:], in0=Bi[:], in1=sfT[:, 1:2])

            # ms MLP: e1T then column-sharded 6144->768, AllGather
            ms_w0_sb = ssw.tile([MODD, LAT2], f32r)
            nc.gpsimd.dma_start(out=ms_w0_sb[:], in_=ms_w0_p[:].bitcast(f32r))
            ms_b0T_sb = ssw.tile([128, 48], f32)
            nc.sync.dma_start(out=ms_b0T_sb[:], in_=ms_b0T_p[:])
            e1T = ssw.tile([128, 48], bf16)
            for j in range(48):
                pt = ssp.tile([128, 2], f32, tag="ss1")
                nc.tensor.matmul(pt[:], ms_w0_sb[:, 128 * j : 128 * (j + 1)],
                                 modT_sb[:], start=True, stop=True)
                nc.scalar.activation(out=e1T[:, j : j + 1], in_=pt[:, 0:1], func=FT.Gelu,
                                     bias=ms_b0T_sb[:, j : j + 1], scale=1.0)
            p3 = ssp.tile([1, 2, 512], f32, tag="ss3")
            with tc.tile_pool(name="msw", bufs=3) as mswp:
                for j in range(48):
                    wt = mswp.tile([128, C], bf16)
                    nc.gpsimd.dma_start(
                        out=wt[:], in_=ms_w1s_p[128 * j : 128 * (j + 1), :])
                    for h2 in range(2):
                        nc.tensor.matmul(
                            p3[:, h2, 0:384], e1T[:, j : j + 1],
                            wt[:, 384 * h2 : 384 * (h2 + 1)],
                            start=(j == 0), stop=(j == 47))
            ms_b1s_sb = ssw.tile([1, C], f32)
            nc.sync.dma_start(out=ms_b1s_sb[:], in_=ms_b1s_p[:])
            mso = ssw.tile([1, C], f32)
            nc.vector.tensor_add(out=mso[:].rearrange("p (a b) -> p a b", a=2),
                                 in0=p3[:, :, 0:384],
                                 in1=ms_b1s_sb[:].rearrange("p (a b) -> p a b", a=2))
            nc.sync.dma_start(out=agi[:], in_=mso[:])
            nc.gpsimd.collective_compute(
                "AllGather", OP.bypass, replica_groups=RG, ins=[agi[:]], outs=[ago[:]])
            sM = ssb.tile([128, 24], f32)
            nc.sync.dma_start(out=sM[:], in_=rap(ago, 0, [[1, 128], [128, 24]]))
            nc.vector.tensor_scalar_add(out=sM[:], in0=sM[:], scalar1=1.0)
            tM = ssb.tile([128, 24], f32)
            nc.sync.dma_start(out=tM[:], in_=rap(ago, LAT, [[1, 128], [128, 24]]))
            fc1bT_sb = ssw.tile([128, 24], f32)
            nc.sync.dma_start(out=fc1bT_sb[:], in_=fc1bT_p[:])
            B1 = ssb.tile([128, 24], f32)
            nc.vector.tensor_mul(out=B1[:], in0=fc1bT_sb[:], in1=sM[:])
            nc.vector.tensor_add(out=B1[:], in0=B1[:], in1=tM[:])
            sM16 = ssb.tile([128, 24], f32)
            nc.vector.tensor_scalar_mul(out=sM16[:], in0=sM[:], scalar1=1.0 / 16.0)

            ssp_ctx.__exit__(None, None, None)
            ss_ctx.__exit__(None, None, None)

            # ---- phase 1 stage A: W-DFT  (X[h,w,c] -> t1d[kw,ri,h,c]) ----
            with (
                tc.tile_pool(name="sa", bufs=1) as sa,
                tc.tile_pool(name="sac", bufs=3) as sac,
                tc.tile_pool(name="sap", bufs=2, space="PSUM") as sap,
            ):
                fw_sb = []
                for ri, p in enumerate([fwr_p, fwi_p]):
                    t = sa.tile([120, 3, KW], fp16, tag=f"fw{ri}")
                    nc.sync.dma_start(
                        out=t[:], in_=rap(p, 0, [[KW, 120], [120 * KW, 3], [1, KW]]))
                    fw_sb.append(t)
                X_sb = sa.tile([120, 3, H, BS], fp16, tag="xsb")
                for ch in range(4):
                    for k in range(3):
                        eng = nc.scalar if (ch + k) % 2 else nc.sync
                        eng.dma_start(
                            out=X_sb[:, k, 45 * ch : 45 * (ch + 1), :],
                            in_=rap(a1o, (45 * ch * W + 120 * k) * BS,
                                    [[BS, 120], [W * BS, 45], [1, BS]]))
                for hs in range(36):
                    hh0 = 5 * hs
                    cp = sac.tile([KW, 2, 5, BS], fp16, tag="cpa")
                    for ri in range(2):
                        ps = sap.tile([KW, 5, BS], f32, tag="pa")
                        for k in range(3):
                            nc.tensor.matmul(ps[:], fw_sb[ri][:, k, :],
                                             X_sb[:, k, hh0 : hh0 + 5, :],
                                             start=(k == 0), stop=(k == 2))
                        nc.vector.tensor_copy(out=cp[:, ri, :, :], in_=ps[:])
                    nc.scalar.dma_start(
                        out=rap(t1d, hh0 * BS,
                                [[2 * H * BS, KW], [H * BS, 2], [BS, 5], [1, BS]]),
                        in_=cp[:])

            # ---- stages B+C+D fused in SBUF, then E ----
            with tc.tile_pool(name="fb", bufs=1) as fb:
                fhs_sb = fb.tile([90, 4, 2 * H], fp16)
                nc.sync.dma_start(
                    out=fhs_sb[:],
                    in_=rap(fhs_p, 0, [[2 * H, 90], [90 * 2 * H, 4], [1, 2 * H]]))
                ifhs_sb = fb.tile([90, 4, 2 * H], fp16)
                nc.sync.dma_start(
                    out=ifhs_sb[:],
                    in_=rap(ifhs_p, 0, [[2 * H, 90], [90 * 2 * H, 4], [1, 2 * H]]))
                w1r_sb = fb.tile([BS, BS], fp16)
                nc.sync.dma_start(out=w1r_sb[:], in_=w1r_p[:])
                w1i_sb = fb.tile([BS, BS], fp16)
                nc.sync.dma_start(out=w1i_sb[:], in_=w1i_p[:])
                w1in_sb = fb.tile([BS, BS], fp16)
                nc.sync.dma_start(out=w1in_sb[:], in_=w1in_p[:])
                w2cr_sb = fb.tile([BS, 2 * BS], fp16)
                nc.sync.dma_start(out=w2cr_sb[:], in_=w2cr_p[:])
                w2ci_sb = fb.tile([BS, 2 * BS], fp16)
                nc.sync.dma_start(out=w2ci_sb[:], in_=w2ci_p[:])
                ifw_sb = []
                for ri, p in enumerate([ifwr_p, ifwi_p]):
                    t = fb.tile([KW, 3, 120], fp16, tag=f"ifw{ri}")
                    nc.sync.dma_start(
                        out=t[:], in_=rap(p, 0, [[W, KW], [120, 3], [1, 120]]))
                    ifw_sb.append(t)
                # T1 resident: [h(90), half, ri, kw, c]
                T1_sb = fb.tile([90, 2, 2, KW, BS], fp16, tag="t1sb")
                for half in range(2):
                    for ri in range(2):
                        eng = nc.scalar if ri else nc.sync
                        eng.dma_start(
                            out=T1_sb[:, half, ri, :, :],
                            in_=rap(t1d, (ri * H + half * 90) * BS,
                                    [[BS, 90], [2 * H * BS, KW], [1, BS]]))
                U_sb = fb.tile([KW, BS, 2 * H], fp16, tag="usb")

                with (
                    tc.tile_pool(name="bcw", bufs=4) as bcw,
                    tc.tile_pool(name="bct", bufs=4) as bct,
                    tc.tile_pool(name="bco", bufs=3) as bco,
                    tc.tile_pool(name="bcp1", bufs=1, space="PSUM") as bcps,
                    tc.tile_pool(name="bcp2", bufs=1, space="PSUM") as bcps2,
                    tc.tile_pool(name="bcp3", bufs=2, space="PSUM") as bcps3,
                    tc.tile_pool(name="bcp4", bufs=1, space="PSUM") as bcps4,
                ):
                    def emit_D(O2t, kw0, G):
                        psU = bcps4.tile([BS, 2, 512], f32, tag="psU")
                        for g in range(G):
                            for q in range(4):
                                ri, half = q // 2, q % 2
                                nc.tensor.matmul(
                                    psU[:, g, 0 : 2 * H],
                                    O2t[:, g, half, ri * BS : (ri + 1) * BS],
                                    ifhs_sb[:, q, :], start=(q == 0), stop=(q == 3))
                        ucp = bcw.tile([BS, 2, 2 * H], fp16, tag="ucp")
                        nc.vector.tensor_copy(out=ucp[:, :G, :], in_=psU[:, :G, 0 : 2 * H])
                        nc.sync.dma_start(
                            out=rap(ud, kw0 * BS * 2 * H,
                                    [[2 * H, BS], [BS * 2 * H, G], [1, 2 * H]]),
                            in_=ucp[:, :G, :])

                    pend = None
                    for pr in range(46):
                        kw0 = 2 * pr
                        G = 2 if kw0 + 1 < KW else 1
                        psF = bcps.tile([BS, 2, 512], f32, tag="psF")
                        for g in range(G):
                            kw = kw0 + g
                            for q in range(4):
                                ri, half = q // 2, q % 2
                                nc.tensor.matmul(
                                    psF[:, g, 0 : 2 * H], T1_sb[:, half, ri, kw, :],
                                    fhs_sb[:, q, :], start=(q == 0), stop=(q == 3))
                        fsb = bcw.tile([BS, 2, 2 * H], fp16, tag="fsb")
                        nc.vector.tensor_copy(out=fsb[:, :G, :], in_=psF[:, :G, 0 : 2 * H])
                        ps1r = bcps2.tile([BS, 2, H], f32, tag="ps1r")
                        ps1i = bcps2.tile([BS, 2, H], f32, tag="ps1i")
                        nc.tensor.matmul(ps1r[:, :G, :], w1r_sb[:], fsb[:, :G, 0:H],
                                         start=True, stop=False)
                        nc.tensor.matmul(ps1r[:, :G, :], w1in_sb[:], fsb[:, :G, H : 2 * H],
                                         start=False, stop=True)
                        nc.tensor.matmul(ps1i[:, :G, :], w1i_sb[:], fsb[:, :G, 0:H],
                                         start=True, stop=False)
                        nc.tensor.matmul(ps1i[:, :G, :], w1r_sb[:], fsb[:, :G, H : 2 * H],
                                         start=False, stop=True)
                        o1r = bcw.tile([BS, 2, H], fp16, tag="o1r")
                        o1i = bcw.tile([BS, 2, H], fp16, tag="o1i")
                        nc.scalar.activation(out=o1r[:, :G, :], in_=ps1r[:, :G, :],
                                             func=FT.Relu, bias=Br[:], scale=sfv[:])
                        nc.scalar.activation(out=o1i[:, :G, :], in_=ps1i[:, :G, :],
                                             func=FT.Relu, bias=Bi[:], scale=sfv[:])
                        o1rf = o1r[:].rearrange("p g k -> p (g k)")
                        o1if = o1i[:].rearrange("p g k -> p (g k)")
                        O2t = bco.tile([90, 2, 2, 2 * BS], fp16, tag="o2sb")
                        for g in range(G):
                            ps2 = bcps3.tile([90, 2, 2 * BS], f32, tag="ps2")
                            for half in range(2):
                                sl = slice(90 * (2 * g + half), 90 * (2 * g + half + 1))
                                nc.tensor.matmul(ps2[:, half, :], o1rf[:, sl], w2cr_sb[:],
                                                 start=True, stop=False)
                                nc.tensor.matmul(ps2[:, half, :], o1if[:, sl], w2ci_sb[:],
                                                 start=False, stop=True)
                            tmp = bct.tile([90, 2, 2 * BS], f32, tag="tmp")
                            nc.vector.tensor_add(out=tmp[:], in0=ps2[:], in1=b2c_b[:90])
                            r1 = bct.tile([90, 2, 2 * BS], f32, tag="r1")
                            nc.vector.tensor_scalar(out=r1[:], in0=tmp[:], scalar1=-LAM,
                                                    scalar2=LAM, op0=OP.max, op1=OP.min)
                            nc.vector.tensor_sub(out=O2t[:, g, :, :],
                                                 in0=tmp[:], in1=r1[:])
                        # stage D (inverse H-DFT) pipelined one iteration
                        # behind B/C so the softshrink chain never stalls
                        # the in-order tensor queue
                        if pend is not None:
                            emit_D(*pend)
                        pend = (O2t, kw0, G)
                    emit_D(*pend)

                # U back to kw-partitioned SBUF, then stage E (inverse W-DFT)
                for chv in range(2):
                    k0 = 46 * chv
                    kn = min(46, KW - k0)
                    eng = nc.scalar if chv else nc.sync
                    eng.dma_start(
                        out=U_sb[k0 : k0 + kn, :, :],
                        in_=rap(ud, k0 * BS * 2 * H,
                                [[BS * 2 * H, kn], [2 * H, BS], [1, 2 * H]]))
                with (
                    tc.tile_pool(name="sec", bufs=4) as sec,
                    tc.tile_pool(name="sep", bufs=2, space="PSUM") as sep,
                ):
                    # chunk A = local rows 0-11 per dest, chunk B = rows 12-22;
                    # A2A for chunk A overlaps stage-E compute of chunk B.
                    for part in range(2):
                        rts = [(0, 4), (4, 4), (8, 4)] if part == 0 else \
                              [(12, 4), (16, 4), (20, 3)]
                        for d in range(N):
                            for roff, nr in rts:
                                h0 = HP * d + roff
                                nr = min(nr, H - h0)
                                if nr <= 0:
                                    continue
                                for wk in range(3):
                                    psE = sep.tile([120, 4, BS], f32, tag="psE")
                                    for ri in range(2):
                                        nc.tensor.matmul(
                                            psE[:, :nr, :], ifw_sb[ri][:, wk, :],
                                            U_sb[:, :, ri * H + h0 : ri * H + h0 + nr]
                                                .rearrange("p c h -> p h c"),
                                            start=(ri == 0), stop=(ri == 1))
                                    ecp = sec.tile([120, 4, BS], fp16, tag="ecp")
                                    nc.vector.tensor_copy(out=ecp[:, :nr, :],
                                                          in_=psE[:, :nr, :])
                                    if part == 0:
                                        dst, off = a2iA, d * MA + (roff * W + wk * 120) * BS
                                    else:
                                        dst, off = a2iB, d * MB + ((roff - 12) * W + wk * 120) * BS
                                    nc.scalar.dma_start(
                                        out=rap(dst, off, [[BS, 120], [W * BS, nr], [1, BS]]),
                                        in_=ecp[:, :nr, :])
                        if part == 0:
                            nc.gpsimd.collective_compute(
                                "AllToAll", OP.bypass, replica_groups=RG,
                                ins=[a2iA[:]], outs=[a2oA[:]])

            nc.gpsimd.collective_compute(
                "AllToAll", OP.bypass, replica_groups=RG, ins=[a2iB[:]], outs=[a2oB[:]])

            # ---- phase 2: h1 = F2 + ln1x + x; LN2; modulated MLP; + h1 ----
            with (
                tc.tile_pool(name="p2w", bufs=1) as p2w,
                tc.tile_pool(name="p2", bufs=4) as p2,
                tc.tile_pool(name="p2h", bufs=12) as p2h,
                tc.tile_pool(name="p2s", bufs=4) as p2s,
                tc.tile_pool(name="p2m", bufs=3) as p2m,
                tc.tile_pool(name="ptp", bufs=2, space="PSUM") as ptp,
                tc.tile_pool(name="php", bufs=2, space="PSUM") as php,
                tc.tile_pool(name="pop", bufs=2, space="PSUM") as pop,
            ):
                PM = mybir.MatmulPerfMode.DoubleRow
                fc1w_sb = p2w.tile([128, 6, LAT], fp8)
                nc.sync.dma_start(
                    out=fc1w_sb[:], in_=rap(fc1w_p, 0, [[LAT, 128], [128 * LAT, 6], [1, LAT]]))
                fc2w_sb = p2w.tile([128, 24, C], fp8)
                nc.sync.dma_start(
                    out=fc2w_sb[:], in_=rap(fc2w_p, 0, [[C, 128], [128 * C, 24], [1, C]]))

                def p2_prep(it):
                    T0 = it * 512
                    ln2T = p2m.tile([128, 6, 4, 128], fp8, tag="ln2T")
                    h1s = []
                    for hf in range(4):
                        t0 = T0 + 128 * hf
                        nload = max(0, min(128, TOKR - t0))
                        xt = p2.tile([128, C], f32, tag="xt2")
                        nc.sync.dma_start(out=xt[:], in_=xs[t0 : t0 + 128, :])
                        f2t = p2.tile([128, N, BS], fp16, tag="f2t")
                        l1t = p2.tile([128, N, BS], fp16, tag="l1t")
                        if nload < 128:
                            nc.vector.memset(f2t[:], 0.0)
                            nc.vector.memset(l1t[:], 0.0)
                        if nload > 0:
                            ta = min(nload, max(0, TA - t0))
                            if ta > 0:
                                nc.sync.dma_start(
                                    out=f2t[:ta],
                                    in_=rap(a2oA, t0 * BS, [[BS, ta], [MA, N], [1, BS]]))
                            if ta < nload:
                                t0b = t0 + ta - TA
                                nc.sync.dma_start(
                                    out=f2t[ta:nload],
                                    in_=rap(a2oB, t0b * BS, [[BS, nload - ta], [MB, N], [1, BS]]))
                            nc.sync.dma_start(
                                out=l1t[:nload],
                                in_=rap(a1i, t0 * BS, [[BS, nload], [TOKR * BS, N], [1, BS]]))
                        h1 = p2h.tile([128, C], f32, tag="h1")
                        nc.vector.tensor_add(out=h1[:], in0=xt[:],
                                             in1=f2t[:].rearrange("p j c -> p (j c)"))
                        nc.vector.tensor_add(out=h1[:], in0=h1[:],
                                             in1=l1t[:].rearrange("p j c -> p (j c)"))
                        h1s.append(h1)
                        st = p2s.tile([128, 2, 6], f32, tag="st2")
                        for g in range(2):
                            nc.vector.bn_stats(out=st[:, g, :], in_=h1[:, 384 * g : 384 * (g + 1)])
                        mv = p2s.tile([128, 2], f32, tag="mv2")
                        nc.vector.bn_aggr(out=mv[:], in_=st[:])
                        rstd = p2s.tile([128, 1], f32, tag="rstd2")
                        nc.scalar.activation(out=rstd[:], in_=mv[:, 1:2], func=FT.Sqrt,
                                             bias=eps_sb[:], scale=1.0)
                        nc.vector.reciprocal(out=rstd[:], in_=rstd[:])
                        ln2 = p2.tile([128, C], bf16, tag="ln2")
                        nc.vector.tensor_scalar(out=ln2[:], in0=h1[:], scalar1=mv[:, 0:1],
                                                scalar2=rstd[:], op0=OP.subtract, op1=OP.mult)
                        nc.vector.tensor_mul(out=ln2[:], in0=ln2[:], in1=n2w_b[:])
                        nc.vector.tensor_add(out=ln2[:], in0=ln2[:], in1=n2b_b[:])
                        for jb in range(2):
                            pst = ptp.tile([128, 3, 128], bf16, tag="pst")
                            for jj in range(3):
                                j = 3 * jb + jj
                                nc.tensor.transpose(pst[:, jj, :],
                                                    ln2[:, 128 * j : 128 * (j + 1)],
                                                    identb[:])
                            nc.vector.tensor_copy(out=ln2T[:, 3 * jb : 3 * jb + 3, hf, :],
                                                  in_=pst[:])
                    return T0, ln2T, h1s

                def p2_mm(T0, ln2T, h1s):
                    hmidT = p2m.tile([128, 24, 512], fp8, tag="hmidT")
                    for l in range(24):
                        psH = php.tile([128, 512], f32, tag="psH")
                        for jp in range(3):
                            nc.tensor.matmul(
                                psH[:],
                                fc1w_sb[:, 2 * jp : 2 * jp + 2, 128 * l : 128 * (l + 1)],
                                ln2T[:, 2 * jp : 2 * jp + 2, :, :]
                                    .rearrange("p j h t -> p j (h t)"),
                                start=(jp == 0), stop=(jp == 2), perf_mode=PM)
                        nc.scalar.activation(out=hmidT[:, l, :], in_=psH[:], func=FT.Gelu,
                                             bias=B1[:, l : l + 1], scale=sM16[:, l : l + 1])
                    for hf in range(4):
                        t0 = T0 + 128 * hf
                        psO = pop.tile([128, 2, 512], f32, tag="psO")
                        for lp in range(12):
                            for h2 in range(2):
                                nc.tensor.matmul(
                                    psO[:, h2, 0:384],
                                    hmidT[:, 2 * lp : 2 * lp + 2, 128 * hf : 128 * (hf + 1)],
                                    fc2w_sb[:, 2 * lp : 2 * lp + 2, 384 * h2 : 384 * (h2 + 1)],
                                    start=(lp == 0), stop=(lp == 11), perf_mode=PM)
                        mo = p2.tile([128, C], f32, tag="mo")
                        nc.scalar.activation(
                            out=mo[:].rearrange("p (a b) -> p a b", a=2),
                            in_=psO[:, :, 0:384], func=FT.Copy, scale=1.0 / 16.0)
                        ot = p2.tile([128, C], f32, tag="ot")
                        nc.gpsimd.tensor_add(out=ot[:], in0=mo[:], in1=h1s[hf][:])
                        nc.gpsimd.tensor_add(out=ot[:], in0=ot[:], in1=fc2b_b[:])
                        nc.sync.dma_start(out=out_p[t0 : t0 + 128, :], in_=ot[:])

                # depth-2 software pipeline: prep(i+2) issues between mm(i)
                # and mm(i+1) so LN2/transposes never stall the PE stream
                sq = [p2_prep(0), p2_prep(1)]
                for it in range(NT2):
                    if it + 2 < NT2:
                        sq.append(p2_prep(it + 2))
                    p2_mm(*sq[it])

    nc.compile()
    return nc


_NC = None


def _get_nc():
    global _NC
    if _NC is None:
        _NC = _build()
    return _NC


def _dft_mats():
    w = np.arange(W); kw = np.arange(KW)
    ang = 2 * np.pi * np.outer(w, kw) / W
    fwr = (np.cos(ang) / np.sqrt(W)).astype(np.float32)
    fwi = (-np.sin(ang) / np.sqrt(W)).astype(np.float32)
    kh = np.arange(H); h = np.arange(H)
    angh = 2 * np.pi * np.outer(kh, h) / H        # [kh, h]
    fhr = np.cos(angh) / np.sqrt(H)
    fhi = -np.sin(angh) / np.sqrt(H)
    fhs = np.zeros((2 * H, 2 * H))
    fhs[:H, :H] = fhr.T; fhs[:H, H:] = fhi.T
    fhs[H:, :H] = -fhi.T; fhs[H:, H:] = fhr.T
    ci = np.cos(angh) / np.sqrt(H)                # [kh, h] for inverse
    si = np.sin(angh) / np.sqrt(H)
    ifhs = np.zeros((2 * H, 2 * H))
    ifhs[:H, :H] = ci; ifhs[:H, H:] = si
    ifhs[H:, :H] = -si; ifhs[H:, H:] = ci
    ckw = np.where(kw == 0, 1.0, 2.0)
    angi = 2 * np.pi * np.outer(kw, np.arange(W)) / W    # [kw, w]
    ifwr = (ckw[:, None] * np.cos(angi) / np.sqrt(W)).astype(np.float32)
    ifwi = (-ckw[:, None] * np.sin(angi) / np.sqrt(W)).astype(np.float32)
    return fwr, fwi, fhs.astype(np.float32), ifhs.astype(np.float32), ifwr, ifwi


def _prepare_in_maps(x, mod_embed, n1w, n1b, n2w, n2b, w1, b1, w2, b2,
                     fs_w0, fs_b0, fs_w1, fs_b1, fc1w, fc1b, fc2w, fc2b,
                     ms_w0, ms_b0, ms_w1, ms_b1):
    f = np.asarray
    x = f(x, dtype=np.float32)
    grid = x.reshape(H, W, C)
    fwr, fwi, fhs, ifhs, ifwr, ifwi = _dft_mats()
    bf = ml_dtypes.bfloat16

    in_maps = []
    for b in range(N):
        r0, r1 = HP * b, min(HP * (b + 1), H)
        xsb = np.zeros((TOKP, C), np.float32)
        xsb[: (r1 - r0) * W] = grid[r0:r1].reshape(-1, C)
        sl = slice(BS * b, BS * (b + 1))
        w2r = f(w2[0, b], np.float32); w2i = f(w2[1, b], np.float32)
        im = {
            "xs": xsb,
            "modT": np.repeat(f(mod_embed, np.float32).reshape(MODD, 1), 2, axis=1).copy(),
            "n1w": f(n1w, np.float32), "n1b": f(n1b, np.float32),
            "n2w": f(n2w, np.float32), "n2b": f(n2b, np.float32),
            "fwr": fwr.astype(np.float16), "fwi": fwi.astype(np.float16),
            "fhs": fhs.astype(np.float16), "ifhs": ifhs.astype(np.float16),
            "ifwr": ifwr.astype(np.float16), "ifwi": ifwi.astype(np.float16),
            "w1r": f(w1[0, b], np.float16).copy(),
            "w1i": f(w1[1, b], np.float16).copy(),
            "w1in": (-f(w1[1, b], np.float16)).copy(),
            "w2cr": np.concatenate([w2r, w2i], axis=1).astype(np.float16),
            "w2ci": np.concatenate([-w2i, w2r], axis=1).astype(np.float16),
            "b1r": f(b1[0, b], np.float32).reshape(BS, 1).copy(),
            "b1i": f(b1[1, b], np.float32).reshape(BS, 1).copy(),
            "b2c": np.concatenate([f(b2[0, b], np.float32), f(b2[1, b], np.float32)]),
            "fs_w0": f(fs_w0, np.float32),
            "fs_b0T": f(fs_b0, np.float32).reshape(12, 128).T.copy(),
            "fs_w1s": np.concatenate(
                [f(fs_w1, np.float32)[:, sl], f(fs_w1, np.float32)[:, C + BS * b : C + BS * (b + 1)]],
                axis=1),
            "fs_b1s": np.concatenate(
                [f(fs_b1, np.float32)[sl], f(fs_b1, np.float32)[C + BS * b : C + BS * (b + 1)]]
            ).reshape(1, -1),
            "ms_w0": f(ms_w0, np.float32),
            "ms_b0T": f(ms_b0, np.float32).reshape(48, 128).T.copy(),
            "ms_w1s": f(ms_w1, np.float32)[:, C * b : C * (b + 1)].astype(bf),
            "ms_b1s": f(ms_b1, np.float32)[C * b : C * (b + 1)].reshape(1, -1),
            "fc1w": (16.0 * f(fc1w, np.float32)).astype(ml_dtypes.float8_e4m3),
            "fc1bT": f(fc1b, np.float32).reshape(24, 128).T.copy(),
            "fc2w": (16.0 * f(fc2w, np.float32)).astype(ml_dtypes.float8_e4m3),
            "fc2b": f(fc2b, np.float32),
        }
        in_maps.append(im)
    return in_maps


def kernel(x, mod_embed, n1w, n1b, n2w, n2b, w1, b1, w2, b2,
           fs_w0, fs_b0, fs_w1, fs_b1, fc1w, fc1b, fc2w, fc2b,
           ms_w0, ms_b0, ms_w1, ms_b1):
    nc = _get_nc()
    in_maps = _prepare_in_maps(
        x, mod_embed, n1w, n1b, n2w, n2b, w1, b1, w2, b2,
        fs_w0, fs_b0, fs_w1, fs_b1, fc1w, fc1b, fc2w, fc2b,
        ms_w0, ms_b0, ms_w1, ms_b1)

    res = run_bass_kernel_spmd(nc, in_maps, core_ids=list(range(N)))
    globals()["last_results"] = res
    out = np.zeros((H, W, C), np.float32)
    for b in range(N):
        r0, r1 = HP * b, min(HP * (b + 1), H)
        out[r0:r1] = res.results[b]["out"][: (r1 - r0) * W].reshape(r1 - r0, W, C)
    return out.reshape(1, H, W, C)



# revision 27
# speedup vs baseline: 1.1264x; 1.0023x over previous
"""AFNO block kernel for 8 Trainium2 NeuronCores.

Sharding: token-shard (H rows, 23 per core padded) for LN/MLP phases;
AllToAll to channel-shard (core i = spectral block i, 96 channels) for the
2D-FFT filter, computed as matmuls against precomputed DFT matrices;
AllToAll back; small AllGather for the column-sharded 6144x6144 scale-shift
MLP weight.
"""

import os
import numpy as np
import ml_dtypes

import concourse.bass as bass
import concourse.bacc as bacc
import concourse.mybir as mybir
import concourse.tile as tile
from concourse.bass_utils import run_bass_kernel_spmd
from concourse.masks import make_identity

f32 = mybir.dt.float32
f32r = mybir.dt.float32r
bf16 = mybir.dt.bfloat16
fp16 = mybir.dt.float16
fp8 = mybir.dt.float8e4
FT = mybir.ActivationFunctionType
OP = mybir.AluOpType

H, W, C = 180, 360, 768
NB, BS, KW = 8, 96, 91
HP = 23                 # rows per shard (8*23 = 184 >= 180)
TOKR = HP * W           # 8280 real token slots per shard
NT2 = 17                # phase-2 tiles of 512
TOKP = NT2 * 512        # 8704 padded tokens per shard
MODD, LAT, LAT2 = 64, 3072, 6144
LAM = 0.01
EPS = 1e-5
N = 8


def rap(t, offset, dims):
    a = t[:] if not isinstance(t, bass.AP) else t
    return bass.AP(tensor=a.tensor, offset=a.offset + offset, ap=[list(d) for d in dims])


def _build():
    nc = bacc.Bacc("TRN2", target_bir_lowering=False, debug=False, num_devices=N)

    def P(name, shp, dt=f32):
        return nc.declare_dram_parameter(name, list(shp), dt, isOutput=False)

    xs = P("xs", [TOKP, C])
    modT = P("modT", [MODD, 2])
    n1w = P("n1w", [C]); n1b = P("n1b", [C])
    n2w = P("n2w", [C]); n2b = P("n2b", [C])
    fwr_p = P("fwr", [W, KW], fp16); fwi_p = P("fwi", [W, KW], fp16)
    fhs_p = P("fhs", [2 * H, 2 * H], fp16)
    ifhs_p = P("ifhs", [2 * H, 2 * H], fp16)
    ifwr_p = P("ifwr", [KW, W], fp16); ifwi_p = P("ifwi", [KW, W], fp16)
    w1r_p = P("w1r", [BS, BS], fp16); w1i_p = P("w1i", [BS, BS], fp16)
    w1in_p = P("w1in", [BS, BS], fp16)
    w2cr_p = P("w2cr", [BS, 2 * BS], fp16)   # [W2r | W2i]
    w2ci_p = P("w2ci", [BS, 2 * BS], fp16)   # [-W2i | W2r]
    b1r_p = P("b1r", [BS, 1]); b1i_p = P("b1i", [BS, 1])
    b2c_p = P("b2c", [2 * BS])               # concat(b2r, b2i)
    fs_w0_p = P("fs_w0", [MODD, 2 * C])
    fs_b0T_p = P("fs_b0T", [128, 12])
    fs_w1s_p = P("fs_w1s", [2 * C, 2 * BS])
    fs_b1s_p = P("fs_b1s", [1, 2 * BS])
    ms_w0_p = P("ms_w0", [MODD, LAT2])
    ms_b0T_p = P("ms_b0T", [128, 48])
    ms_w1s_p = P("ms_w1s", [LAT2, C], bf16)
    ms_b1s_p = P("ms_b1s", [1, C])
    fc1w_p = P("fc1w", [C, LAT], fp8)
    fc1bT_p = P("fc1bT", [128, 24])
    fc2w_p = P("fc2w", [LAT, C], fp8)
    fc2b_p = P("fc2b", [C])
    out_p = nc.declare_dram_parameter("out", [TOKP, C], f32, isOutput=True)
    DBG = False

    # internal DRAM
    a1i = nc.dram_tensor("a1i", [N, TOKR * BS], fp16)
    a1o = nc.dram_tensor("a1o", [N, TOKR * BS], fp16)
    MA = 12 * W * BS        # chunk A: local rows 0-11 per dest
    MB = 11 * W * BS        # chunk B: local rows 12-22
    TA = 12 * W             # tokens per dest covered by chunk A
    a2iA = nc.dram_tensor("a2iA", [N, MA], fp16)
    a2oA = nc.dram_tensor("a2oA", [N, MA], fp16)
    a2iB = nc.dram_tensor("a2iB", [N, MB], fp16)
    a2oB = nc.dram_tensor("a2oB", [N, MB], fp16)
    t1d = nc.dram_tensor("t1d", [KW, 2, H, BS], fp16)   # [kw][ri][h][c]
    ud = nc.dram_tensor("ud", [KW, BS, 2 * H], fp16)
    sfd = nc.dram_tensor("sfd", [1, 2 * BS], f32)
    agi = nc.dram_tensor("agi", [1, C], f32)
    ago = nc.dram_tensor("ago", [N, C], f32)

    RG = [list(range(N))]

    with tile.TileContext(nc) as tc:
        with (
            tc.tile_pool(name="const", bufs=1) as cpool,
            tc.tile_pool(name="ssb", bufs=1) as ssb,
        ):
            # ---- broadcast constants ----
            def bcast(p, n, name):
                t = cpool.tile([128, n], f32, tag=name)
                nc.sync.dma_start(out=t[:], in_=rap(p, 0, [[0, 128], [1, n]]))
                return t

            n1w_b = bcast(n1w, C, "n1w"); n1b_b = bcast(n1b, C, "n1b")
            n2w_b = bcast(n2w, C, "n2w"); n2b_b = bcast(n2b, C, "n2b")
            fc2b_b = bcast(fc2b_p, C, "fc2b")
            b2c_b = cpool.tile([128, 2, 2 * BS], f32, tag="b2c")
            for bh in range(2):
                nc.sync.dma_start(out=b2c_b[:, bh, :],
                                  in_=rap(b2c_p, 0, [[0, 128], [1, 2 * BS]]))
            eps_sb = cpool.tile([128, 1], f32, tag="eps")
            nc.vector.memset(eps_sb[:], EPS)
            nlam_sb = cpool.tile([128, 1], f32, tag="nlam")
            nc.vector.memset(nlam_sb[:], -LAM)
            ident = cpool.tile([128, 128], f32, tag="ident")
            make_identity(nc, ident[:])
            identb = cpool.tile([128, 128], bf16, tag="identb")
            nc.vector.tensor_copy(out=identb[:], in_=ident[:])


            # ---- phase 0: LN1 + scatter into A2A-1 send buffer ----
            with (
                tc.tile_pool(name="p0", bufs=6) as p0,
                tc.tile_pool(name="p0s", bufs=8) as p0s,
            ):
                for it in range(65):
                    t0 = it * 128
                    nrow = min(128, TOKR - t0)
                    xt = p0.tile([128, C], f32, tag="xt")
                    nc.sync.dma_start(out=xt[:], in_=xs[t0 : t0 + 128, :])
                    st = p0s.tile([128, 2, 6], f32, tag="st")
                    for g in range(2):
                        nc.vector.bn_stats(out=st[:, g, :], in_=xt[:, 384 * g : 384 * (g + 1)])
                    mv = p0s.tile([128, 2], f32, tag="mv")
                    nc.vector.bn_aggr(out=mv[:], in_=st[:])
                    rstd = p0s.tile([128, 1], f32, tag="rstd")
                    nc.scalar.activation(out=rstd[:], in_=mv[:, 1:2], func=FT.Sqrt,
                                         bias=eps_sb[:], scale=1.0)
                    nc.vector.reciprocal(out=rstd[:], in_=rstd[:])
                    ln = p0.tile([128, C], f32, tag="ln")
                    nc.vector.tensor_scalar(out=ln[:], in0=xt[:], scalar1=mv[:, 0:1],
                                            scalar2=rstd[:], op0=OP.subtract, op1=OP.mult)
                    aeng = nc.gpsimd if it % 3 == 2 else nc.vector
                    aeng.tensor_mul(out=ln[:], in0=ln[:], in1=n1w_b[:])
                    lnh = p0.tile([128, C], fp16, tag="lnh")
                    aeng.tensor_add(out=lnh[:], in0=ln[:], in1=n1b_b[:])
                    nc.scalar.dma_start(
                        out=rap(a1i, t0 * BS, [[BS, nrow], [TOKR * BS, N], [1, BS]]),
                        in_=lnh[:nrow].rearrange("p (j c) -> p j c", j=N),
                    )

            nc.gpsimd.collective_compute(
                "AllToAll", OP.bypass, replica_groups=RG, ins=[a1i[:]], outs=[a1o[:]])

            # ---- scale-shift MLPs (overlap with A2A1 window) ----
            ss_ctx = tc.tile_pool(name="ssw", bufs=1)
            ssw = ss_ctx.__enter__()
            ssp_ctx = tc.tile_pool(name="ssp", bufs=1, space="PSUM")
            ssp = ssp_ctx.__enter__()
            modT_sb = ssw.tile([MODD, 2], f32r)
            nc.sync.dma_start(out=modT_sb[:], in_=modT[:].bitcast(f32r))
            fs_w0_sb = ssw.tile([MODD, 2 * C], f32r)
            nc.sync.dma_start(out=fs_w0_sb[:], in_=fs_w0_p[:].bitcast(f32r))
            fs_b0T_sb = ssw.tile([128, 12], f32)
            nc.sync.dma_start(out=fs_b0T_sb[:], in_=fs_b0T_p[:])
            e0T = ssw.tile([128, 12], f32r)
            for j in range(12):
                pt = ssp.tile([128, 2], f32, tag="ss1")
                nc.tensor.matmul(pt[:], fs_w0_sb[:, 128 * j : 128 * (j + 1)],
                                 modT_sb[:], start=True, stop=True)
                nc.scalar.activation(out=e0T[:, j : j + 1], in_=pt[:, 0:1], func=FT.Gelu,
                                     bias=fs_b0T_sb[:, j : j + 1], scale=1.0)
            fs_w1s_sb = ssw.tile([128, 12, 2 * BS], f32r)
            nc.sync.dma_start(
                out=fs_w1s_sb[:],
                in_=rap(fs_w1s_p, 0, [[2 * BS, 128], [128 * 2 * BS, 12], [1, 2 * BS]]).bitcast(f32r),
            )
            fs_b1s_sb = ssw.tile([1, 2 * BS], f32)
            nc.sync.dma_start(out=fs_b1s_sb[:], in_=fs_b1s_p[:])
            p2 = ssp.tile([1, 2 * BS], f32, tag="ss2")
            for j in range(12):
                nc.tensor.matmul(p2[:], e0T[:, j : j + 1], fs_w1s_sb[:, j, :],
                                 start=(j == 0), stop=(j == 11))
            sfo = ssw.tile([1, 2 * BS], f32)
            nc.vector.tensor_add(out=sfo[:], in0=p2[:], in1=fs_b1s_sb[:])
            nc.sync.dma_start(out=sfd[:], in_=sfo[:])
            sfT = ssw.tile([BS, 2], f32)
            nc.sync.dma_start(out=sfT[:], in_=rap(sfd, 0, [[1, BS], [BS, 2]]))
            sfv = ssb.tile([BS, 1], f32)
            nc.vector.tensor_scalar_add(out=sfv[:], in0=sfT[:, 0:1], scalar1=1.0)
            b1r_sb = ssw.tile([BS, 1], f32)
            nc.sync.dma_start(out=b1r_sb[:], in_=b1r_p[:])
            b1i_sb = ssw.tile([BS, 1], f32)
            nc.sync.dma_start(out=b1i_sb[:], in_=b1i_p[:])
            Br = ssb.tile([BS, 1], f32)
            nc.vector.tensor_mul(out=Br[:], in0=b1r_sb[:], in1=sfv[:])
            nc.vector.tensor_add(out=Br[:], in0=Br[:], in1=sfT[:, 1:2])
            Bi = ssb.tile([BS, 1], f32)
            nc.vector.tensor_mul(out=Bi[:], in0=b1i_sb[:], in1=sfv[:])
            nc.vector.tensor_add(out=Bi[:], in0=Bi[:], in1=sfT[:, 1:2])

            # ms MLP: e1T then column-sharded 6144->768, AllGather
            ms_w0_sb = ssw.tile([MODD, LAT2], f32r)
            nc.gpsimd.dma_start(out=ms_w0_sb[:], in_=ms_w0_p[:].bitcast(f32r))
            ms_b0T_sb = ssw.tile([128, 48], f32)
            nc.sync.dma_start(out=ms_b0T_sb[:], in_=ms_b0T_p[:])
            e1T = ssw.tile([128, 48], bf16)
            for j in range(48):
                pt = ssp.tile([128, 2], f32, tag="ss1")
                nc.tensor.matmul(pt[:], ms_w0_sb[:, 128 * j : 128 * (j + 1)],
                                 modT_sb[:], start=True, stop=True)
                nc.scalar.activation(out=e1T[:, j : j + 1], in_=pt[:, 0:1], func=FT.Gelu,
                                     bias=ms_b0T_sb[:, j : j + 1], scale=1.0)
            p3 = ssp.tile([1, 2, 512], f32, tag="ss3")
            with tc.tile_pool(name="msw", bufs=3) as mswp:
                for j in range(48):
                    wt = mswp.tile([128, C], bf16)
                    nc.gpsimd.dma_start(
                        out=wt[:], in_=ms_w1s_p[128 * j : 128 * (j + 1), :])
                    for h2 in range(2):
                        nc.tensor.matmul(
                            p3[:, h2, 0:384], e1T[:, j : j + 1],
                            wt[:, 384 * h2 : 384 * (h2 + 1)],
                            start=(j == 0), stop=(j == 47))
            ms_b1s_sb = ssw.tile([1, C], f32)
            nc.sync.dma_start(out=ms_b1s_sb[:], in_=ms_b1s_p[:])
            mso = ssw.tile([1, C], f32)
            nc.vector.tensor_add(out=mso[:].rearrange("p (a b) -> p a b", a=2),
                                 in0=p3[:, :, 0:384],
                                 in1=ms_b1s_sb[:].rearrange("p (a b) -> p a b", a=2))
            nc.sync.dma_start(out=agi[:], in_=mso[:])
            nc.gpsimd.collective_compute(
                "AllGather", OP.bypass, replica_groups=RG, ins=[agi[:]], outs=[ago[:]])
            sM = ssb.tile([128, 24], f32)
            nc.sync.dma_start(out=sM[:], in_=rap(ago, 0, [[1, 128], [128, 24]]))
            nc.vector.tensor_scalar_add(out=sM[:], in0=sM[:], scalar1=1.0)
            tM = ssb.tile([128, 24], f32)
            nc.sync.dma_start(out=tM[:], in_=rap(ago, LAT, [[1, 128], [128, 24]]))
            fc1bT_sb = ssw.tile([128, 24], f32)
            nc.sync.dma_start(out=fc1bT_sb[:], in_=fc1bT_p[:])
            B1 = ssb.tile([128, 24], f32)
            nc.vector.tensor_mul(out=B1[:], in0=fc1bT_sb[:], in1=sM[:])
            nc.vector.tensor_add(out=B1[:], in0=B1[:], in1=tM[:])
            sM16 = ssb.tile([128, 24], f32)
            nc.vector.tensor_scalar_mul(out=sM16[:], in0=sM[:], scalar1=1.0 / 16.0)

            ssp_ctx.__exit__(None, None, None)
            ss_ctx.__exit__(None, None, None)

            # ---- phase 1 stage A: W-DFT  (X[h,w,c] -> t1d[kw,ri,h,c]) ----
            with (
                tc.tile_pool(name="sa", bufs=1) as sa,
                tc.tile_pool(name="sac", bufs=3) as sac,
                tc.tile_pool(name="sap", bufs=2, space="PSUM") as sap,
            ):
                fw_sb = []
                for ri, p in enumerate([fwr_p, fwi_p]):
                    t = sa.tile([120, 3, KW], fp16, tag=f"fw{ri}")
                    nc.sync.dma_start(
                        out=t[:], in_=rap(p, 0, [[KW, 120], [120 * KW, 3], [1, KW]]))
                    fw_sb.append(t)
                X_sb = sa.tile([120, 3, H, BS], fp16, tag="xsb")
                for ch in range(4):
                    for k in range(3):
                        eng = nc.scalar if (ch + k) % 2 else nc.sync
                        eng.dma_start(
                            out=X_sb[:, k, 45 * ch : 45 * (ch + 1), :],
                            in_=rap(a1o, (45 * ch * W + 120 * k) * BS,
                                    [[BS, 120], [W * BS, 45], [1, BS]]))
                for hs in range(36):
                    hh0 = 5 * hs
                    cp = sac.tile([KW, 2, 5, BS], fp16, tag="cpa")
                    for ri in range(2):
                        ps = sap.tile([KW, 5, BS], f32, tag="pa")
                        for k in range(3):
                            nc.tensor.matmul(ps[:], fw_sb[ri][:, k, :],
                                             X_sb[:, k, hh0 : hh0 + 5, :],
                                             start=(k == 0), stop=(k == 2))
                        nc.vector.tensor_copy(out=cp[:, ri, :, :], in_=ps[:])
                    nc.scalar.dma_start(
                        out=rap(t1d, hh0 * BS,
                                [[2 * H * BS, KW], [H * BS, 2], [BS, 5], [1, BS]]),
                        in_=cp[:])

            # ---- stages B+C+D fused in SBUF, then E ----
            with tc.tile_pool(name="fb", bufs=1) as fb:
                fhs_sb = fb.tile([90, 4, 2 * H], fp16)
                nc.sync.dma_start(
                    out=fhs_sb[:],
                    in_=rap(fhs_p, 0, [[2 * H, 90], [90 * 2 * H, 4], [1, 2 * H]]))
                ifhs_sb = fb.tile([90, 4, 2 * H], fp16)
                nc.sync.dma_start(
                    out=ifhs_sb[:],
                    in_=rap(ifhs_p, 0, [[2 * H, 90], [90 * 2 * H, 4], [1, 2 * H]]))
                w1r_sb = fb.tile([BS, BS], fp16)
                nc.sync.dma_start(out=w1r_sb[:], in_=w1r_p[:])
                w1i_sb = fb.tile([BS, BS], fp16)
                nc.sync.dma_start(out=w1i_sb[:], in_=w1i_p[:])
                w1in_sb = fb.tile([BS, BS], fp16)
                nc.sync.dma_start(out=w1in_sb[:], in_=w1in_p[:])
                w2cr_sb = fb.tile([BS, 2 * BS], fp16)
                nc.sync.dma_start(out=w2cr_sb[:], in_=w2cr_p[:])
                w2ci_sb = fb.tile([BS, 2 * BS], fp16)
                nc.sync.dma_start(out=w2ci_sb[:], in_=w2ci_p[:])
                ifw_sb = []
                for ri, p in enumerate([ifwr_p, ifwi_p]):
                    t = fb.tile([KW, 3, 120], fp16, tag=f"ifw{ri}")
                    nc.sync.dma_start(
                        out=t[:], in_=rap(p, 0, [[W, KW], [120, 3], [1, 120]]))
                    ifw_sb.append(t)
                # T1 resident: [h(90), half, ri, kw, c]
                T1_sb = fb.tile([90, 2, 2, KW, BS], fp16, tag="t1sb")
                for half in range(2):
                    for ri in range(2):
                        eng = nc.scalar if ri else nc.sync
                        eng.dma_start(
                            out=T1_sb[:, half, ri, :, :],
                            in_=rap(t1d, (ri * H + half * 90) * BS,
                                    [[BS, 90], [2 * H * BS, KW], [1, BS]]))
                U_sb = fb.tile([KW, BS, 2 * H], fp16, tag="usb")

                with (
                    tc.tile_pool(name="bcw", bufs=4) as bcw,
                    tc.tile_pool(name="bct", bufs=4) as bct,
                    tc.tile_pool(name="bco", bufs=3) as bco,
                    tc.tile_pool(name="bcp1", bufs=1, space="PSUM") as bcps,
                    tc.tile_pool(name="bcp2", bufs=1, space="PSUM") as bcps2,
                    tc.tile_pool(name="bcp3", bufs=2, space="PSUM") as bcps3,
                    tc.tile_pool(name="bcp4", bufs=1, space="PSUM") as bcps4,
                ):
                    def emit_D(O2t, kw0, G):
                        psU = bcps4.tile([BS, 2, 512], f32, tag="psU")
                        for g in range(G):
                            for q in range(4):
                                ri, half = q // 2, q % 2
                                nc.tensor.matmul(
                                    psU[:, g, 0 : 2 * H],
                                    O2t[:, g, half, ri * BS : (ri + 1) * BS],
                                    ifhs_sb[:, q, :], start=(q == 0), stop=(q == 3))
                        ucp = bcw.tile([BS, 2, 2 * H], fp16, tag="ucp")
                        nc.vector.tensor_copy(out=ucp[:, :G, :], in_=psU[:, :G, 0 : 2 * H])
                        nc.sync.dma_start(
                            out=rap(ud, kw0 * BS * 2 * H,
                                    [[2 * H, BS], [BS * 2 * H, G], [1, 2 * H]]),
                            in_=ucp[:, :G, :])

                    pend = None
                    for pr in range(46):
                        kw0 = 2 * pr
                        G = 2 if kw0 + 1 < KW else 1
                        psF = bcps.tile([BS, 2, 512], f32, tag="psF")
                        for g in range(G):
                            kw = kw0 + g
                            for q in range(4):
                                ri, half = q // 2, q % 2
                                nc.tensor.matmul(
                                    psF[:, g, 0 : 2 * H], T1_sb[:, half, ri, kw, :],
                                    fhs_sb[:, q, :], start=(q == 0), stop=(q == 3))
                        fsb = bcw.tile([BS, 2, 2 * H], fp16, tag="fsb")
                        nc.vector.tensor_copy(out=fsb[:, :G, :], in_=psF[:, :G, 0 : 2 * H])
                        ps1r = bcps2.tile([BS, 2, H], f32, tag="ps1r")
                        ps1i = bcps2.tile([BS, 2, H], f32, tag="ps1i")
                        nc.tensor.matmul(ps1r[:, :G, :], w1r_sb[:], fsb[:, :G, 0:H],
                                         start=True, stop=False)
                        nc.tensor.matmul(ps1r[:, :G, :], w1in_sb[:], fsb[:, :G, H : 2 * H],
                                         start=False, stop=True)
                        nc.tensor.matmul(ps1i[:, :G, :], w1i_sb[:], fsb[:, :G, 0:H],
                                         start=True, stop=False)
                        nc.tensor.matmul(ps1i[:, :G, :], w1r_sb[:], fsb[:, :G, H : 2 * H],
                                         start=False, stop=True)
                        o1r = bcw.tile([BS, 2, H], fp16, tag="o1r")
                        o1i = bcw.tile([BS, 2, H], fp16, tag="o1i")
                        nc.scalar.activation(out=o1r[:, :G, :], in_=ps1r[:, :G, :],
                                             func=FT.Relu, bias=Br[:], scale=sfv[:])
                        nc.scalar.activation(out=o1i[:, :G, :], in_=ps1i[:, :G, :],
                                             func=FT.Relu, bias=Bi[:], scale=sfv[:])
                        o1rf = o1r[:].rearrange("p g k -> p (g k)")
                        o1if = o1i[:].rearrange("p g k -> p (g k)")
                        O2t = bco.tile([90, 2, 2, 2 * BS], fp16, tag="o2sb")
                        for g in range(G):
                            ps2 = bcps3.tile([90, 2, 2 * BS], f32, tag="ps2")
                            for half in range(2):
                                sl = slice(90 * (2 * g + half), 90 * (2 * g + half + 1))
                                nc.tensor.matmul(ps2[:, half, :], o1rf[:, sl], w2cr_sb[:],
                                                 start=True, stop=False)
                                nc.tensor.matmul(ps2[:, half, :], o1if[:, sl], w2ci_sb[:],
                                                 start=False, stop=True)
                            tmp = bct.tile([90, 2, 2 * BS], f32, tag="tmp")
                            nc.vector.tensor_add(out=tmp[:], in0=ps2[:], in1=b2c_b[:90])
                            r1 = bct.tile([90, 2, 2 * BS], f32, tag="r1")
                            nc.vector.tensor_scalar(out=r1[:], in0=tmp[:], scalar1=-LAM,
                                                    scalar2=LAM, op0=OP.max, op1=OP.min)
                            nc.vector.tensor_sub(out=O2t[:, g, :, :],
                                                 in0=tmp[:], in1=r1[:])
                        # stage D (inverse H-DFT) pipelined one iteration
                        # behind B/C so the softshrink chain never stalls
                        # the in-order tensor queue
                        if pend is not None:
                            emit_D(*pend)
                        pend = (O2t, kw0, G)
                    emit_D(*pend)

                # U back to kw-partitioned SBUF, then stage E (inverse W-DFT)
                for chv in range(2):
                    k0 = 46 * chv
                    kn = min(46, KW - k0)
                    eng = nc.scalar if chv else nc.sync
                    eng.dma_start(
                        out=U_sb[k0 : k0 + kn, :, :],
                        in_=rap(ud, k0 * BS * 2 * H,
                                [[BS * 2 * H, kn], [2 * H, BS], [1, 2 * H]]))
                with (
                    tc.tile_pool(name="sec", bufs=4) as sec,
                    tc.tile_pool(name="sep", bufs=2, space="PSUM") as sep,
                ):
                    # chunk A = local rows 0-11 per dest, chunk B = rows 12-22;
                    # A2A for chunk A overlaps stage-E compute of chunk B.
                    for part in range(2):
                        rts = [(0, 4), (4, 4), (8, 4)] if part == 0 else \
                              [(12, 4), (16, 4), (20, 3)]
                        for d in range(N):
                            for roff, nr in rts:
                                h0 = HP * d + roff
                                nr = min(nr, H - h0)
                                if nr <= 0:
                                    continue
                                for wk in range(3):
                                    psE = sep.tile([120, 4, BS], f32, tag="psE")
                                    for ri in range(2):
                                        nc.tensor.matmul(
                                            psE[:, :nr, :], ifw_sb[ri][:, wk, :],
                                            U_sb[:, :, ri * H + h0 : ri * H + h0 + nr]
                                                .rearrange("p c h -> p h c"),
                                            start=(ri == 0), stop=(ri == 1))
                                    ecp = sec.tile([120, 4, BS], fp16, tag="ecp")
                                    nc.vector.tensor_copy(out=ecp[:, :nr, :],
                                                          in_=psE[:, :nr, :])
                                    if part == 0:
                                        dst, off = a2iA, d * MA + (roff * W + wk * 120) * BS
                                    else:
                                        dst, off = a2iB, d * MB + ((roff - 12) * W + wk * 120) * BS
                                    nc.scalar.dma_start(
                                        out=rap(dst, off, [[BS, 120], [W * BS, nr], [1, BS]]),
                                        in_=ecp[:, :nr, :])
                        if part == 0:
                            nc.gpsimd.collective_compute(
                                "AllToAll", OP.bypass, replica_groups=RG,
                                ins=[a2iA[:]], outs=[a2oA[:]])

            nc.gpsimd.collective_compute(
                "AllToAll", OP.bypass, replica_groups=RG, ins=[a2iB[:]], outs=[a2oB[:]])

            # ---- phase 2: h1 = F2 + ln1x + x; LN2; modulated MLP; + h1 ----
            with (
                tc.tile_pool(name="p2w", bufs=1) as p2w,
                tc.tile_pool(name="p2", bufs=4) as p2,
                tc.tile_pool(name="p2h", bufs=12) as p2h,
                tc.tile_pool(name="p2s", bufs=4) as p2s,
                tc.tile_pool(name="p2m", bufs=3) as p2m,
                tc.tile_pool(name="ptp", bufs=2, space="PSUM") as ptp,
                tc.tile_pool(name="php", bufs=2, space="PSUM") as php,
                tc.tile_pool(name="pop", bufs=2, space="PSUM") as pop,
            ):
                PM = mybir.MatmulPerfMode.DoubleRow
                fc1w_sb = p2w.tile([128, 6, LAT], fp8)
                nc.sync.dma_start(
                    out=fc1w_sb[:], in_=rap(fc1w_p, 0, [[LAT, 128], [128 * LAT, 6], [1, LAT]]))
                fc2w_sb = p2w.tile([128, 24, C], fp8)
                nc.sync.dma_start(
                    out=fc2w_sb[:], in_=rap(fc2w_p, 0, [[C, 128], [128 * C, 24], [1, C]]))

                def p2_prep(it):
                    T0 = it * 512
                    ln2T = p2m.tile([128, 6, 4, 128], fp8, tag="ln2T")
                    h1s = []
                    for hf in range(4):
                        t0 = T0 + 128 * hf
                        nload = max(0, min(128, TOKR - t0))
                        xt = p2.tile([128, C], f32, tag="xt2")
                        nc.sync.dma_start(out=xt[:], in_=xs[t0 : t0 + 128, :])
                        f2t = p2.tile([128, N, BS], fp16, tag="f2t")
                        l1t = p2.tile([128, N, BS], fp16, tag="l1t")
                        if nload < 128:
                            nc.vector.memset(f2t[:], 0.0)
                            nc.vector.memset(l1t[:], 0.0)
                        if nload > 0:
                            ta = min(nload, max(0, TA - t0))
                            if ta > 0:
                                nc.sync.dma_start(
                                    out=f2t[:ta],
                                    in_=rap(a2oA, t0 * BS, [[BS, ta], [MA, N], [1, BS]]))
                            if ta < nload:
                                t0b = t0 + ta - TA
                                nc.sync.dma_start(
                                    out=f2t[ta:nload],
                                    in_=rap(a2oB, t0b * BS, [[BS, nload - ta], [MB, N], [1, BS]]))
                            nc.sync.dma_start(
                                out=l1t[:nload],
                                in_=rap(a1i, t0 * BS, [[BS, nload], [TOKR * BS, N], [1, BS]]))
                        h1 = p2h.tile([128, C], f32, tag="h1")
                        nc.vector.tensor_add(out=h1[:], in0=xt[:],
                                             in1=f2t[:].rearrange("p j c -> p (j c)"))
                        nc.vector.tensor_add(out=h1[:], in0=h1[:],
                                             in1=l1t[:].rearrange("p j c -> p (j c)"))
                        h1s.append(h1)
                        st = p2s.tile([128, 2, 6], f32, tag="st2")
                        for g in range(2):
                            nc.vector.bn_stats(out=st[:, g, :], in_=h1[:, 384 * g : 384 * (g + 1)])
                        mv = p2s.tile([128, 2], f32, tag="mv2")
                        nc.vector.bn_aggr(out=mv[:], in_=st[:])
                        rstd = p2s.tile([128, 1], f32, tag="rstd2")
                        nc.scalar.activation(out=rstd[:], in_=mv[:, 1:2], func=FT.Sqrt,
                                             bias=eps_sb[:], scale=1.0)
                        nc.vector.reciprocal(out=rstd[:], in_=rstd[:])
                        ln2 = p2.tile([128, C], bf16, tag="ln2")
                        nc.vector.tensor_scalar(out=ln2[:], in0=h1[:], scalar1=mv[:, 0:1],
                                                scalar2=rstd[:], op0=OP.subtract, op1=OP.mult)
                        nc.vector.tensor_mul(out=ln2[:], in0=ln2[:], in1=n2w_b[:])
                        nc.vector.tensor_add(out=ln2[:], in0=ln2[:], in1=n2b_b[:])
                        for jb in range(2):
                            pst = ptp.tile([128, 3, 128], bf16, tag="pst")
                            for jj in range(3):
                                j = 3 * jb + jj
                                nc.tensor.transpose(pst[:, jj, :],
                                                    ln2[:, 128 * j : 128 * (j + 1)],
                                                    identb[:])
                            nc.vector.tensor_copy(out=ln2T[:, 3 * jb : 3 * jb + 3, hf, :],
                                                  in_=pst[:])
                    return T0, ln2T, h1s

                def p2_mm(T0, ln2T, h1s):
                    hmidT = p2m.tile([128, 24, 512], fp8, tag="hmidT")
                    for l in range(24):
                        psH = php.tile([128, 512], f32, tag="psH")
                        for jp in range(3):
                            nc.tensor.matmul(
                                psH[:],
                                fc1w_sb[:, 2 * jp : 2 * jp + 2, 128 * l : 128 * (l + 1)],
                                ln2T[:, 2 * jp : 2 * jp + 2, :, :]
                                    .rearrange("p j h t -> p j (h t)"),
                                start=(jp == 0), stop=(jp == 2), perf_mode=PM)
                        nc.scalar.activation(out=hmidT[:, l, :], in_=psH[:], func=FT.Gelu,
                                             bias=B1[:, l : l + 1], scale=sM16[:, l : l + 1])
                    for hf in range(4):
                        t0 = T0 + 128 * hf
                        psO = pop.tile([128, 2, 512], f32, tag="psO")
                        for lp in range(12):
                            for h2 in range(2):
                                nc.tensor.matmul(
                                    psO[:, h2, 0:384],
                                    hmidT[:, 2 * lp : 2 * lp + 2, 128 * hf : 128 * (hf + 1)],
                                    fc2w_sb[:, 2 * lp : 2 * lp + 2, 384 * h2 : 384 * (h2 + 1)],
                                    start=(lp == 0), stop=(lp == 11), perf_mode=PM)
                        ot = p2.tile([128, C], f32, tag="ot")
                        nc.vector.scalar_tensor_tensor(
                            out=ot[:].rearrange("p (a b) -> p a b", a=2),
                            in0=psO[:, :, 0:384], scalar=1.0 / 16.0,
                            in1=h1s[hf][:].rearrange("p (a b) -> p a b", a=2),
                            op0=OP.mult, op1=OP.add)
                        nc.gpsimd.tensor_add(out=ot[:], in0=ot[:], in1=fc2b_b[:])
                        nc.sync.dma_start(out=out_p[t0 : t0 + 128, :], in_=ot[:])

                # depth-2 software pipeline: prep(i+2) issues between mm(i)
                # and mm(i+1) so LN2/transposes never stall the PE stream
                sq = [p2_prep(0), p2_prep(1)]
                for it in range(NT2):
                    if it + 2 < NT2:
                        sq.append(p2_prep(it + 2))
                    p2_mm(*sq[it])

    nc.compile()
    return nc


_NC = None


def _get_nc():
    global _NC
    if _NC is None:
        _NC = _build()
    return _NC


def _dft_mats():
    w = np.arange(W); kw = np.arange(KW)
    ang = 2 * np.pi * np.outer(w, kw) / W
    fwr = (np.cos(ang) / np.sqrt(W)).astype(np.float32)
    fwi = (-np.sin(ang) / np.sqrt(W)).astype(np.float32)
    kh = np.arange(H); h = np.arange(H)
    angh = 2 * np.pi * np.outer(kh, h) / H        # [kh, h]
    fhr = np.cos(angh) / np.sqrt(H)
    fhi = -np.sin(angh) / np.sqrt(H)
    fhs = np.zeros((2 * H, 2 * H))
    fhs[:H, :H] = fhr.T; fhs[:H, H:] = fhi.T
    fhs[H:, :H] = -fhi.T; fhs[H:, H:] = fhr.T
    ci = np.cos(angh) / np.sqrt(H)                # [kh, h] for inverse
    si = np.sin(angh) / np.sqrt(H)
    ifhs = np.zeros((2 * H, 2 * H))
    ifhs[:H, :H] = ci; ifhs[:H, H:] = si
    ifhs[H:, :H] = -si; ifhs[H:, H:] = ci
    ckw = np.where(kw == 0, 1.0, 2.0)
    angi = 2 * np.pi * np.outer(kw, np.arange(W)) / W    # [kw, w]
    ifwr = (ckw[:, None] * np.cos(angi) / np.sqrt(W)).astype(np.float32)
    ifwi = (-ckw[:, None] * np.sin(angi) / np.sqrt(W)).astype(np.float32)
    return fwr, fwi, fhs.astype(np.float32), ifhs.astype(np.float32), ifwr, ifwi


def _prepare_in_maps(x, mod_embed, n1w, n1b, n2w, n2b, w1, b1, w2, b2,
                     fs_w0, fs_b0, fs_w1, fs_b1, fc1w, fc1b, fc2w, fc2b,
                     ms_w0, ms_b0, ms_w1, ms_b1):
    f = np.asarray
    x = f(x, dtype=np.float32)
    grid = x.reshape(H, W, C)
    fwr, fwi, fhs, ifhs, ifwr, ifwi = _dft_mats()
    bf = ml_dtypes.bfloat16

    in_maps = []
    for b in range(N):
        r0, r1 = HP * b, min(HP * (b + 1), H)
        xsb = np.zeros((TOKP, C), np.float32)
        xsb[: (r1 - r0) * W] = grid[r0:r1].reshape(-1, C)
        sl = slice(BS * b, BS * (b + 1))
        w2r = f(w2[0, b], np.float32); w2i = f(w2[1, b], np.float32)
        im = {
            "xs": xsb,
            "modT": np.repeat(f(mod_embed, np.float32).reshape(MODD, 1), 2, axis=1).copy(),
            "n1w": f(n1w, np.float32), "n1b": f(n1b, np.float32),
            "n2w": f(n2w, np.float32), "n2b": f(n2b, np.float32),
            "fwr": fwr.astype(np.float16), "fwi": fwi.astype(np.float16),
            "fhs": fhs.astype(np.float16), "ifhs": ifhs.astype(np.float16),
            "ifwr": ifwr.astype(np.float16), "ifwi": ifwi.astype(np.float16),
            "w1r": f(w1[0, b], np.float16).copy(),
            "w1i": f(w1[1, b], np.float16).copy(),
            "w1in": (-f(w1[1, b], np.float16)).copy(),
            "w2cr": np.concatenate([w2r, w2i], axis=1).astype(np.float16),
            "w2ci": np.concatenate([-w2i, w2r], axis=1).astype(np.float16),
            "b1r": f(b1[0, b], np.float32).reshape(BS, 1).copy(),
            "b1i": f(b1[1, b], np.float32).reshape(BS, 1).copy(),
            "b2c": np.concatenate([f(b2[0, b], np.float32), f(b2[1, b], np.float32)]),
            "fs_w0": f(fs_w0, np.float32),
            "fs_b0T": f(fs_b0, np.float32).reshape(12, 128).T.copy(),
            "fs_w1s": np.concatenate(
                [f(fs_w1, np.float32)[:, sl], f(fs_w1, np.float32)[:, C + BS * b : C + BS * (b + 1)]],
                axis=1),
            "fs_b1s": np.concatenate(
                [f(fs_b1, np.float32)[sl], f(fs_b1, np.float32)[C + BS * b : C + BS * (b + 1)]]
            ).reshape(1, -1),
            "ms_w0": f(ms_w0, np.float32),
            "ms_b0T": f(ms_b0, np.float32).reshape(48, 128).T.copy(),
            "ms_w1s": f(ms_w1, np.float32)[:, C * b : C * (b + 1)].astype(bf),
            "ms_b1s": f(ms_b1, np.float32)[C * b : C * (b + 1)].reshape(1, -1),
            "fc1w": (16.0 * f(fc1w, np.float32)).astype(ml_dtypes.float8_e4m3),
            "fc1bT": f(fc1b, np.float32).reshape(24, 128).T.copy(),
            "fc2w": (16.0 * f(fc2w, np.float32)).astype(ml_dtypes.float8_e4m3),
            "fc2b": f(fc2b, np.float32),
        }
        in_maps.append(im)
    return in_maps


def kernel(x, mod_embed, n1w, n1b, n2w, n2b, w1, b1, w2, b2,
           fs_w0, fs_b0, fs_w1, fs_b1, fc1w, fc1b, fc2w, fc2b,
           ms_w0, ms_b0, ms_w1, ms_b1):
    nc = _get_nc()
    in_maps = _prepare_in_maps(
        x, mod_embed, n1w, n1b, n2w, n2b, w1, b1, w2, b2,
        fs_w0, fs_b0, fs_w1, fs_b1, fc1w, fc1b, fc2w, fc2b,
        ms_w0, ms_b0, ms_w1, ms_b1)

    res = run_bass_kernel_spmd(nc, in_maps, core_ids=list(range(N)))
    globals()["last_results"] = res
    out = np.zeros((H, W, C), np.float32)
    for b in range(N):
        r0, r1 = HP * b, min(HP * (b + 1), H)
        out[r0:r1] = res.results[b]["out"][: (r1 - r0) * W].reshape(r1 - r0, W, C)
    return out.reshape(1, H, W, C)



# revision 34
# speedup vs baseline: 1.1340x; 1.0068x over previous
"""AFNO block kernel for 8 Trainium2 NeuronCores.

Sharding: token-shard (H rows, 23 per core padded) for LN/MLP phases;
AllToAll to channel-shard (core i = spectral block i, 96 channels) for the
2D-FFT filter, computed as matmuls against precomputed DFT matrices;
AllToAll back; small AllGather for the column-sharded 6144x6144 scale-shift
MLP weight.
"""

import os
import numpy as np
import ml_dtypes

import concourse.bass as bass
import concourse.bacc as bacc
import concourse.mybir as mybir
import concourse.tile as tile
from concourse.bass_utils import run_bass_kernel_spmd
from concourse.masks import make_identity

f32 = mybir.dt.float32
f32r = mybir.dt.float32r
bf16 = mybir.dt.bfloat16
fp16 = mybir.dt.float16
fp8 = mybir.dt.float8e4
FT = mybir.ActivationFunctionType
OP = mybir.AluOpType

H, W, C = 180, 360, 768
NB, BS, KW = 8, 96, 91
HP = 23                 # rows per shard (8*23 = 184 >= 180)
TOKR = HP * W           # 8280 real token slots per shard
NT2 = 17                # phase-2 tiles of 512
TOKP = NT2 * 512        # 8704 padded tokens per shard
MODD, LAT, LAT2 = 64, 3072, 6144
LAM = 0.01
EPS = 1e-5
N = 8


def rap(t, offset, dims):
    a = t[:] if not isinstance(t, bass.AP) else t
    return bass.AP(tensor=a.tensor, offset=a.offset + offset, ap=[list(d) for d in dims])


def _build():
    nc = bacc.Bacc("TRN2", target_bir_lowering=False, debug=False, num_devices=N)

    def P(name, shp, dt=f32):
        return nc.declare_dram_parameter(name, list(shp), dt, isOutput=False)

    xs = P("xs", [TOKP, C])
    modT = P("modT", [MODD, 2])
    n1w = P("n1w", [C]); n1b = P("n1b", [C])
    n2w = P("n2w", [C]); n2b = P("n2b", [C])
    fwr_p = P("fwr", [W, KW], fp16); fwi_p = P("fwi", [W, KW], fp16)
    fhs_p = P("fhs", [2 * H, 2 * H], fp16)
    ifhs_p = P("ifhs", [2 * H, 2 * H], fp16)
    ifwr_p = P("ifwr", [KW, W], fp16); ifwi_p = P("ifwi", [KW, W], fp16)
    w1r_p = P("w1r", [BS, BS], fp16); w1i_p = P("w1i", [BS, BS], fp16)
    w1in_p = P("w1in", [BS, BS], fp16)
    w2cr_p = P("w2cr", [BS, 2 * BS], fp16)   # [W2r | W2i]
    w2ci_p = P("w2ci", [BS, 2 * BS], fp16)   # [-W2i | W2r]
    b1r_p = P("b1r", [BS, 1]); b1i_p = P("b1i", [BS, 1])
    b2c_p = P("b2c", [2 * BS])               # concat(b2r, b2i)
    fs_w0_p = P("fs_w0", [MODD, 2 * C])
    fs_b0T_p = P("fs_b0T", [128, 12])
    fs_w1s_p = P("fs_w1s", [2 * C, 2 * BS])
    fs_b1s_p = P("fs_b1s", [1, 2 * BS])
    ms_w0_p = P("ms_w0", [MODD, LAT2])
    ms_b0T_p = P("ms_b0T", [128, 48])
    ms_w1s_p = P("ms_w1s", [LAT2, C], bf16)
    ms_b1s_p = P("ms_b1s", [1, C])
    fc1w_p = P("fc1w", [C, LAT], fp8)
    fc1bT_p = P("fc1bT", [128, 24])
    fc2w_p = P("fc2w", [LAT, C], fp8)
    fc2b_p = P("fc2b", [C])
    out_p = nc.declare_dram_parameter("out", [TOKP, C], f32, isOutput=True)
    DBG = False

    # internal DRAM
    a1i = nc.dram_tensor("a1i", [N, TOKR * BS], fp16)
    a1o = nc.dram_tensor("a1o", [N, TOKR * BS], fp16)
    MA = 12 * W * BS        # chunk A: local rows 0-11 per dest
    MB = 11 * W * BS        # chunk B: local rows 12-22
    TA = 12 * W             # tokens per dest covered by chunk A
    a2iA = nc.dram_tensor("a2iA", [N, MA], fp16)
    a2oA = nc.dram_tensor("a2oA", [N, MA], fp16)
    a2iB = nc.dram_tensor("a2iB", [N, MB], fp16)
    a2oB = nc.dram_tensor("a2oB", [N, MB], fp16)
    t1d = nc.dram_tensor("t1d", [KW, 2, H, BS], fp16)   # [kw][ri][h][c]
    ud = nc.dram_tensor("ud", [KW, BS, 2 * H], fp16)
    sfd = nc.dram_tensor("sfd", [1, 2 * BS], f32)
    agi = nc.dram_tensor("agi", [1, C], f32)
    ago = nc.dram_tensor("ago", [N, C], f32)

    RG = [list(range(N))]

    with tile.TileContext(nc) as tc:
        with (
            tc.tile_pool(name="const", bufs=1) as cpool,
            tc.tile_pool(name="ssb", bufs=1) as ssb,
        ):
            # ---- broadcast constants ----
            def bcast(p, n, name):
                t = cpool.tile([128, n], f32, tag=name)
                nc.sync.dma_start(out=t[:], in_=rap(p, 0, [[0, 128], [1, n]]))
                return t

            n1w_b = bcast(n1w, C, "n1w"); n1b_b = bcast(n1b, C, "n1b")
            n2w_b = bcast(n2w, C, "n2w"); n2b_b = bcast(n2b, C, "n2b")
            fc2b_b = bcast(fc2b_p, C, "fc2b")
            b2c_b = cpool.tile([128, 2, 2 * BS], f32, tag="b2c")
            for bh in range(2):
                nc.sync.dma_start(out=b2c_b[:, bh, :],
                                  in_=rap(b2c_p, 0, [[0, 128], [1, 2 * BS]]))
            eps_sb = cpool.tile([128, 1], f32, tag="eps")
            nc.vector.memset(eps_sb[:], EPS)
            nlam_sb = cpool.tile([128, 1], f32, tag="nlam")
            nc.vector.memset(nlam_sb[:], -LAM)
            ident = cpool.tile([128, 128], f32, tag="ident")
            make_identity(nc, ident[:])
            identb = cpool.tile([128, 128], bf16, tag="identb")
            nc.vector.tensor_copy(out=identb[:], in_=ident[:])


            # ---- phase 0: LN1 + scatter into A2A-1 send buffer ----
            with (
                tc.tile_pool(name="p0", bufs=6) as p0,
                tc.tile_pool(name="p0s", bufs=8) as p0s,
            ):
                for it in range(65):
                    t0 = it * 128
                    nrow = min(128, TOKR - t0)
                    xt = p0.tile([128, C], f32, tag="xt")
                    nc.sync.dma_start(out=xt[:], in_=xs[t0 : t0 + 128, :])
                    st = p0s.tile([128, 2, 6], f32, tag="st")
                    for g in range(2):
                        nc.vector.bn_stats(out=st[:, g, :], in_=xt[:, 384 * g : 384 * (g + 1)])
                    mv = p0s.tile([128, 2], f32, tag="mv")
                    nc.vector.bn_aggr(out=mv[:], in_=st[:])
                    rstd = p0s.tile([128, 1], f32, tag="rstd")
                    nc.scalar.activation(out=rstd[:], in_=mv[:, 1:2], func=FT.Sqrt,
                                         bias=eps_sb[:], scale=1.0)
                    nc.vector.reciprocal(out=rstd[:], in_=rstd[:])
                    ln = p0.tile([128, C], f32, tag="ln")
                    nc.vector.tensor_scalar(out=ln[:], in0=xt[:], scalar1=mv[:, 0:1],
                                            scalar2=rstd[:], op0=OP.subtract, op1=OP.mult)
                    aeng = nc.gpsimd if it % 3 == 2 else nc.vector
                    aeng.tensor_mul(out=ln[:], in0=ln[:], in1=n1w_b[:])
                    lnh = p0.tile([128, C], fp16, tag="lnh")
                    aeng.tensor_add(out=lnh[:], in0=ln[:], in1=n1b_b[:])
                    nc.scalar.dma_start(
                        out=rap(a1i, t0 * BS, [[BS, nrow], [TOKR * BS, N], [1, BS]]),
                        in_=lnh[:nrow].rearrange("p (j c) -> p j c", j=N),
                    )

            nc.gpsimd.collective_compute(
                "AllToAll", OP.bypass, replica_groups=RG, ins=[a1i[:]], outs=[a1o[:]])

            # ---- scale-shift MLPs (overlap with A2A1 window) ----
            ss_ctx = tc.tile_pool(name="ssw", bufs=1)
            ssw = ss_ctx.__enter__()
            ssp_ctx = tc.tile_pool(name="ssp", bufs=1, space="PSUM")
            ssp = ssp_ctx.__enter__()
            modT_sb = ssw.tile([MODD, 2], f32r)
            nc.sync.dma_start(out=modT_sb[:], in_=modT[:].bitcast(f32r))
            fs_w0_sb = ssw.tile([MODD, 2 * C], f32r)
            nc.sync.dma_start(out=fs_w0_sb[:], in_=fs_w0_p[:].bitcast(f32r))
            fs_b0T_sb = ssw.tile([128, 12], f32)
            nc.sync.dma_start(out=fs_b0T_sb[:], in_=fs_b0T_p[:])
            e0T = ssw.tile([128, 12], f32r)
            for j in range(12):
                pt = ssp.tile([128, 2], f32, tag="ss1")
                nc.tensor.matmul(pt[:], fs_w0_sb[:, 128 * j : 128 * (j + 1)],
                                 modT_sb[:], start=True, stop=True)
                nc.scalar.activation(out=e0T[:, j : j + 1], in_=pt[:, 0:1], func=FT.Gelu,
                                     bias=fs_b0T_sb[:, j : j + 1], scale=1.0)
            fs_w1s_sb = ssw.tile([128, 12, 2 * BS], f32r)
            nc.sync.dma_start(
                out=fs_w1s_sb[:],
                in_=rap(fs_w1s_p, 0, [[2 * BS, 128], [128 * 2 * BS, 12], [1, 2 * BS]]).bitcast(f32r),
            )
            fs_b1s_sb = ssw.tile([1, 2 * BS], f32)
            nc.sync.dma_start(out=fs_b1s_sb[:], in_=fs_b1s_p[:])
            p2 = ssp.tile([1, 2 * BS], f32, tag="ss2")
            for j in range(12):
                nc.tensor.matmul(p2[:], e0T[:, j : j + 1], fs_w1s_sb[:, j, :],
                                 start=(j == 0), stop=(j == 11))
            sfo = ssw.tile([1, 2 * BS], f32)
            nc.vector.tensor_add(out=sfo[:], in0=p2[:], in1=fs_b1s_sb[:])
            nc.sync.dma_start(out=sfd[:], in_=sfo[:])
            sfT = ssw.tile([BS, 2], f32)
            nc.sync.dma_start(out=sfT[:], in_=rap(sfd, 0, [[1, BS], [BS, 2]]))
            sfv = ssb.tile([BS, 1], f32)
            nc.vector.tensor_scalar_add(out=sfv[:], in0=sfT[:, 0:1], scalar1=1.0)
            b1r_sb = ssw.tile([BS, 1], f32)
            nc.sync.dma_start(out=b1r_sb[:], in_=b1r_p[:])
            b1i_sb = ssw.tile([BS, 1], f32)
            nc.sync.dma_start(out=b1i_sb[:], in_=b1i_p[:])
            Br = ssb.tile([BS, 1], f32)
            nc.vector.tensor_mul(out=Br[:], in0=b1r_sb[:], in1=sfv[:])
            nc.vector.tensor_add(out=Br[:], in0=Br[:], in1=sfT[:, 1:2])
            Bi = ssb.tile([BS, 1], f32)
            nc.vector.tensor_mul(out=Bi[:], in0=b1i_sb[:], in1=sfv[:])
            nc.vector.tensor_add(out=Bi[:], in0=Bi[:], in1=sfT[:, 1:2])

            # ms MLP: e1T then column-sharded 6144->768, AllGather
            ms_w0_sb = ssw.tile([MODD, LAT2], f32r)
            nc.gpsimd.dma_start(out=ms_w0_sb[:], in_=ms_w0_p[:].bitcast(f32r))
            ms_b0T_sb = ssw.tile([128, 48], f32)
            nc.sync.dma_start(out=ms_b0T_sb[:], in_=ms_b0T_p[:])
            e1T = ssw.tile([128, 48], bf16)
            for j in range(48):
                pt = ssp.tile([128, 2], f32, tag="ss1")
                nc.tensor.matmul(pt[:], ms_w0_sb[:, 128 * j : 128 * (j + 1)],
                                 modT_sb[:], start=True, stop=True)
                nc.scalar.activation(out=e1T[:, j : j + 1], in_=pt[:, 0:1], func=FT.Gelu,
                                     bias=ms_b0T_sb[:, j : j + 1], scale=1.0)
            p3 = ssp.tile([1, 2, 512], f32, tag="ss3")
            with tc.tile_pool(name="msw", bufs=3) as mswp:
                for j in range(48):
                    wt = mswp.tile([128, C], bf16)
                    nc.gpsimd.dma_start(
                        out=wt[:], in_=ms_w1s_p[128 * j : 128 * (j + 1), :])
                    for h2 in range(2):
                        nc.tensor.matmul(
                            p3[:, h2, 0:384], e1T[:, j : j + 1],
                            wt[:, 384 * h2 : 384 * (h2 + 1)],
                            start=(j == 0), stop=(j == 47))
            ms_b1s_sb = ssw.tile([1, C], f32)
            nc.sync.dma_start(out=ms_b1s_sb[:], in_=ms_b1s_p[:])
            mso = ssw.tile([1, C], f32)
            nc.vector.tensor_add(out=mso[:].rearrange("p (a b) -> p a b", a=2),
                                 in0=p3[:, :, 0:384],
                                 in1=ms_b1s_sb[:].rearrange("p (a b) -> p a b", a=2))
            nc.sync.dma_start(out=agi[:], in_=mso[:])
            nc.gpsimd.collective_compute(
                "AllGather", OP.bypass, replica_groups=RG, ins=[agi[:]], outs=[ago[:]])
            sM = ssb.tile([128, 24], f32)
            nc.sync.dma_start(out=sM[:], in_=rap(ago, 0, [[1, 128], [128, 24]]))
            nc.vector.tensor_scalar_add(out=sM[:], in0=sM[:], scalar1=1.0)
            tM = ssb.tile([128, 24], f32)
            nc.sync.dma_start(out=tM[:], in_=rap(ago, LAT, [[1, 128], [128, 24]]))
            fc1bT_sb = ssw.tile([128, 24], f32)
            nc.sync.dma_start(out=fc1bT_sb[:], in_=fc1bT_p[:])
            B1 = ssb.tile([128, 24], f32)
            nc.vector.tensor_mul(out=B1[:], in0=fc1bT_sb[:], in1=sM[:])
            nc.vector.tensor_add(out=B1[:], in0=B1[:], in1=tM[:])
            sM16 = ssb.tile([128, 24], f32)
            nc.vector.tensor_scalar_mul(out=sM16[:], in0=sM[:], scalar1=1.0 / 16.0)

            ssp_ctx.__exit__(None, None, None)
            ss_ctx.__exit__(None, None, None)

            # ---- phase 1 stage A: W-DFT  (X[h,w,c] -> t1d[kw,ri,h,c]) ----
            with (
                tc.tile_pool(name="sa", bufs=1) as sa,
                tc.tile_pool(name="sac", bufs=3) as sac,
                tc.tile_pool(name="sap", bufs=2, space="PSUM") as sap,
            ):
                fw_sb = []
                for ri, p in enumerate([fwr_p, fwi_p]):
                    t = sa.tile([120, 3, KW], fp16, tag=f"fw{ri}")
                    nc.sync.dma_start(
                        out=t[:], in_=rap(p, 0, [[KW, 120], [120 * KW, 3], [1, KW]]))
                    fw_sb.append(t)
                X_sb = sa.tile([120, 3, H, BS], fp16, tag="xsb")
                for ch in range(4):
                    for k in range(3):
                        eng = nc.scalar if (ch + k) % 2 else nc.sync
                        eng.dma_start(
                            out=X_sb[:, k, 45 * ch : 45 * (ch + 1), :],
                            in_=rap(a1o, (45 * ch * W + 120 * k) * BS,
                                    [[BS, 120], [W * BS, 45], [1, BS]]))
                for hs in range(36):
                    hh0 = 5 * hs
                    cp = sac.tile([KW, 2, 5, BS], fp16, tag="cpa")
                    for ri in range(2):
                        ps = sap.tile([KW, 5, BS], f32, tag="pa")
                        for k in range(3):
                            nc.tensor.matmul(ps[:], fw_sb[ri][:, k, :],
                                             X_sb[:, k, hh0 : hh0 + 5, :],
                                             start=(k == 0), stop=(k == 2))
                        nc.vector.tensor_copy(out=cp[:, ri, :, :], in_=ps[:])
                    nc.scalar.dma_start(
                        out=rap(t1d, hh0 * BS,
                                [[2 * H * BS, KW], [H * BS, 2], [BS, 5], [1, BS]]),
                        in_=cp[:])

            # ---- stages B+C+D fused in SBUF, then E ----
            with tc.tile_pool(name="fb", bufs=1) as fb:
                fhs_sb = fb.tile([90, 4, 2 * H], fp16)
                nc.sync.dma_start(
                    out=fhs_sb[:],
                    in_=rap(fhs_p, 0, [[2 * H, 90], [90 * 2 * H, 4], [1, 2 * H]]))
                ifhs_sb = fb.tile([90, 4, 2 * H], fp16)
                nc.sync.dma_start(
                    out=ifhs_sb[:],
                    in_=rap(ifhs_p, 0, [[2 * H, 90], [90 * 2 * H, 4], [1, 2 * H]]))
                w1r_sb = fb.tile([BS, BS], fp16)
                nc.sync.dma_start(out=w1r_sb[:], in_=w1r_p[:])
                w1i_sb = fb.tile([BS, BS], fp16)
                nc.sync.dma_start(out=w1i_sb[:], in_=w1i_p[:])
                w1in_sb = fb.tile([BS, BS], fp16)
                nc.sync.dma_start(out=w1in_sb[:], in_=w1in_p[:])
                w2cr_sb = fb.tile([BS, 2 * BS], fp16)
                nc.sync.dma_start(out=w2cr_sb[:], in_=w2cr_p[:])
                w2ci_sb = fb.tile([BS, 2 * BS], fp16)
                nc.sync.dma_start(out=w2ci_sb[:], in_=w2ci_p[:])
                ifw_sb = []
                for ri, p in enumerate([ifwr_p, ifwi_p]):
                    t = fb.tile([KW, 3, 120], fp16, tag=f"ifw{ri}")
                    nc.sync.dma_start(
                        out=t[:], in_=rap(p, 0, [[W, KW], [120, 3], [1, 120]]))
                    ifw_sb.append(t)
                # T1 resident: [h(90), half, ri, kw, c]
                T1_sb = fb.tile([90, 2, 2, KW, BS], fp16, tag="t1sb")
                for half in range(2):
                    for ri in range(2):
                        eng = nc.scalar if ri else nc.sync
                        eng.dma_start(
                            out=T1_sb[:, half, ri, :, :],
                            in_=rap(t1d, (ri * H + half * 90) * BS,
                                    [[BS, 90], [2 * H * BS, KW], [1, BS]]))
                U_sb = fb.tile([KW, BS, 2 * H], fp16, tag="usb")

                with (
                    tc.tile_pool(name="bcw", bufs=4) as bcw,
                    tc.tile_pool(name="bct", bufs=4) as bct,
                    tc.tile_pool(name="bco", bufs=3) as bco,
                    tc.tile_pool(name="bcp1", bufs=1, space="PSUM") as bcps,
                    tc.tile_pool(name="bcp2", bufs=1, space="PSUM") as bcps2,
                    tc.tile_pool(name="bcp3", bufs=2, space="PSUM") as bcps3,
                    tc.tile_pool(name="bcp4", bufs=1, space="PSUM") as bcps4,
                ):
                    def emit_D(O2t, kw0, G):
                        psU = bcps4.tile([BS, 2, 512], f32, tag="psU")
                        for g in range(G):
                            for q in range(4):
                                ri, half = q // 2, q % 2
                                nc.tensor.matmul(
                                    psU[:, g, 0 : 2 * H],
                                    O2t[:, g, half, ri * BS : (ri + 1) * BS],
                                    ifhs_sb[:, q, :], start=(q == 0), stop=(q == 3))
                        ucp = bcw.tile([BS, 2, 2 * H], fp16, tag="ucp")
                        nc.vector.tensor_copy(out=ucp[:, :G, :], in_=psU[:, :G, 0 : 2 * H])
                        nc.sync.dma_start(
                            out=rap(ud, kw0 * BS * 2 * H,
                                    [[2 * H, BS], [BS * 2 * H, G], [1, 2 * H]]),
                            in_=ucp[:, :G, :])

                    pend = None
                    for pr in range(46):
                        kw0 = 2 * pr
                        G = 2 if kw0 + 1 < KW else 1
                        psF = bcps.tile([BS, 2, 512], f32, tag="psF")
                        for g in range(G):
                            kw = kw0 + g
                            for q in range(4):
                                ri, half = q // 2, q % 2
                                nc.tensor.matmul(
                                    psF[:, g, 0 : 2 * H], T1_sb[:, half, ri, kw, :],
                                    fhs_sb[:, q, :], start=(q == 0), stop=(q == 3))
                        fsb = bcw.tile([BS, 2, 2 * H], fp16, tag="fsb")
                        nc.vector.tensor_copy(out=fsb[:, :G, :], in_=psF[:, :G, 0 : 2 * H])
                        ps1r = bcps2.tile([BS, 2, H], f32, tag="ps1r")
                        ps1i = bcps2.tile([BS, 2, H], f32, tag="ps1i")
                        nc.tensor.matmul(ps1r[:, :G, :], w1r_sb[:], fsb[:, :G, 0:H],
                                         start=True, stop=False)
                        nc.tensor.matmul(ps1r[:, :G, :], w1in_sb[:], fsb[:, :G, H : 2 * H],
                                         start=False, stop=True)
                        nc.tensor.matmul(ps1i[:, :G, :], w1i_sb[:], fsb[:, :G, 0:H],
                                         start=True, stop=False)
                        nc.tensor.matmul(ps1i[:, :G, :], w1r_sb[:], fsb[:, :G, H : 2 * H],
                                         start=False, stop=True)
                        o1r = bcw.tile([BS, 2, H], fp16, tag="o1r")
                        o1i = bcw.tile([BS, 2, H], fp16, tag="o1i")
                        nc.scalar.activation(out=o1r[:, :G, :], in_=ps1r[:, :G, :],
                                             func=FT.Relu, bias=Br[:], scale=sfv[:])
                        nc.scalar.activation(out=o1i[:, :G, :], in_=ps1i[:, :G, :],
                                             func=FT.Relu, bias=Bi[:], scale=sfv[:])
                        o1rf = o1r[:].rearrange("p g k -> p (g k)")
                        o1if = o1i[:].rearrange("p g k -> p (g k)")
                        O2t = bco.tile([90, 2, 2, 2 * BS], fp16, tag="o2sb")
                        for g in range(G):
                            ps2 = bcps3.tile([90, 2, 2 * BS], f32, tag="ps2")
                            for half in range(2):
                                sl = slice(90 * (2 * g + half), 90 * (2 * g + half + 1))
                                nc.tensor.matmul(ps2[:, half, :], o1rf[:, sl], w2cr_sb[:],
                                                 start=True, stop=False)
                                nc.tensor.matmul(ps2[:, half, :], o1if[:, sl], w2ci_sb[:],
                                                 start=False, stop=True)
                            tmp = bct.tile([90, 2, 2 * BS], f32, tag="tmp")
                            nc.vector.tensor_add(out=tmp[:], in0=ps2[:], in1=b2c_b[:90])
                            r1 = bct.tile([90, 2, 2 * BS], f32, tag="r1")
                            nc.vector.tensor_scalar(out=r1[:], in0=tmp[:], scalar1=-LAM,
                                                    scalar2=LAM, op0=OP.max, op1=OP.min)
                            nc.vector.tensor_sub(out=O2t[:, g, :, :],
                                                 in0=tmp[:], in1=r1[:])
                        # stage D (inverse H-DFT) pipelined one iteration
                        # behind B/C so the softshrink chain never stalls
                        # the in-order tensor queue
                        if pend is not None:
                            emit_D(*pend)
                        pend = (O2t, kw0, G)
                    emit_D(*pend)

                # U back to kw-partitioned SBUF, then stage E (inverse W-DFT)
                for chv in range(2):
                    k0 = 46 * chv
                    kn = min(46, KW - k0)
                    eng = nc.scalar if chv else nc.sync
                    eng.dma_start(
                        out=U_sb[k0 : k0 + kn, :, :],
                        in_=rap(ud, k0 * BS * 2 * H,
                                [[BS * 2 * H, kn], [2 * H, BS], [1, 2 * H]]))
                with (
                    tc.tile_pool(name="sec", bufs=4) as sec,
                    tc.tile_pool(name="sep", bufs=2, space="PSUM") as sep,
                ):
                    # chunk A = local rows 0-11 per dest, chunk B = rows 12-22;
                    # A2A for chunk A overlaps stage-E compute of chunk B.
                    for part in range(2):
                        rts = [(0, 4), (4, 4), (8, 4)] if part == 0 else \
                              [(12, 4), (16, 4), (20, 3)]
                        for d in range(N):
                            for roff, nr in rts:
                                h0 = HP * d + roff
                                nr = min(nr, H - h0)
                                if nr <= 0:
                                    continue
                                for wk in range(3):
                                    psE = sep.tile([120, 4, BS], f32, tag="psE")
                                    for ri in range(2):
                                        nc.tensor.matmul(
                                            psE[:, :nr, :], ifw_sb[ri][:, wk, :],
                                            U_sb[:, :, ri * H + h0 : ri * H + h0 + nr]
                                                .rearrange("p c h -> p h c"),
                                            start=(ri == 0), stop=(ri == 1))
                                    ecp = sec.tile([120, 4, BS], fp16, tag="ecp")
                                    nc.vector.tensor_copy(out=ecp[:, :nr, :],
                                                          in_=psE[:, :nr, :])
                                    if part == 0:
                                        dst, off = a2iA, d * MA + (roff * W + wk * 120) * BS
                                    else:
                                        dst, off = a2iB, d * MB + ((roff - 12) * W + wk * 120) * BS
                                    nc.scalar.dma_start(
                                        out=rap(dst, off, [[BS, 120], [W * BS, nr], [1, BS]]),
                                        in_=ecp[:, :nr, :])
                        if part == 0:
                            nc.gpsimd.collective_compute(
                                "AllToAll", OP.bypass, replica_groups=RG,
                                ins=[a2iA[:]], outs=[a2oA[:]])

            nc.gpsimd.collective_compute(
                "AllToAll", OP.bypass, replica_groups=RG, ins=[a2iB[:]], outs=[a2oB[:]])

            # ---- phase 2: h1 = F2 + ln1x + x; LN2; modulated MLP; + h1 ----
            with (
                tc.tile_pool(name="p2w", bufs=1) as p2w,
                tc.tile_pool(name="p2", bufs=4) as p2,
                tc.tile_pool(name="p2h", bufs=16) as p2h,
                tc.tile_pool(name="p2hm", bufs=2) as p2hm,
                tc.tile_pool(name="p2s", bufs=4) as p2s,
                tc.tile_pool(name="p2m", bufs=4) as p2m,
                tc.tile_pool(name="ptp", bufs=2, space="PSUM") as ptp,
                tc.tile_pool(name="php", bufs=2, space="PSUM") as php,
                tc.tile_pool(name="pop", bufs=2, space="PSUM") as pop,
            ):
                PM = mybir.MatmulPerfMode.DoubleRow
                fc1w_sb = p2w.tile([128, 6, LAT], fp8)
                nc.sync.dma_start(
                    out=fc1w_sb[:], in_=rap(fc1w_p, 0, [[LAT, 128], [128 * LAT, 6], [1, LAT]]))
                fc2w_sb = p2w.tile([128, 24, C], fp8)
                nc.sync.dma_start(
                    out=fc2w_sb[:], in_=rap(fc2w_p, 0, [[C, 128], [128 * C, 24], [1, C]]))

                def p2_prep(it):
                    T0 = it * 512
                    ln2T = p2m.tile([128, 6, 4, 128], fp8, tag="ln2T")
                    h1s = []
                    for hf in range(4):
                        t0 = T0 + 128 * hf
                        nload = max(0, min(128, TOKR - t0))
                        xt = p2.tile([128, C], f32, tag="xt2")
                        nc.sync.dma_start(out=xt[:], in_=xs[t0 : t0 + 128, :])
                        f2t = p2.tile([128, N, BS], fp16, tag="f2t")
                        l1t = p2.tile([128, N, BS], fp16, tag="l1t")
                        if nload < 128:
                            nc.vector.memset(f2t[:], 0.0)
                            nc.vector.memset(l1t[:], 0.0)
                        if nload > 0:
                            ta = min(nload, max(0, TA - t0))
                            if ta > 0:
                                nc.sync.dma_start(
                                    out=f2t[:ta],
                                    in_=rap(a2oA, t0 * BS, [[BS, ta], [MA, N], [1, BS]]))
                            if ta < nload:
                                t0b = t0 + ta - TA
                                nc.sync.dma_start(
                                    out=f2t[ta:nload],
                                    in_=rap(a2oB, t0b * BS, [[BS, nload - ta], [MB, N], [1, BS]]))
                            nc.sync.dma_start(
                                out=l1t[:nload],
                                in_=rap(a1i, t0 * BS, [[BS, nload], [TOKR * BS, N], [1, BS]]))
                        h1 = p2h.tile([128, C], f32, tag="h1")
                        nc.vector.tensor_add(out=h1[:], in0=xt[:],
                                             in1=f2t[:].rearrange("p j c -> p (j c)"))
                        nc.vector.tensor_add(out=h1[:], in0=h1[:],
                                             in1=l1t[:].rearrange("p j c -> p (j c)"))
                        h1s.append(h1)
                        st = p2s.tile([128, 2, 6], f32, tag="st2")
                        for g in range(2):
                            nc.vector.bn_stats(out=st[:, g, :], in_=h1[:, 384 * g : 384 * (g + 1)])
                        mv = p2s.tile([128, 2], f32, tag="mv2")
                        nc.vector.bn_aggr(out=mv[:], in_=st[:])
                        rstd = p2s.tile([128, 1], f32, tag="rstd2")
                        nc.scalar.activation(out=rstd[:], in_=mv[:, 1:2], func=FT.Sqrt,
                                             bias=eps_sb[:], scale=1.0)
                        nc.vector.reciprocal(out=rstd[:], in_=rstd[:])
                        ln2 = p2.tile([128, C], bf16, tag="ln2")
                        nc.vector.tensor_scalar(out=ln2[:], in0=h1[:], scalar1=mv[:, 0:1],
                                                scalar2=rstd[:], op0=OP.subtract, op1=OP.mult)
                        nc.vector.tensor_mul(out=ln2[:], in0=ln2[:], in1=n2w_b[:])
                        nc.vector.tensor_add(out=ln2[:], in0=ln2[:], in1=n2b_b[:])
                        for jb in range(2):
                            pst = ptp.tile([128, 3, 128], bf16, tag="pst")
                            for jj in range(3):
                                j = 3 * jb + jj
                                nc.tensor.transpose(pst[:, jj, :],
                                                    ln2[:, 128 * j : 128 * (j + 1)],
                                                    identb[:])
                            nc.vector.tensor_copy(out=ln2T[:, 3 * jb : 3 * jb + 3, hf, :],
                                                  in_=pst[:])
                    return T0, ln2T, h1s

                def p2_mm(T0, ln2T, h1s):
                    hmidT = p2hm.tile([128, 24, 512], fp8, tag="hmidT")
                    for l in range(24):
                        psH = php.tile([128, 512], f32, tag="psH")
                        for jp in range(3):
                            nc.tensor.matmul(
                                psH[:],
                                fc1w_sb[:, 2 * jp : 2 * jp + 2, 128 * l : 128 * (l + 1)],
                                ln2T[:, 2 * jp : 2 * jp + 2, :, :]
                                    .rearrange("p j h t -> p j (h t)"),
                                start=(jp == 0), stop=(jp == 2), perf_mode=PM)
                        nc.scalar.activation(out=hmidT[:, l, :], in_=psH[:], func=FT.Gelu,
                                             bias=B1[:, l : l + 1], scale=sM16[:, l : l + 1])
                    for hf in range(4):
                        t0 = T0 + 128 * hf
                        psO = pop.tile([128, 2, 512], f32, tag="psO")
                        for lp in range(12):
                            for h2 in range(2):
                                nc.tensor.matmul(
                                    psO[:, h2, 0:384],
                                    hmidT[:, 2 * lp : 2 * lp + 2, 128 * hf : 128 * (hf + 1)],
                                    fc2w_sb[:, 2 * lp : 2 * lp + 2, 384 * h2 : 384 * (h2 + 1)],
                                    start=(lp == 0), stop=(lp == 11), perf_mode=PM)
                        ot = p2.tile([128, C], f32, tag="ot")
                        nc.vector.scalar_tensor_tensor(
                            out=ot[:].rearrange("p (a b) -> p a b", a=2),
                            in0=psO[:, :, 0:384], scalar=1.0 / 16.0,
                            in1=h1s[hf][:].rearrange("p (a b) -> p a b", a=2),
                            op0=OP.mult, op1=OP.add)
                        nc.gpsimd.tensor_add(out=ot[:], in0=ot[:], in1=fc2b_b[:])
                        nc.sync.dma_start(out=out_p[t0 : t0 + 128, :], in_=ot[:])

                # depth-4 software pipeline, preps paired so scalar runs
                # [Sqrt x8, Gelu x48] per pair (one act-table swap per tile)
                sq = [p2_prep(i) for i in range(min(4, NT2))]
                for itp in range(0, NT2, 2):
                    p2_mm(*sq[itp])
                    if itp + 1 < NT2:
                        p2_mm(*sq[itp + 1])
                    for j in (itp + 4, itp + 5):
                        if j < NT2:
                            sq.append(p2_prep(j))

    nc.compile()
    return nc


_NC = None


def _get_nc():
    global _NC
    if _NC is None:
        _NC = _build()
    return _NC


def _dft_mats():
    w = np.arange(W); kw = np.arange(KW)
    ang = 2 * np.pi * np.outer(w, kw) / W
    fwr = (np.cos(ang) / np.sqrt(W)).astype(np.float32)
    fwi = (-np.sin(ang) / np.sqrt(W)).astype(np.float32)
    kh = np.arange(H); h = np.arange(H)
    angh = 2 * np.pi * np.outer(kh, h) / H        # [kh, h]
    fhr = np.cos(angh) / np.sqrt(H)
    fhi = -np.sin(angh) / np.sqrt(H)
    fhs = np.zeros((2 * H, 2 * H))
    fhs[:H, :H] = fhr.T; fhs[:H, H:] = fhi.T
    fhs[H:, :H] = -fhi.T; fhs[H:, H:] = fhr.T
    ci = np.cos(angh) / np.sqrt(H)                # [kh, h] for inverse
    si = np.sin(angh) / np.sqrt(H)
    ifhs = np.zeros((2 * H, 2 * H))
    ifhs[:H, :H] = ci; ifhs[:H, H:] = si
    ifhs[H:, :H] = -si; ifhs[H:, H:] = ci
    ckw = np.where(kw == 0, 1.0, 2.0)
    angi = 2 * np.pi * np.outer(kw, np.arange(W)) / W    # [kw, w]
    ifwr = (ckw[:, None] * np.cos(angi) / np.sqrt(W)).astype(np.float32)
    ifwi = (-ckw[:, None] * np.sin(angi) / np.sqrt(W)).astype(np.float32)
    return fwr, fwi, fhs.astype(np.float32), ifhs.astype(np.float32), ifwr, ifwi


def _prepare_in_maps(x, mod_embed, n1w, n1b, n2w, n2b, w1, b1, w2, b2,
                     fs_w0, fs_b0, fs_w1, fs_b1, fc1w, fc1b, fc2w, fc2b,
                     ms_w0, ms_b0, ms_w1, ms_b1):
    f = np.asarray
    x = f(x, dtype=np.float32)
    grid = x.reshape(H, W, C)
    fwr, fwi, fhs, ifhs, ifwr, ifwi = _dft_mats()
    bf = ml_dtypes.bfloat16

    in_maps = []
    for b in range(N):
        r0, r1 = HP * b, min(HP * (b + 1), H)
        xsb = np.zeros((TOKP, C), np.float32)
        xsb[: (r1 - r0) * W] = grid[r0:r1].reshape(-1, C)
        sl = slice(BS * b, BS * (b + 1))
        w2r = f(w2[0, b], np.float32); w2i = f(w2[1, b], np.float32)
        im = {
            "xs": xsb,
            "modT": np.repeat(f(mod_embed, np.float32).reshape(MODD, 1), 2, axis=1).copy(),
            "n1w": f(n1w, np.float32), "n1b": f(n1b, np.float32),
            "n2w": f(n2w, np.float32), "n2b": f(n2b, np.float32),
            "fwr": fwr.astype(np.float16), "fwi": fwi.astype(np.float16),
            "fhs": fhs.astype(np.float16), "ifhs": ifhs.astype(np.float16),
            "ifwr": ifwr.astype(np.float16), "ifwi": ifwi.astype(np.float16),
            "w1r": f(w1[0, b], np.float16).copy(),
            "w1i": f(w1[1, b], np.float16).copy(),
            "w1in": (-f(w1[1, b], np.float16)).copy(),
            "w2cr": np.concatenate([w2r, w2i], axis=1).astype(np.float16),
            "w2ci": np.concatenate([-w2i, w2r], axis=1).astype(np.float16),
            "b1r": f(b1[0, b], np.float32).reshape(BS, 1).copy(),
            "b1i": f(b1[1, b], np.float32).reshape(BS, 1).copy(),
            "b2c": np.concatenate([f(b2[0, b], np.float32), f(b2[1, b], np.float32)]),
            "fs_w0": f(fs_w0, np.float32),
            "fs_b0T": f(fs_b0, np.float32).reshape(12, 128).T.copy(),
            "fs_w1s": np.concatenate(
                [f(fs_w1, np.float32)[:, sl], f(fs_w1, np.float32)[:, C + BS * b : C + BS * (b + 1)]],
                axis=1),
            "fs_b1s": np.concatenate(
                [f(fs_b1, np.float32)[sl], f(fs_b1, np.float32)[C + BS * b : C + BS * (b + 1)]]
            ).reshape(1, -1),
            "ms_w0": f(ms_w0, np.float32),
            "ms_b0T": f(ms_b0, np.float32).reshape(48, 128).T.copy(),
            "ms_w1s": f(ms_w1, np.float32)[:, C * b : C * (b + 1)].astype(bf),
            "ms_b1s": f(ms_b1, np.float32)[C * b : C * (b + 1)].reshape(1, -1),
            "fc1w": (16.0 * f(fc1w, np.float32)).astype(ml_dtypes.float8_e4m3),
            "fc1bT": f(fc1b, np.float32).reshape(24, 128).T.copy(),
            "fc2w": (16.0 * f(fc2w, np.float32)).astype(ml_dtypes.float8_e4m3),
            "fc2b": f(fc2b, np.float32),
        }
        in_maps.append(im)
    return in_maps


def kernel(x, mod_embed, n1w, n1b, n2w, n2b, w1, b1, w2, b2,
           fs_w0, fs_b0, fs_w1, fs_b1, fc1w, fc1b, fc2w, fc2b,
           ms_w0, ms_b0, ms_w1, ms_b1):
    nc = _get_nc()
    in_maps = _prepare_in_maps(
        x, mod_embed, n1w, n1b, n2w, n2b, w1, b1, w2, b2,
        fs_w0, fs_b0, fs_w1, fs_b1, fc1w, fc1b, fc2w, fc2b,
        ms_w0, ms_b0, ms_w1, ms_b1)

    res = run_bass_kernel_spmd(nc, in_maps, core_ids=list(range(N)))
    globals()["last_results"] = res
    out = np.zeros((H, W, C), np.float32)
    for b in range(N):
        r0, r1 = HP * b, min(HP * (b + 1), H)
        out[r0:r1] = res.results[b]["out"][: (r1 - r0) * W].reshape(r1 - r0, W, C)
    return out.reshape(1, H, W, C)



# revision 35
# speedup vs baseline: 1.1412x; 1.0063x over previous
"""AFNO block kernel for 8 Trainium2 NeuronCores.

Sharding: token-shard (H rows, 23 per core padded) for LN/MLP phases;
AllToAll to channel-shard (core i = spectral block i, 96 channels) for the
2D-FFT filter, computed as matmuls against precomputed DFT matrices;
AllToAll back; small AllGather for the column-sharded 6144x6144 scale-shift
MLP weight.
"""

import os
import numpy as np
import ml_dtypes

import concourse.bass as bass
import concourse.bacc as bacc
import concourse.mybir as mybir
import concourse.tile as tile
from concourse.bass_utils import run_bass_kernel_spmd
from concourse.masks import make_identity

f32 = mybir.dt.float32
f32r = mybir.dt.float32r
bf16 = mybir.dt.bfloat16
fp16 = mybir.dt.float16
fp8 = mybir.dt.float8e4
FT = mybir.ActivationFunctionType
OP = mybir.AluOpType

H, W, C = 180, 360, 768
NB, BS, KW = 8, 96, 91
HP = 23                 # rows per shard (8*23 = 184 >= 180)
TOKR = HP * W           # 8280 real token slots per shard
NT2 = 17                # phase-2 tiles of 512
TOKP = NT2 * 512        # 8704 padded tokens per shard
MODD, LAT, LAT2 = 64, 3072, 6144
LAM = 0.01
EPS = 1e-5
N = 8


def rap(t, offset, dims):
    a = t[:] if not isinstance(t, bass.AP) else t
    return bass.AP(tensor=a.tensor, offset=a.offset + offset, ap=[list(d) for d in dims])


def _build():
    nc = bacc.Bacc("TRN2", target_bir_lowering=False, debug=False, num_devices=N)

    def P(name, shp, dt=f32):
        return nc.declare_dram_parameter(name, list(shp), dt, isOutput=False)

    xs = P("xs", [TOKP, C])
    modT = P("modT", [MODD, 2])
    n1w = P("n1w", [C]); n1b = P("n1b", [C])
    n2w = P("n2w", [C]); n2b = P("n2b", [C])
    fwr_p = P("fwr", [W, KW], fp16); fwi_p = P("fwi", [W, KW], fp16)
    fhs_p = P("fhs", [2 * H, 2 * H], fp16)
    ifhs_p = P("ifhs", [2 * H, 2 * H], fp16)
    ifwr_p = P("ifwr", [KW, W], fp16); ifwi_p = P("ifwi", [KW, W], fp16)
    w1r_p = P("w1r", [BS, BS], fp16); w1i_p = P("w1i", [BS, BS], fp16)
    w1in_p = P("w1in", [BS, BS], fp16)
    w2cr_p = P("w2cr", [BS, 2 * BS], fp16)   # [W2r | W2i]
    w2ci_p = P("w2ci", [BS, 2 * BS], fp16)   # [-W2i | W2r]
    b1r_p = P("b1r", [BS, 1]); b1i_p = P("b1i", [BS, 1])
    b2c_p = P("b2c", [2 * BS])               # concat(b2r, b2i)
    fs_w0_p = P("fs_w0", [MODD, 2 * C])
    fs_b0T_p = P("fs_b0T", [128, 12])
    fs_w1s_p = P("fs_w1s", [2 * C, 2 * BS])
    fs_b1s_p = P("fs_b1s", [1, 2 * BS])
    ms_w0_p = P("ms_w0", [MODD, LAT2])
    ms_b0T_p = P("ms_b0T", [128, 48])
    ms_w1s_p = P("ms_w1s", [LAT2, C], bf16)
    ms_b1s_p = P("ms_b1s", [1, C])
    fc1w_p = P("fc1w", [C, LAT], fp8)
    fc1bT_p = P("fc1bT", [128, 24])
    fc2w_p = P("fc2w", [LAT, C], fp8)
    fc2b_p = P("fc2b", [C])
    out_p = nc.declare_dram_parameter("out", [TOKP, C], f32, isOutput=True)
    DBG = False

    # internal DRAM
    a1i = nc.dram_tensor("a1i", [N, TOKR * BS], fp16)
    a1o = nc.dram_tensor("a1o", [N, TOKR * BS], fp16)
    MA = 12 * W * BS        # chunk A: local rows 0-11 per dest
    MB = 11 * W * BS        # chunk B: local rows 12-22
    TA = 12 * W             # tokens per dest covered by chunk A
    a2iA = nc.dram_tensor("a2iA", [N, MA], fp16)
    a2oA = nc.dram_tensor("a2oA", [N, MA], fp16)
    a2iB = nc.dram_tensor("a2iB", [N, MB], fp16)
    a2oB = nc.dram_tensor("a2oB", [N, MB], fp16)
    t1d = nc.dram_tensor("t1d", [KW, 2, H, BS], fp16)   # [kw][ri][h][c]
    ud = nc.dram_tensor("ud", [KW, BS, 2 * H], fp16)
    sfd = nc.dram_tensor("sfd", [1, 2 * BS], f32)
    agi = nc.dram_tensor("agi", [1, C], f32)
    ago = nc.dram_tensor("ago", [N, C], f32)

    RG = [list(range(N))]

    with tile.TileContext(nc) as tc:
        with (
            tc.tile_pool(name="const", bufs=1) as cpool,
            tc.tile_pool(name="ssb", bufs=1) as ssb,
        ):
            # ---- broadcast constants ----
            def bcast(p, n, name):
                t = cpool.tile([128, n], f32, tag=name)
                nc.sync.dma_start(out=t[:], in_=rap(p, 0, [[0, 128], [1, n]]))
                return t

            n1w_b = bcast(n1w, C, "n1w"); n1b_b = bcast(n1b, C, "n1b")
            n2w_b = bcast(n2w, C, "n2w"); n2b_b = bcast(n2b, C, "n2b")
            fc2b_b = bcast(fc2b_p, C, "fc2b")
            b2c_b = cpool.tile([128, 2, 2 * BS], f32, tag="b2c")
            for bh in range(2):
                nc.sync.dma_start(out=b2c_b[:, bh, :],
                                  in_=rap(b2c_p, 0, [[0, 128], [1, 2 * BS]]))
            eps_sb = cpool.tile([128, 1], f32, tag="eps")
            nc.vector.memset(eps_sb[:], EPS)
            nlam_sb = cpool.tile([128, 1], f32, tag="nlam")
            nc.vector.memset(nlam_sb[:], -LAM)
            ident = cpool.tile([128, 128], f32, tag="ident")
            make_identity(nc, ident[:])
            identb = cpool.tile([128, 128], bf16, tag="identb")
            nc.vector.tensor_copy(out=identb[:], in_=ident[:])


            # ---- phase 0: LN1 + scatter into A2A-1 send buffer ----
            with (
                tc.tile_pool(name="p0", bufs=6) as p0,
                tc.tile_pool(name="p0s", bufs=8) as p0s,
            ):
                for it in range(65):
                    t0 = it * 128
                    nrow = min(128, TOKR - t0)
                    xt = p0.tile([128, C], f32, tag="xt")
                    nc.sync.dma_start(out=xt[:], in_=xs[t0 : t0 + 128, :])
                    st = p0s.tile([128, 2, 6], f32, tag="st")
                    for g in range(2):
                        nc.vector.bn_stats(out=st[:, g, :], in_=xt[:, 384 * g : 384 * (g + 1)])
                    mv = p0s.tile([128, 2], f32, tag="mv")
                    nc.vector.bn_aggr(out=mv[:], in_=st[:])
                    rstd = p0s.tile([128, 1], f32, tag="rstd")
                    nc.scalar.activation(out=rstd[:], in_=mv[:, 1:2], func=FT.Sqrt,
                                         bias=eps_sb[:], scale=1.0)
                    nc.vector.reciprocal(out=rstd[:], in_=rstd[:])
                    ln = p0.tile([128, C], f32, tag="ln")
                    nc.vector.tensor_scalar(out=ln[:], in0=xt[:], scalar1=mv[:, 0:1],
                                            scalar2=rstd[:], op0=OP.subtract, op1=OP.mult)
                    aeng = nc.vector
                    aeng.tensor_mul(out=ln[:], in0=ln[:], in1=n1w_b[:])
                    lnh = p0.tile([128, C], fp16, tag="lnh")
                    aeng.tensor_add(out=lnh[:], in0=ln[:], in1=n1b_b[:])
                    nc.scalar.dma_start(
                        out=rap(a1i, t0 * BS, [[BS, nrow], [TOKR * BS, N], [1, BS]]),
                        in_=lnh[:nrow].rearrange("p (j c) -> p j c", j=N),
                    )

            nc.gpsimd.collective_compute(
                "AllToAll", OP.bypass, replica_groups=RG, ins=[a1i[:]], outs=[a1o[:]])

            # ---- scale-shift MLPs (overlap with A2A1 window) ----
            ss_ctx = tc.tile_pool(name="ssw", bufs=1)
            ssw = ss_ctx.__enter__()
            ssp_ctx = tc.tile_pool(name="ssp", bufs=1, space="PSUM")
            ssp = ssp_ctx.__enter__()
            modT_sb = ssw.tile([MODD, 2], f32r)
            nc.sync.dma_start(out=modT_sb[:], in_=modT[:].bitcast(f32r))
            fs_w0_sb = ssw.tile([MODD, 2 * C], f32r)
            nc.sync.dma_start(out=fs_w0_sb[:], in_=fs_w0_p[:].bitcast(f32r))
            fs_b0T_sb = ssw.tile([128, 12], f32)
            nc.sync.dma_start(out=fs_b0T_sb[:], in_=fs_b0T_p[:])
            e0T = ssw.tile([128, 12], f32r)
            for j in range(12):
                pt = ssp.tile([128, 2], f32, tag="ss1")
                nc.tensor.matmul(pt[:], fs_w0_sb[:, 128 * j : 128 * (j + 1)],
                                 modT_sb[:], start=True, stop=True)
                nc.scalar.activation(out=e0T[:, j : j + 1], in_=pt[:, 0:1], func=FT.Gelu,
                                     bias=fs_b0T_sb[:, j : j + 1], scale=1.0)
            fs_w1s_sb = ssw.tile([128, 12, 2 * BS], f32r)
            nc.sync.dma_start(
                out=fs_w1s_sb[:],
                in_=rap(fs_w1s_p, 0, [[2 * BS, 128], [128 * 2 * BS, 12], [1, 2 * BS]]).bitcast(f32r),
            )
            fs_b1s_sb = ssw.tile([1, 2 * BS], f32)
            nc.sync.dma_start(out=fs_b1s_sb[:], in_=fs_b1s_p[:])
            p2 = ssp.tile([1, 2 * BS], f32, tag="ss2")
            for j in range(12):
                nc.tensor.matmul(p2[:], e0T[:, j : j + 1], fs_w1s_sb[:, j, :],
                                 start=(j == 0), stop=(j == 11))
            sfo = ssw.tile([1, 2 * BS], f32)
            nc.vector.tensor_add(out=sfo[:], in0=p2[:], in1=fs_b1s_sb[:])
            nc.sync.dma_start(out=sfd[:], in_=sfo[:])
            sfT = ssw.tile([BS, 2], f32)
            nc.sync.dma_start(out=sfT[:], in_=rap(sfd, 0, [[1, BS], [BS, 2]]))
            sfv = ssb.tile([BS, 1], f32)
            nc.vector.tensor_scalar_add(out=sfv[:], in0=sfT[:, 0:1], scalar1=1.0)
            b1r_sb = ssw.tile([BS, 1], f32)
            nc.sync.dma_start(out=b1r_sb[:], in_=b1r_p[:])
            b1i_sb = ssw.tile([BS, 1], f32)
            nc.sync.dma_start(out=b1i_sb[:], in_=b1i_p[:])
            Br = ssb.tile([BS, 1], f32)
            nc.vector.tensor_mul(out=Br[:], in0=b1r_sb[:], in1=sfv[:])
            nc.vector.tensor_add(out=Br[:], in0=Br[:], in1=sfT[:, 1:2])
            Bi = ssb.tile([BS, 1], f32)
            nc.vector.tensor_mul(out=Bi[:], in0=b1i_sb[:], in1=sfv[:])
            nc.vector.tensor_add(out=Bi[:], in0=Bi[:], in1=sfT[:, 1:2])

            # ms MLP: e1T then column-sharded 6144->768, AllGather
            ms_w0_sb = ssw.tile([MODD, LAT2], f32r)
            nc.gpsimd.dma_start(out=ms_w0_sb[:], in_=ms_w0_p[:].bitcast(f32r))
            ms_b0T_sb = ssw.tile([128, 48], f32)
            nc.sync.dma_start(out=ms_b0T_sb[:], in_=ms_b0T_p[:])
            e1T = ssw.tile([128, 48], bf16)
            for j in range(48):
                pt = ssp.tile([128, 2], f32, tag="ss1")
                nc.tensor.matmul(pt[:], ms_w0_sb[:, 128 * j : 128 * (j + 1)],
                                 modT_sb[:], start=True, stop=True)
                nc.scalar.activation(out=e1T[:, j : j + 1], in_=pt[:, 0:1], func=FT.Gelu,
                                     bias=ms_b0T_sb[:, j : j + 1], scale=1.0)
            p3 = ssp.tile([1, 2, 512], f32, tag="ss3")
            with tc.tile_pool(name="msw", bufs=3) as mswp:
                for j in range(48):
                    wt = mswp.tile([128, C], bf16)
                    nc.gpsimd.dma_start(
                        out=wt[:], in_=ms_w1s_p[128 * j : 128 * (j + 1), :])
                    for h2 in range(2):
                        nc.tensor.matmul(
                            p3[:, h2, 0:384], e1T[:, j : j + 1],
                            wt[:, 384 * h2 : 384 * (h2 + 1)],
                            start=(j == 0), stop=(j == 47))
            ms_b1s_sb = ssw.tile([1, C], f32)
            nc.sync.dma_start(out=ms_b1s_sb[:], in_=ms_b1s_p[:])
            mso = ssw.tile([1, C], f32)
            nc.vector.tensor_add(out=mso[:].rearrange("p (a b) -> p a b", a=2),
                                 in0=p3[:, :, 0:384],
                                 in1=ms_b1s_sb[:].rearrange("p (a b) -> p a b", a=2))
            nc.sync.dma_start(out=agi[:], in_=mso[:])
            nc.gpsimd.collective_compute(
                "AllGather", OP.bypass, replica_groups=RG, ins=[agi[:]], outs=[ago[:]])
            sM = ssb.tile([128, 24], f32)
            nc.sync.dma_start(out=sM[:], in_=rap(ago, 0, [[1, 128], [128, 24]]))
            nc.vector.tensor_scalar_add(out=sM[:], in0=sM[:], scalar1=1.0)
            tM = ssb.tile([128, 24], f32)
            nc.sync.dma_start(out=tM[:], in_=rap(ago, LAT, [[1, 128], [128, 24]]))
            fc1bT_sb = ssw.tile([128, 24], f32)
            nc.sync.dma_start(out=fc1bT_sb[:], in_=fc1bT_p[:])
            B1 = ssb.tile([128, 24], f32)
            nc.vector.tensor_mul(out=B1[:], in0=fc1bT_sb[:], in1=sM[:])
            nc.vector.tensor_add(out=B1[:], in0=B1[:], in1=tM[:])
            sM16 = ssb.tile([128, 24], f32)
            nc.vector.tensor_scalar_mul(out=sM16[:], in0=sM[:], scalar1=1.0 / 16.0)

            ssp_ctx.__exit__(None, None, None)
            ss_ctx.__exit__(None, None, None)

            # ---- phase 1 stage A: W-DFT  (X[h,w,c] -> t1d[kw,ri,h,c]) ----
            with (
                tc.tile_pool(name="sa", bufs=1) as sa,
                tc.tile_pool(name="sac", bufs=3) as sac,
                tc.tile_pool(name="sap", bufs=2, space="PSUM") as sap,
            ):
                fw_sb = []
                for ri, p in enumerate([fwr_p, fwi_p]):
                    t = sa.tile([120, 3, KW], fp16, tag=f"fw{ri}")
                    nc.sync.dma_start(
                        out=t[:], in_=rap(p, 0, [[KW, 120], [120 * KW, 3], [1, KW]]))
                    fw_sb.append(t)
                X_sb = sa.tile([120, 3, H, BS], fp16, tag="xsb")
                for ch in range(4):
                    for k in range(3):
                        eng = nc.scalar if (ch + k) % 2 else nc.sync
                        eng.dma_start(
                            out=X_sb[:, k, 45 * ch : 45 * (ch + 1), :],
                            in_=rap(a1o, (45 * ch * W + 120 * k) * BS,
                                    [[BS, 120], [W * BS, 45], [1, BS]]))
                for hs in range(36):
                    hh0 = 5 * hs
                    cp = sac.tile([KW, 2, 5, BS], fp16, tag="cpa")
                    for ri in range(2):
                        ps = sap.tile([KW, 5, BS], f32, tag="pa")
                        for k in range(3):
                            nc.tensor.matmul(ps[:], fw_sb[ri][:, k, :],
                                             X_sb[:, k, hh0 : hh0 + 5, :],
                                             start=(k == 0), stop=(k == 2))
                        nc.vector.tensor_copy(out=cp[:, ri, :, :], in_=ps[:])
                    nc.scalar.dma_start(
                        out=rap(t1d, hh0 * BS,
                                [[2 * H * BS, KW], [H * BS, 2], [BS, 5], [1, BS]]),
                        in_=cp[:])

            # ---- stages B+C+D fused in SBUF, then E ----
            with tc.tile_pool(name="fb", bufs=1) as fb:
                fhs_sb = fb.tile([90, 4, 2 * H], fp16)
                nc.sync.dma_start(
                    out=fhs_sb[:],
                    in_=rap(fhs_p, 0, [[2 * H, 90], [90 * 2 * H, 4], [1, 2 * H]]))
                ifhs_sb = fb.tile([90, 4, 2 * H], fp16)
                nc.sync.dma_start(
                    out=ifhs_sb[:],
                    in_=rap(ifhs_p, 0, [[2 * H, 90], [90 * 2 * H, 4], [1, 2 * H]]))
                w1r_sb = fb.tile([BS, BS], fp16)
                nc.sync.dma_start(out=w1r_sb[:], in_=w1r_p[:])
                w1i_sb = fb.tile([BS, BS], fp16)
                nc.sync.dma_start(out=w1i_sb[:], in_=w1i_p[:])
                w1in_sb = fb.tile([BS, BS], fp16)
                nc.sync.dma_start(out=w1in_sb[:], in_=w1in_p[:])
                w2cr_sb = fb.tile([BS, 2 * BS], fp16)
                nc.sync.dma_start(out=w2cr_sb[:], in_=w2cr_p[:])
                w2ci_sb = fb.tile([BS, 2 * BS], fp16)
                nc.sync.dma_start(out=w2ci_sb[:], in_=w2ci_p[:])
                ifw_sb = []
                for ri, p in enumerate([ifwr_p, ifwi_p]):
                    t = fb.tile([KW, 3, 120], fp16, tag=f"ifw{ri}")
                    nc.sync.dma_start(
                        out=t[:], in_=rap(p, 0, [[W, KW], [120, 3], [1, 120]]))
                    ifw_sb.append(t)
                # T1 resident: [h(90), half, ri, kw, c]
                T1_sb = fb.tile([90, 2, 2, KW, BS], fp16, tag="t1sb")
                for half in range(2):
                    for ri in range(2):
                        eng = nc.scalar if ri else nc.sync
                        eng.dma_start(
                            out=T1_sb[:, half, ri, :, :],
                            in_=rap(t1d, (ri * H + half * 90) * BS,
                                    [[BS, 90], [2 * H * BS, KW], [1, BS]]))
                U_sb = fb.tile([KW, BS, 2 * H], fp16, tag="usb")

                with (
                    tc.tile_pool(name="bcw", bufs=4) as bcw,
                    tc.tile_pool(name="bct", bufs=4) as bct,
                    tc.tile_pool(name="bco", bufs=3) as bco,
                    tc.tile_pool(name="bcp1", bufs=1, space="PSUM") as bcps,
                    tc.tile_pool(name="bcp2", bufs=1, space="PSUM") as bcps2,
                    tc.tile_pool(name="bcp3", bufs=2, space="PSUM") as bcps3,
                    tc.tile_pool(name="bcp4", bufs=1, space="PSUM") as bcps4,
                ):
                    def emit_D(O2t, kw0, G):
                        psU = bcps4.tile([BS, 2, 512], f32, tag="psU")
                        for g in range(G):
                            for q in range(4):
                                ri, half = q // 2, q % 2
                                nc.tensor.matmul(
                                    psU[:, g, 0 : 2 * H],
                                    O2t[:, g, half, ri * BS : (ri + 1) * BS],
                                    ifhs_sb[:, q, :], start=(q == 0), stop=(q == 3))
                        ucp = bcw.tile([BS, 2, 2 * H], fp16, tag="ucp")
                        nc.vector.tensor_copy(out=ucp[:, :G, :], in_=psU[:, :G, 0 : 2 * H])
                        nc.sync.dma_start(
                            out=rap(ud, kw0 * BS * 2 * H,
                                    [[2 * H, BS], [BS * 2 * H, G], [1, 2 * H]]),
                            in_=ucp[:, :G, :])

                    pend = None
                    for pr in range(46):
                        kw0 = 2 * pr
                        G = 2 if kw0 + 1 < KW else 1
                        psF = bcps.tile([BS, 2, 512], f32, tag="psF")
                        for g in range(G):
                            kw = kw0 + g
                            for q in range(4):
                                ri, half = q // 2, q % 2
                                nc.tensor.matmul(
                                    psF[:, g, 0 : 2 * H], T1_sb[:, half, ri, kw, :],
                                    fhs_sb[:, q, :], start=(q == 0), stop=(q == 3))
                        fsb = bcw.tile([BS, 2, 2 * H], fp16, tag="fsb")
                        nc.vector.tensor_copy(out=fsb[:, :G, :], in_=psF[:, :G, 0 : 2 * H])
                        ps1r = bcps2.tile([BS, 2, H], f32, tag="ps1r")
                        ps1i = bcps2.tile([BS, 2, H], f32, tag="ps1i")
                        nc.tensor.matmul(ps1r[:, :G, :], w1r_sb[:], fsb[:, :G, 0:H],
                                         start=True, stop=False)
                        nc.tensor.matmul(ps1r[:, :G, :], w1in_sb[:], fsb[:, :G, H : 2 * H],
                                         start=False, stop=True)
                        nc.tensor.matmul(ps1i[:, :G, :], w1i_sb[:], fsb[:, :G, 0:H],
                                         start=True, stop=False)
                        nc.tensor.matmul(ps1i[:, :G, :], w1r_sb[:], fsb[:, :G, H : 2 * H],
                                         start=False, stop=True)
                        o1r = bcw.tile([BS, 2, H], fp16, tag="o1r")
                        o1i = bcw.tile([BS, 2, H], fp16, tag="o1i")
                        nc.scalar.activation(out=o1r[:, :G, :], in_=ps1r[:, :G, :],
                                             func=FT.Relu, bias=Br[:], scale=sfv[:])
                        nc.scalar.activation(out=o1i[:, :G, :], in_=ps1i[:, :G, :],
                                             func=FT.Relu, bias=Bi[:], scale=sfv[:])
                        o1rf = o1r[:].rearrange("p g k -> p (g k)")
                        o1if = o1i[:].rearrange("p g k -> p (g k)")
                        O2t = bco.tile([90, 2, 2, 2 * BS], fp16, tag="o2sb")
                        for g in range(G):
                            ps2 = bcps3.tile([90, 2, 2 * BS], f32, tag="ps2")
                            for half in range(2):
                                sl = slice(90 * (2 * g + half), 90 * (2 * g + half + 1))
                                nc.tensor.matmul(ps2[:, half, :], o1rf[:, sl], w2cr_sb[:],
                                                 start=True, stop=False)
                                nc.tensor.matmul(ps2[:, half, :], o1if[:, sl], w2ci_sb[:],
                                                 start=False, stop=True)
                            tmp = bct.tile([90, 2, 2 * BS], f32, tag="tmp")
                            nc.vector.tensor_add(out=tmp[:], in0=ps2[:], in1=b2c_b[:90])
                            r1 = bct.tile([90, 2, 2 * BS], f32, tag="r1")
                            nc.vector.tensor_scalar(out=r1[:], in0=tmp[:], scalar1=-LAM,
                                                    scalar2=LAM, op0=OP.max, op1=OP.min)
                            nc.vector.tensor_sub(out=O2t[:, g, :, :],
                                                 in0=tmp[:], in1=r1[:])
                        # stage D (inverse H-DFT) pipelined one iteration
                        # behind B/C so the softshrink chain never stalls
                        # the in-order tensor queue
                        if pend is not None:
                            emit_D(*pend)
                        pend = (O2t, kw0, G)
                    emit_D(*pend)

                # U back to kw-partitioned SBUF, then stage E (inverse W-DFT)
                for chv in range(2):
                    k0 = 46 * chv
                    kn = min(46, KW - k0)
                    eng = nc.scalar if chv else nc.sync
                    eng.dma_start(
                        out=U_sb[k0 : k0 + kn, :, :],
                        in_=rap(ud, k0 * BS * 2 * H,
                                [[BS * 2 * H, kn], [2 * H, BS], [1, 2 * H]]))
                with (
                    tc.tile_pool(name="sec", bufs=4) as sec,
                    tc.tile_pool(name="sep", bufs=2, space="PSUM") as sep,
                ):
                    # chunk A = local rows 0-11 per dest, chunk B = rows 12-22;
                    # A2A for chunk A overlaps stage-E compute of chunk B.
                    for part in range(2):
                        rts = [(0, 4), (4, 4), (8, 4)] if part == 0 else \
                              [(12, 4), (16, 4), (20, 3)]
                        for d in range(N):
                            for roff, nr in rts:
                                h0 = HP * d + roff
                                nr = min(nr, H - h0)
                                if nr <= 0:
                                    continue
                                for wk in range(3):
                                    psE = sep.tile([120, 4, BS], f32, tag="psE")
                                    for ri in range(2):
                                        nc.tensor.matmul(
                                            psE[:, :nr, :], ifw_sb[ri][:, wk, :],
                                            U_sb[:, :, ri * H + h0 : ri * H + h0 + nr]
                                                .rearrange("p c h -> p h c"),
                                            start=(ri == 0), stop=(ri == 1))
                                    ecp = sec.tile([120, 4, BS], fp16, tag="ecp")
                                    nc.vector.tensor_copy(out=ecp[:, :nr, :],
                                                          in_=psE[:, :nr, :])
                                    if part == 0:
                                        dst, off = a2iA, d * MA + (roff * W + wk * 120) * BS
                                    else:
                                        dst, off = a2iB, d * MB + ((roff - 12) * W + wk * 120) * BS
                                    nc.scalar.dma_start(
                                        out=rap(dst, off, [[BS, 120], [W * BS, nr], [1, BS]]),
                                        in_=ecp[:, :nr, :])
                        if part == 0:
                            nc.gpsimd.collective_compute(
                                "AllToAll", OP.bypass, replica_groups=RG,
                                ins=[a2iA[:]], outs=[a2oA[:]])

            nc.gpsimd.collective_compute(
                "AllToAll", OP.bypass, replica_groups=RG, ins=[a2iB[:]], outs=[a2oB[:]])

            # ---- phase 2: h1 = F2 + ln1x + x; LN2; modulated MLP; + h1 ----
            with (
                tc.tile_pool(name="p2w", bufs=1) as p2w,
                tc.tile_pool(name="p2", bufs=4) as p2,
                tc.tile_pool(name="p2h", bufs=16) as p2h,
                tc.tile_pool(name="p2hm", bufs=2) as p2hm,
                tc.tile_pool(name="p2s", bufs=4) as p2s,
                tc.tile_pool(name="p2m", bufs=4) as p2m,
                tc.tile_pool(name="ptp", bufs=2, space="PSUM") as ptp,
                tc.tile_pool(name="php", bufs=2, space="PSUM") as php,
                tc.tile_pool(name="pop", bufs=2, space="PSUM") as pop,
            ):
                PM = mybir.MatmulPerfMode.DoubleRow
                fc1w_sb = p2w.tile([128, 6, LAT], fp8)
                nc.sync.dma_start(
                    out=fc1w_sb[:], in_=rap(fc1w_p, 0, [[LAT, 128], [128 * LAT, 6], [1, LAT]]))
                fc2w_sb = p2w.tile([128, 24, C], fp8)
                nc.sync.dma_start(
                    out=fc2w_sb[:], in_=rap(fc2w_p, 0, [[C, 128], [128 * C, 24], [1, C]]))

                def p2_prep(it):
                    T0 = it * 512
                    ln2T = p2m.tile([128, 6, 4, 128], fp8, tag="ln2T")
                    h1s = []
                    for hf in range(4):
                        t0 = T0 + 128 * hf
                        nload = max(0, min(128, TOKR - t0))
                        xt = p2.tile([128, C], f32, tag="xt2")
                        nc.sync.dma_start(out=xt[:], in_=xs[t0 : t0 + 128, :])
                        f2t = p2.tile([128, N, BS], fp16, tag="f2t")
                        l1t = p2.tile([128, N, BS], fp16, tag="l1t")
                        if nload < 128:
                            nc.vector.memset(f2t[:], 0.0)
                            nc.vector.memset(l1t[:], 0.0)
                        if nload > 0:
                            ta = min(nload, max(0, TA - t0))
                            if ta > 0:
                                nc.sync.dma_start(
                                    out=f2t[:ta],
                                    in_=rap(a2oA, t0 * BS, [[BS, ta], [MA, N], [1, BS]]))
                            if ta < nload:
                                t0b = t0 + ta - TA
                                nc.sync.dma_start(
                                    out=f2t[ta:nload],
                                    in_=rap(a2oB, t0b * BS, [[BS, nload - ta], [MB, N], [1, BS]]))
                            nc.sync.dma_start(
                                out=l1t[:nload],
                                in_=rap(a1i, t0 * BS, [[BS, nload], [TOKR * BS, N], [1, BS]]))
                        h1 = p2h.tile([128, C], f32, tag="h1")
                        nc.vector.tensor_add(out=h1[:], in0=xt[:],
                                             in1=f2t[:].rearrange("p j c -> p (j c)"))
                        nc.vector.tensor_add(out=h1[:], in0=h1[:],
                                             in1=l1t[:].rearrange("p j c -> p (j c)"))
                        h1s.append(h1)
                        st = p2s.tile([128, 2, 6], f32, tag="st2")
                        for g in range(2):
                            nc.vector.bn_stats(out=st[:, g, :], in_=h1[:, 384 * g : 384 * (g + 1)])
                        mv = p2s.tile([128, 2], f32, tag="mv2")
                        nc.vector.bn_aggr(out=mv[:], in_=st[:])
                        rstd = p2s.tile([128, 1], f32, tag="rstd2")
                        nc.scalar.activation(out=rstd[:], in_=mv[:, 1:2], func=FT.Sqrt,
                                             bias=eps_sb[:], scale=1.0)
                        nc.vector.reciprocal(out=rstd[:], in_=rstd[:])
                        ln2 = p2.tile([128, C], bf16, tag="ln2")
                        nc.vector.tensor_scalar(out=ln2[:], in0=h1[:], scalar1=mv[:, 0:1],
                                                scalar2=rstd[:], op0=OP.subtract, op1=OP.mult)
                        nc.vector.tensor_mul(out=ln2[:], in0=ln2[:], in1=n2w_b[:])
                        nc.vector.tensor_add(out=ln2[:], in0=ln2[:], in1=n2b_b[:])
                        for jb in range(2):
                            pst = ptp.tile([128, 3, 128], bf16, tag="pst")
                            for jj in range(3):
                                j = 3 * jb + jj
                                nc.tensor.transpose(pst[:, jj, :],
                                                    ln2[:, 128 * j : 128 * (j + 1)],
                                                    identb[:])
                            nc.vector.tensor_copy(out=ln2T[:, 3 * jb : 3 * jb + 3, hf, :],
                                                  in_=pst[:])
                    return T0, ln2T, h1s

                def p2_mm(T0, ln2T, h1s):
                    hmidT = p2hm.tile([128, 24, 512], fp8, tag="hmidT")
                    for l in range(24):
                        psH = php.tile([128, 512], f32, tag="psH")
                        for jp in range(3):
                            nc.tensor.matmul(
                                psH[:],
                                fc1w_sb[:, 2 * jp : 2 * jp + 2, 128 * l : 128 * (l + 1)],
                                ln2T[:, 2 * jp : 2 * jp + 2, :, :]
                                    .rearrange("p j h t -> p j (h t)"),
                                start=(jp == 0), stop=(jp == 2), perf_mode=PM)
                        nc.scalar.activation(out=hmidT[:, l, :], in_=psH[:], func=FT.Gelu,
                                             bias=B1[:, l : l + 1], scale=sM16[:, l : l + 1])
                    for hf in range(4):
                        t0 = T0 + 128 * hf
                        psO = pop.tile([128, 2, 512], f32, tag="psO")
                        for lp in range(12):
                            for h2 in range(2):
                                nc.tensor.matmul(
                                    psO[:, h2, 0:384],
                                    hmidT[:, 2 * lp : 2 * lp + 2, 128 * hf : 128 * (hf + 1)],
                                    fc2w_sb[:, 2 * lp : 2 * lp + 2, 384 * h2 : 384 * (h2 + 1)],
                                    start=(lp == 0), stop=(lp == 11), perf_mode=PM)
                        ot = p2.tile([128, C], f32, tag="ot")
                        nc.vector.scalar_tensor_tensor(
                            out=ot[:].rearrange("p (a b) -> p a b", a=2),
                            in0=psO[:, :, 0:384], scalar=1.0 / 16.0,
                            in1=h1s[hf][:].rearrange("p (a b) -> p a b", a=2),
                            op0=OP.mult, op1=OP.add)
                        nc.gpsimd.tensor_add(out=ot[:], in0=ot[:], in1=fc2b_b[:])
                        nc.sync.dma_start(out=out_p[t0 : t0 + 128, :], in_=ot[:])

                # depth-4 software pipeline, preps paired so scalar runs
                # [Sqrt x8, Gelu x48] per pair (one act-table swap per tile)
                sq = [p2_prep(i) for i in range(min(4, NT2))]
                for itp in range(0, NT2, 2):
                    p2_mm(*sq[itp])
                    if itp + 1 < NT2:
                        p2_mm(*sq[itp + 1])
                    for j in (itp + 4, itp + 5):
                        if j < NT2:
                            sq.append(p2_prep(j))

    nc.compile()
    return nc


_NC = None


def _get_nc():
    global _NC
    if _NC is None:
        _NC = _build()
    return _NC


def _dft_mats():
    w = np.arange(W); kw = np.arange(KW)
    ang = 2 * np.pi * np.outer(w, kw) / W
    fwr = (np.cos(ang) / np.sqrt(W)).astype(np.float32)
    fwi = (-np.sin(ang) / np.sqrt(W)).astype(np.float32)
    kh = np.arange(H); h = np.arange(H)
    angh = 2 * np.pi * np.outer(kh, h) / H        # [kh, h]
    fhr = np.cos(angh) / np.sqrt(H)
    fhi = -np.sin(angh) / np.sqrt(H)
    fhs = np.zeros((2 * H, 2 * H))
    fhs[:H, :H] = fhr.T; fhs[:H, H:] = fhi.T
    fhs[H:, :H] = -fhi.T; fhs[H:, H:] = fhr.T
    ci = np.cos(angh) / np.sqrt(H)                # [kh, h] for inverse
    si = np.sin(angh) / np.sqrt(H)
    ifhs = np.zeros((2 * H, 2 * H))
    ifhs[:H, :H] = ci; ifhs[:H, H:] = si
    ifhs[H:, :H] = -si; ifhs[H:, H:] = ci
    ckw = np.where(kw == 0, 1.0, 2.0)
    angi = 2 * np.pi * np.outer(kw, np.arange(W)) / W    # [kw, w]
    ifwr = (ckw[:, None] * np.cos(angi) / np.sqrt(W)).astype(np.float32)
    ifwi = (-ckw[:, None] * np.sin(angi) / np.sqrt(W)).astype(np.float32)
    return fwr, fwi, fhs.astype(np.float32), ifhs.astype(np.float32), ifwr, ifwi


def _prepare_in_maps(x, mod_embed, n1w, n1b, n2w, n2b, w1, b1, w2, b2,
                     fs_w0, fs_b0, fs_w1, fs_b1, fc1w, fc1b, fc2w, fc2b,
                     ms_w0, ms_b0, ms_w1, ms_b1):
    f = np.asarray
    x = f(x, dtype=np.float32)
    grid = x.reshape(H, W, C)
    fwr, fwi, fhs, ifhs, ifwr, ifwi = _dft_mats()
    bf = ml_dtypes.bfloat16

    in_maps = []
    for b in range(N):
        r0, r1 = HP * b, min(HP * (b + 1), H)
        xsb = np.zeros((TOKP, C), np.float32)
        xsb[: (r1 - r0) * W] = grid[r0:r1].reshape(-1, C)
        sl = slice(BS * b, BS * (b + 1))
        w2r = f(w2[0, b], np.float32); w2i = f(w2[1, b], np.float32)
        im = {
            "xs": xsb,
            "modT": np.repeat(f(mod_embed, np.float32).reshape(MODD, 1), 2, axis=1).copy(),
            "n1w": f(n1w, np.float32), "n1b": f(n1b, np.float32),
            "n2w": f(n2w, np.float32), "n2b": f(n2b, np.float32),
            "fwr": fwr.astype(np.float16), "fwi": fwi.astype(np.float16),
            "fhs": fhs.astype(np.float16), "ifhs": ifhs.astype(np.float16),
            "ifwr": ifwr.astype(np.float16), "ifwi": ifwi.astype(np.float16),
            "w1r": f(w1[0, b], np.float16).copy(),
            "w1i": f(w1[1, b], np.float16).copy(),
            "w1in": (-f(w1[1, b], np.float16)).copy(),
            "w2cr": np.concatenate([w2r, w2i], axis=1).astype(np.float16),
            "w2ci": np.concatenate([-w2i, w2r], axis=1).astype(np.float16),
            "b1r": f(b1[0, b], np.float32).reshape(BS, 1).copy(),
            "b1i": f(b1[1, b], np.float32).reshape(BS, 1).copy(),
            "b2c": np.concatenate([f(b2[0, b], np.float32), f(b2[1, b], np.float32)]),
            "fs_w0": f(fs_w0, np.float32),
            "fs_b0T": f(fs_b0, np.float32).reshape(12, 128).T.copy(),
            "fs_w1s": np.concatenate(
                [f(fs_w1, np.float32)[:, sl], f(fs_w1, np.float32)[:, C + BS * b : C + BS * (b + 1)]],
                axis=1),
            "fs_b1s": np.concatenate(
                [f(fs_b1, np.float32)[sl], f(fs_b1, np.float32)[C + BS * b : C + BS * (b + 1)]]
            ).reshape(1, -1),
            "ms_w0": f(ms_w0, np.float32),
            "ms_b0T": f(ms_b0, np.float32).reshape(48, 128).T.copy(),
            "ms_w1s": f(ms_w1, np.float32)[:, C * b : C * (b + 1)].astype(bf),
            "ms_b1s": f(ms_b1, np.float32)[C * b : C * (b + 1)].reshape(1, -1),
            "fc1w": (16.0 * f(fc1w, np.float32)).astype(ml_dtypes.float8_e4m3),
            "fc1bT": f(fc1b, np.float32).reshape(24, 128).T.copy(),
            "fc2w": (16.0 * f(fc2w, np.float32)).astype(ml_dtypes.float8_e4m3),
            "fc2b": f(fc2b, np.float32),
        }
        in_maps.append(im)
    return in_maps


def kernel(x, mod_embed, n1w, n1b, n2w, n2b, w1, b1, w2, b2,
           fs_w0, fs_b0, fs_w1, fs_b1, fc1w, fc1b, fc2w, fc2b,
           ms_w0, ms_b0, ms_w1, ms_b1):
    nc = _get_nc()
    in_maps = _prepare_in_maps(
        x, mod_embed, n1w, n1b, n2w, n2b, w1, b1, w2, b2,
        fs_w0, fs_b0, fs_w1, fs_b1, fc1w, fc1b, fc2w, fc2b,
        ms_w0, ms_b0, ms_w1, ms_b1)

    res = run_bass_kernel_spmd(nc, in_maps, core_ids=list(range(N)))
    globals()["last_results"] = res
    out = np.zeros((H, W, C), np.float32)
    for b in range(N):
        r0, r1 = HP * b, min(HP * (b + 1), H)
        out[r0:r1] = res.results[b]["out"][: (r1 - r0) * W].reshape(r1 - r0, W, C)
    return out.reshape(1, H, W, C)



# revision 40
# speedup vs baseline: 1.1862x; 1.0394x over previous
"""AFNO block kernel for 8 Trainium2 NeuronCores.

Sharding: token-shard (H rows, 23 per core padded) for LN/MLP phases;
AllToAll to channel-shard (core i = spectral block i, 96 channels) for the
2D-FFT filter, computed as matmuls against precomputed DFT matrices;
AllToAll back; small AllGather for the column-sharded 6144x6144 scale-shift
MLP weight.
"""

import os
import numpy as np
import ml_dtypes

import concourse.bass as bass
import concourse.bacc as bacc
import concourse.mybir as mybir
import concourse.tile as tile
from concourse.bass_utils import run_bass_kernel_spmd
from concourse.masks import make_identity

f32 = mybir.dt.float32
f32r = mybir.dt.float32r
bf16 = mybir.dt.bfloat16
fp16 = mybir.dt.float16
fp8 = mybir.dt.float8e4
FT = mybir.ActivationFunctionType
OP = mybir.AluOpType

H, W, C = 180, 360, 768
NB, BS, KW = 8, 96, 91
HP = 23                 # rows per shard (8*23 = 184 >= 180)
TOKR = HP * W           # 8280 real token slots per shard
NT2 = 17                # phase-2 tiles of 512
TOKP = NT2 * 512        # 8704 padded tokens per shard
MODD, LAT, LAT2 = 64, 3072, 6144
LAM = 0.01
EPS = 1e-5
N = 8


def rap(t, offset, dims):
    a = t[:] if not isinstance(t, bass.AP) else t
    return bass.AP(tensor=a.tensor, offset=a.offset + offset, ap=[list(d) for d in dims])


def _build():
    nc = bacc.Bacc("TRN2", target_bir_lowering=False, debug=False, num_devices=N)

    def P(name, shp, dt=f32):
        return nc.declare_dram_parameter(name, list(shp), dt, isOutput=False)

    xs = P("xs", [TOKP, C])
    modT = P("modT", [MODD, 2])
    n1w = P("n1w", [C]); n1b = P("n1b", [C])
    n2w = P("n2w", [C]); n2b = P("n2b", [C])
    fwr_p = P("fwr", [W, KW], fp16); fwi_p = P("fwi", [W, KW], fp16)
    fhs_p = P("fhs", [2 * H, 2 * H], fp16)
    ifhs_p = P("ifhs", [2 * H, 2 * H], fp16)
    ifwr_p = P("ifwr", [KW, W], fp16); ifwi_p = P("ifwi", [KW, W], fp16)
    w1r_p = P("w1r", [BS, BS], fp16); w1i_p = P("w1i", [BS, BS], fp16)
    w1in_p = P("w1in", [BS, BS], fp16)
    w2cr_p = P("w2cr", [BS, 2 * BS], fp16)   # [W2r | W2i]
    w2ci_p = P("w2ci", [BS, 2 * BS], fp16)   # [-W2i | W2r]
    b1r_p = P("b1r", [BS, 1]); b1i_p = P("b1i", [BS, 1])
    b2c_p = P("b2c", [2 * BS])               # concat(b2r, b2i)
    fs_w0_p = P("fs_w0", [MODD, 2 * C])
    fs_b0T_p = P("fs_b0T", [128, 12])
    fs_w1s_p = P("fs_w1s", [2 * C, 2 * BS])
    fs_b1s_p = P("fs_b1s", [1, 2 * BS])
    ms_w0_p = P("ms_w0", [MODD, LAT2])
    ms_b0T_p = P("ms_b0T", [128, 48])
    ms_w1s_p = P("ms_w1s", [LAT2, C], bf16)
    ms_b1s_p = P("ms_b1s", [1, C])
    fc1w_p = P("fc1w", [C, LAT], fp8)
    fc1bT_p = P("fc1bT", [128, 24])
    fc2w_p = P("fc2w", [LAT, C], fp8)
    fc2b_p = P("fc2b", [C])
    out_p = nc.declare_dram_parameter("out", [TOKP, C], f32, isOutput=True)
    DBG = False

    # internal DRAM
    a1i = nc.dram_tensor("a1i", [N, TOKR * BS], fp16)
    a1o = nc.dram_tensor("a1o", [N, TOKR * BS], fp16)
    MA = 12 * W * BS        # chunk A: local rows 0-11 per dest
    MB = 11 * W * BS        # chunk B: local rows 12-22
    TA = 12 * W             # tokens per dest covered by chunk A
    a2iA = nc.dram_tensor("a2iA", [N, MA], fp16)
    a2oA = nc.dram_tensor("a2oA", [N, MA], fp16)
    a2iB = nc.dram_tensor("a2iB", [N, MB], fp16)
    a2oB = nc.dram_tensor("a2oB", [N, MB], fp16)
    t1d = nc.dram_tensor("t1d", [KW, 2, H, BS], fp16)   # [kw][ri][h][c]
    ud = nc.dram_tensor("ud", [KW, BS, 2 * H], fp16)
    sfd = nc.dram_tensor("sfd", [1, 2 * BS], f32)
    agi = nc.dram_tensor("agi", [1, C], f32)
    ago = nc.dram_tensor("ago", [N, C], f32)

    RG = [list(range(N))]

    with tile.TileContext(nc) as tc:
        with (
            tc.tile_pool(name="const", bufs=1) as cpool,
            tc.tile_pool(name="ssb", bufs=1) as ssb,
        ):
            # ---- broadcast constants ----
            def bcast(p, n, name):
                t = cpool.tile([128, n], f32, tag=name)
                nc.sync.dma_start(out=t[:], in_=rap(p, 0, [[0, 128], [1, n]]))
                return t

            n1w_b = bcast(n1w, C, "n1w"); n1b_b = bcast(n1b, C, "n1b")
            n2w_b = bcast(n2w, C, "n2w"); n2b_b = bcast(n2b, C, "n2b")
            fc2b_b = bcast(fc2b_p, C, "fc2b")
            b2c_b = cpool.tile([128, 2, 2 * BS], f32, tag="b2c")
            for bh in range(2):
                nc.sync.dma_start(out=b2c_b[:, bh, :],
                                  in_=rap(b2c_p, 0, [[0, 128], [1, 2 * BS]]))
            eps_sb = cpool.tile([128, 1], f32, tag="eps")
            nc.vector.memset(eps_sb[:], EPS)
            nlam_sb = cpool.tile([128, 1], f32, tag="nlam")
            nc.vector.memset(nlam_sb[:], -LAM)
            ident = cpool.tile([128, 128], f32, tag="ident")
            make_identity(nc, ident[:])
            identb = cpool.tile([128, 128], bf16, tag="identb")
            nc.vector.tensor_copy(out=identb[:], in_=ident[:])


            # ---- phase 0: LN1 + scatter into A2A-1 send buffer ----
            with (
                tc.tile_pool(name="p0", bufs=6) as p0,
                tc.tile_pool(name="p0s", bufs=8) as p0s,
            ):
                for it in range(65):
                    t0 = it * 128
                    nrow = min(128, TOKR - t0)
                    xt = p0.tile([128, C], f32, tag="xt")
                    nc.sync.dma_start(out=xt[:], in_=xs[t0 : t0 + 128, :])
                    st = p0s.tile([128, 2, 6], f32, tag="st")
                    for g in range(2):
                        nc.vector.bn_stats(out=st[:, g, :], in_=xt[:, 384 * g : 384 * (g + 1)])
                    mv = p0s.tile([128, 2], f32, tag="mv")
                    nc.vector.bn_aggr(out=mv[:], in_=st[:])
                    rstd = p0s.tile([128, 1], f32, tag="rstd")
                    nc.scalar.activation(out=rstd[:], in_=mv[:, 1:2], func=FT.Sqrt,
                                         bias=eps_sb[:], scale=1.0)
                    nc.vector.reciprocal(out=rstd[:], in_=rstd[:])
                    ln = p0.tile([128, C], f32, tag="ln")
                    nc.vector.tensor_scalar(out=ln[:], in0=xt[:], scalar1=mv[:, 0:1],
                                            scalar2=rstd[:], op0=OP.subtract, op1=OP.mult)
                    aeng = nc.vector
                    aeng.tensor_mul(out=ln[:], in0=ln[:], in1=n1w_b[:])
                    lnh = p0.tile([128, C], fp16, tag="lnh")
                    aeng.tensor_add(out=lnh[:], in0=ln[:], in1=n1b_b[:])
                    nc.scalar.dma_start(
                        out=rap(a1i, t0 * BS, [[BS, nrow], [TOKR * BS, N], [1, BS]]),
                        in_=lnh[:nrow].rearrange("p (j c) -> p j c", j=N),
                    )

            nc.gpsimd.collective_compute(
                "AllToAll", OP.bypass, replica_groups=RG, ins=[a1i[:]], outs=[a1o[:]])

            # ---- scale-shift MLPs (overlap with A2A1 window) ----
            ss_ctx = tc.tile_pool(name="ssw", bufs=1)
            ssw = ss_ctx.__enter__()
            ssp_ctx = tc.tile_pool(name="ssp", bufs=1, space="PSUM")
            ssp = ssp_ctx.__enter__()
            modT_sb = ssw.tile([MODD, 2], f32r)
            nc.sync.dma_start(out=modT_sb[:], in_=modT[:].bitcast(f32r))
            fs_w0_sb = ssw.tile([MODD, 2 * C], f32r)
            nc.sync.dma_start(out=fs_w0_sb[:], in_=fs_w0_p[:].bitcast(f32r))
            fs_b0T_sb = ssw.tile([128, 12], f32)
            nc.sync.dma_start(out=fs_b0T_sb[:], in_=fs_b0T_p[:])
            e0T = ssw.tile([128, 12], f32r)
            for j in range(12):
                pt = ssp.tile([128, 2], f32, tag="ss1")
                nc.tensor.matmul(pt[:], fs_w0_sb[:, 128 * j : 128 * (j + 1)],
                                 modT_sb[:], start=True, stop=True)
                nc.scalar.activation(out=e0T[:, j : j + 1], in_=pt[:, 0:1], func=FT.Gelu,
                                     bias=fs_b0T_sb[:, j : j + 1], scale=1.0)
            fs_w1s_sb = ssw.tile([128, 12, 2 * BS], f32r)
            nc.sync.dma_start(
                out=fs_w1s_sb[:],
                in_=rap(fs_w1s_p, 0, [[2 * BS, 128], [128 * 2 * BS, 12], [1, 2 * BS]]).bitcast(f32r),
            )
            fs_b1s_sb = ssw.tile([1, 2 * BS], f32)
            nc.sync.dma_start(out=fs_b1s_sb[:], in_=fs_b1s_p[:])
            p2 = ssp.tile([1, 2 * BS], f32, tag="ss2")
            for j in range(12):
                nc.tensor.matmul(p2[:], e0T[:, j : j + 1], fs_w1s_sb[:, j, :],
                                 start=(j == 0), stop=(j == 11))
            sfo = ssw.tile([1, 2 * BS], f32)
            nc.vector.tensor_add(out=sfo[:], in0=p2[:], in1=fs_b1s_sb[:])
            nc.sync.dma_start(out=sfd[:], in_=sfo[:])
            sfT = ssw.tile([BS, 2], f32)
            nc.sync.dma_start(out=sfT[:], in_=rap(sfd, 0, [[1, BS], [BS, 2]]))
            sfv = ssb.tile([BS, 1], f32)
            nc.vector.tensor_scalar_add(out=sfv[:], in0=sfT[:, 0:1], scalar1=1.0)
            b1r_sb = ssw.tile([BS, 1], f32)
            nc.sync.dma_start(out=b1r_sb[:], in_=b1r_p[:])
            b1i_sb = ssw.tile([BS, 1], f32)
            nc.sync.dma_start(out=b1i_sb[:], in_=b1i_p[:])
            Br = ssb.tile([BS, 1], f32)
            nc.vector.tensor_mul(out=Br[:], in0=b1r_sb[:], in1=sfv[:])
            nc.vector.tensor_add(out=Br[:], in0=Br[:], in1=sfT[:, 1:2])
            Bi = ssb.tile([BS, 1], f32)
            nc.vector.tensor_mul(out=Bi[:], in0=b1i_sb[:], in1=sfv[:])
            nc.vector.tensor_add(out=Bi[:], in0=Bi[:], in1=sfT[:, 1:2])

            # ms MLP: e1T then column-sharded 6144->768, AllGather
            ms_w0_sb = ssw.tile([MODD, LAT2], f32r)
            nc.gpsimd.dma_start(out=ms_w0_sb[:], in_=ms_w0_p[:].bitcast(f32r))
            ms_b0T_sb = ssw.tile([128, 48], f32)
            nc.sync.dma_start(out=ms_b0T_sb[:], in_=ms_b0T_p[:])
            e1T = ssw.tile([128, 48], bf16)
            for j in range(48):
                pt = ssp.tile([128, 2], f32, tag="ss1")
                nc.tensor.matmul(pt[:], ms_w0_sb[:, 128 * j : 128 * (j + 1)],
                                 modT_sb[:], start=True, stop=True)
                nc.scalar.activation(out=e1T[:, j : j + 1], in_=pt[:, 0:1], func=FT.Gelu,
                                     bias=ms_b0T_sb[:, j : j + 1], scale=1.0)
            p3 = ssp.tile([1, 2, 512], f32, tag="ss3")
            with tc.tile_pool(name="msw", bufs=3) as mswp:
                for j in range(48):
                    wt = mswp.tile([128, C], bf16)
                    nc.gpsimd.dma_start(
                        out=wt[:], in_=ms_w1s_p[128 * j : 128 * (j + 1), :])
                    for h2 in range(2):
                        nc.tensor.matmul(
                            p3[:, h2, 0:384], e1T[:, j : j + 1],
                            wt[:, 384 * h2 : 384 * (h2 + 1)],
                            start=(j == 0), stop=(j == 47))
            ms_b1s_sb = ssw.tile([1, C], f32)
            nc.sync.dma_start(out=ms_b1s_sb[:], in_=ms_b1s_p[:])
            mso = ssw.tile([1, C], f32)
            nc.vector.tensor_add(out=mso[:].rearrange("p (a b) -> p a b", a=2),
                                 in0=p3[:, :, 0:384],
                                 in1=ms_b1s_sb[:].rearrange("p (a b) -> p a b", a=2))
            nc.sync.dma_start(out=agi[:], in_=mso[:])
            nc.gpsimd.collective_compute(
                "AllGather", OP.bypass, replica_groups=RG, ins=[agi[:]], outs=[ago[:]])
            sM = ssb.tile([128, 24], f32)
            nc.sync.dma_start(out=sM[:], in_=rap(ago, 0, [[1, 128], [128, 24]]))
            nc.vector.tensor_scalar_add(out=sM[:], in0=sM[:], scalar1=1.0)
            tM = ssb.tile([128, 24], f32)
            nc.sync.dma_start(out=tM[:], in_=rap(ago, LAT, [[1, 128], [128, 24]]))
            fc1bT_sb = ssw.tile([128, 24], f32)
            nc.sync.dma_start(out=fc1bT_sb[:], in_=fc1bT_p[:])
            B1 = ssb.tile([128, 24], f32)
            nc.vector.tensor_mul(out=B1[:], in0=fc1bT_sb[:], in1=sM[:])
            nc.vector.tensor_add(out=B1[:], in0=B1[:], in1=tM[:])
            sM16 = ssb.tile([128, 24], f32)
            nc.vector.tensor_scalar_mul(out=sM16[:], in0=sM[:], scalar1=1.0 / 16.0)

            ssp_ctx.__exit__(None, None, None)
            ss_ctx.__exit__(None, None, None)

            # ---- phase 1 stage A: W-DFT  (X[h,w,c] -> t1d[kw,ri,h,c]) ----
            with (
                tc.tile_pool(name="sa", bufs=1) as sa,
                tc.tile_pool(name="sac", bufs=3) as sac,
                tc.tile_pool(name="sap", bufs=2, space="PSUM") as sap,
            ):
                fw_sb = []
                for ri, p in enumerate([fwr_p, fwi_p]):
                    t = sa.tile([120, 3, KW], fp16, tag=f"fw{ri}")
                    nc.sync.dma_start(
                        out=t[:], in_=rap(p, 0, [[KW, 120], [120 * KW, 3], [1, KW]]))
                    fw_sb.append(t)
                X_sb = sa.tile([120, 3, H, BS], fp16, tag="xsb")
                for ch in range(4):
                    for k in range(3):
                        eng = nc.scalar if (ch + k) % 2 else nc.sync
                        eng.dma_start(
                            out=X_sb[:, k, 45 * ch : 45 * (ch + 1), :],
                            in_=rap(a1o, (45 * ch * W + 120 * k) * BS,
                                    [[BS, 120], [W * BS, 45], [1, BS]]))
                for hs in range(36):
                    hh0 = 5 * hs
                    cp = sac.tile([KW, 2, 5, BS], fp16, tag="cpa")
                    for ri in range(2):
                        ps = sap.tile([KW, 5, BS], f32, tag="pa")
                        for k in range(3):
                            nc.tensor.matmul(ps[:], fw_sb[ri][:, k, :],
                                             X_sb[:, k, hh0 : hh0 + 5, :],
                                             start=(k == 0), stop=(k == 2))
                        nc.vector.tensor_copy(out=cp[:, ri, :, :], in_=ps[:])
                    nc.scalar.dma_start(
                        out=rap(t1d, hh0 * BS,
                                [[2 * H * BS, KW], [H * BS, 2], [BS, 5], [1, BS]]),
                        in_=cp[:])

            # ---- stages B+C+D fused in SBUF, then E ----
            with tc.tile_pool(name="fb", bufs=1) as fb:
                # forward H-DFT stacked matrix chunked as 3x120 (vs 4x90):
                # two fewer serialized matmuls per kw pair in stage B
                fhs_sb = fb.tile([120, 3, 2 * H], fp16)
                nc.sync.dma_start(
                    out=fhs_sb[:],
                    in_=rap(fhs_p, 0, [[2 * H, 120], [120 * 2 * H, 3], [1, 2 * H]]))
                ifhs_sb = fb.tile([90, 4, 2 * H], fp16)
                nc.sync.dma_start(
                    out=ifhs_sb[:],
                    in_=rap(ifhs_p, 0, [[2 * H, 90], [90 * 2 * H, 4], [1, 2 * H]]))
                w1r_sb = fb.tile([BS, BS], fp16)
                nc.sync.dma_start(out=w1r_sb[:], in_=w1r_p[:])
                w1i_sb = fb.tile([BS, BS], fp16)
                nc.sync.dma_start(out=w1i_sb[:], in_=w1i_p[:])
                w1in_sb = fb.tile([BS, BS], fp16)
                nc.sync.dma_start(out=w1in_sb[:], in_=w1in_p[:])
                w2cr_sb = fb.tile([BS, 2 * BS], fp16)
                nc.sync.dma_start(out=w2cr_sb[:], in_=w2cr_p[:])
                w2ci_sb = fb.tile([BS, 2 * BS], fp16)
                nc.sync.dma_start(out=w2ci_sb[:], in_=w2ci_p[:])
                ifw_sb = []
                for ri, p in enumerate([ifwr_p, ifwi_p]):
                    t = fb.tile([KW, 3, 120], fp16, tag=f"ifw{ri}")
                    nc.sync.dma_start(
                        out=t[:], in_=rap(p, 0, [[W, KW], [120, 3], [1, 120]]))
                    ifw_sb.append(t)
                # T1 resident: [hri(120), chunk(3), kw, c] with hri = ri*H + h
                T1_sb = fb.tile([120, 3, KW, BS], fp16, tag="t1sb")
                for k in range(3):
                    eng = nc.scalar if k == 1 else nc.sync
                    eng.dma_start(
                        out=T1_sb[:, k, :, :],
                        in_=rap(t1d, 120 * k * BS,
                                [[BS, 120], [2 * H * BS, KW], [1, BS]]))
                U_sb = fb.tile([KW, BS, 2 * H], fp16, tag="usb")

                with (
                    tc.tile_pool(name="bcw", bufs=4) as bcw,
                    tc.tile_pool(name="bct", bufs=4) as bct,
                    tc.tile_pool(name="bco", bufs=3) as bco,
                    tc.tile_pool(name="bcp1", bufs=1, space="PSUM") as bcps,
                    tc.tile_pool(name="bcp2", bufs=1, space="PSUM") as bcps2,
                    tc.tile_pool(name="bcp3", bufs=2, space="PSUM") as bcps3,
                    tc.tile_pool(name="bcp4", bufs=1, space="PSUM") as bcps4,
                ):
                    def emit_D(O2t, kw0, G):
                        psU = bcps4.tile([BS, 2, 512], f32, tag="psU")
                        for g in range(G):
                            for q in range(4):
                                ri, half = q // 2, q % 2
                                nc.tensor.matmul(
                                    psU[:, g, 0 : 2 * H],
                                    O2t[:, g, half, ri * BS : (ri + 1) * BS],
                                    ifhs_sb[:, q, :], start=(q == 0), stop=(q == 3))
                        ucp = bcw.tile([BS, 2, 2 * H], fp16, tag="ucp")
                        nc.vector.tensor_copy(out=ucp[:, :G, :], in_=psU[:, :G, 0 : 2 * H])
                        nc.sync.dma_start(
                            out=rap(ud, kw0 * BS * 2 * H,
                                    [[2 * H, BS], [BS * 2 * H, G], [1, 2 * H]]),
                            in_=ucp[:, :G, :])

                    pend = None
                    for pr in range(46):
                        kw0 = 2 * pr
                        G = 2 if kw0 + 1 < KW else 1
                        psF = bcps.tile([BS, 2, 512], f32, tag="psF")
                        for g in range(G):
                            kw = kw0 + g
                            for k in range(3):
                                nc.tensor.matmul(
                                    psF[:, g, 0 : 2 * H], T1_sb[:, k, kw, :],
                                    fhs_sb[:, k, :], start=(k == 0), stop=(k == 2))
                        fsb = bcw.tile([BS, 2, 2 * H], fp16, tag="fsb")
                        nc.vector.tensor_copy(out=fsb[:, :G, :], in_=psF[:, :G, 0 : 2 * H])
                        ps1r = bcps2.tile([BS, 2, H], f32, tag="ps1r")
                        ps1i = bcps2.tile([BS, 2, H], f32, tag="ps1i")
                        nc.tensor.matmul(ps1r[:, :G, :], w1r_sb[:], fsb[:, :G, 0:H],
                                         start=True, stop=False)
                        nc.tensor.matmul(ps1r[:, :G, :], w1in_sb[:], fsb[:, :G, H : 2 * H],
                                         start=False, stop=True)
                        nc.tensor.matmul(ps1i[:, :G, :], w1i_sb[:], fsb[:, :G, 0:H],
                                         start=True, stop=False)
                        nc.tensor.matmul(ps1i[:, :G, :], w1r_sb[:], fsb[:, :G, H : 2 * H],
                                         start=False, stop=True)
                        o1r = bcw.tile([BS, 2, H], fp16, tag="o1r")
                        o1i = bcw.tile([BS, 2, H], fp16, tag="o1i")
                        nc.scalar.activation(out=o1r[:, :G, :], in_=ps1r[:, :G, :],
                                             func=FT.Relu, bias=Br[:], scale=sfv[:])
                        nc.scalar.activation(out=o1i[:, :G, :], in_=ps1i[:, :G, :],
                                             func=FT.Relu, bias=Bi[:], scale=sfv[:])
                        o1rf = o1r[:].rearrange("p g k -> p (g k)")
                        o1if = o1i[:].rearrange("p g k -> p (g k)")
                        O2t = bco.tile([90, 2, 2, 2 * BS], fp16, tag="o2sb")
                        for g in range(G):
                            ps2 = bcps3.tile([90, 2, 2 * BS], f32, tag="ps2")
                            for half in range(2):
                                sl = slice(90 * (2 * g + half), 90 * (2 * g + half + 1))
                                nc.tensor.matmul(ps2[:, half, :], o1rf[:, sl], w2cr_sb[:],
                                                 start=True, stop=False)
                                nc.tensor.matmul(ps2[:, half, :], o1if[:, sl], w2ci_sb[:],
                                                 start=False, stop=True)
                            tmp = bct.tile([90, 2, 2 * BS], f32, tag="tmp")
                            nc.vector.tensor_add(out=tmp[:], in0=ps2[:], in1=b2c_b[:90])
                            r1 = bct.tile([90, 2, 2 * BS], f32, tag="r1")
                            nc.vector.tensor_scalar(out=r1[:], in0=tmp[:], scalar1=-LAM,
                                                    scalar2=LAM, op0=OP.max, op1=OP.min)
                            nc.vector.tensor_sub(out=O2t[:, g, :, :],
                                                 in0=tmp[:], in1=r1[:])
                        # stage D (inverse H-DFT) pipelined one iteration
                        # behind B/C so the softshrink chain never stalls
                        # the in-order tensor queue
                        if pend is not None:
                            emit_D(*pend)
                        pend = (O2t, kw0, G)
                    emit_D(*pend)

                # U back to kw-partitioned SBUF, then stage E (inverse W-DFT)
                for chv in range(2):
                    k0 = 46 * chv
                    kn = min(46, KW - k0)
                    eng = nc.scalar if chv else nc.sync
                    eng.dma_start(
                        out=U_sb[k0 : k0 + kn, :, :],
                        in_=rap(ud, k0 * BS * 2 * H,
                                [[BS * 2 * H, kn], [2 * H, BS], [1, 2 * H]]))
                with (
                    tc.tile_pool(name="sec", bufs=4) as sec,
                    tc.tile_pool(name="sep", bufs=2, space="PSUM") as sep,
                ):
                    # chunk A = local rows 0-11 per dest, chunk B = rows 12-22;
                    # A2A for chunk A overlaps stage-E compute of chunk B.
                    for part in range(2):
                        rts = [(0, 4), (4, 4), (8, 4)] if part == 0 else \
                              [(12, 4), (16, 4), (20, 3)]
                        for d in range(N):
                            for roff, nr in rts:
                                h0 = HP * d + roff
                                nr = min(nr, H - h0)
                                if nr <= 0:
                                    continue
                                for wk in range(3):
                                    psE = sep.tile([120, 4, BS], f32, tag="psE")
                                    for ri in range(2):
                                        nc.tensor.matmul(
                                            psE[:, :nr, :], ifw_sb[ri][:, wk, :],
                                            U_sb[:, :, ri * H + h0 : ri * H + h0 + nr]
                                                .rearrange("p c h -> p h c"),
                                            start=(ri == 0), stop=(ri == 1))
                                    ecp = sec.tile([120, 4, BS], fp16, tag="ecp")
                                    nc.vector.tensor_copy(out=ecp[:, :nr, :],
                                                          in_=psE[:, :nr, :])
                                    if part == 0:
                                        dst, off = a2iA, d * MA + (roff * W + wk * 120) * BS
                                    else:
                                        dst, off = a2iB, d * MB + ((roff - 12) * W + wk * 120) * BS
                                    nc.scalar.dma_start(
                                        out=rap(dst, off, [[BS, 120], [W * BS, nr], [1, BS]]),
                                        in_=ecp[:, :nr, :])
                        if part == 0:
                            nc.gpsimd.collective_compute(
                                "AllToAll", OP.bypass, replica_groups=RG,
                                ins=[a2iA[:]], outs=[a2oA[:]])

            nc.gpsimd.collective_compute(
                "AllToAll", OP.bypass, replica_groups=RG, ins=[a2iB[:]], outs=[a2oB[:]])

            # ---- phase 2: h1 = F2 + ln1x + x; LN2; modulated MLP; + h1 ----
            with (
                tc.tile_pool(name="p2w", bufs=1) as p2w,
                tc.tile_pool(name="p2", bufs=4) as p2,
                tc.tile_pool(name="p2h", bufs=16) as p2h,
                tc.tile_pool(name="p2hm", bufs=2) as p2hm,
                tc.tile_pool(name="p2s", bufs=4) as p2s,
                tc.tile_pool(name="p2m", bufs=4) as p2m,
                tc.tile_pool(name="ptp", bufs=2, space="PSUM") as ptp,
                tc.tile_pool(name="php", bufs=2, space="PSUM") as php,
                tc.tile_pool(name="pop", bufs=2, space="PSUM") as pop,
            ):
                PM = mybir.MatmulPerfMode.DoubleRow
                fc1w_sb = p2w.tile([128, 6, LAT], fp8)
                nc.sync.dma_start(
                    out=fc1w_sb[:], in_=rap(fc1w_p, 0, [[LAT, 128], [128 * LAT, 6], [1, LAT]]))
                fc2w_sb = p2w.tile([128, 24, C], fp8)
                nc.sync.dma_start(
                    out=fc2w_sb[:], in_=rap(fc2w_p, 0, [[C, 128], [128 * C, 24], [1, C]]))

                def p2_prep(it):
                    T0 = it * 512
                    ln2T = p2m.tile([128, 6, 4, 128], fp8, tag="ln2T")
                    h1s = []
                    for hf in range(4):
                        t0 = T0 + 128 * hf
                        nload = max(0, min(128, TOKR - t0))
                        xt = p2.tile([128, C], f32, tag="xt2")
                        nc.sync.dma_start(out=xt[:], in_=xs[t0 : t0 + 128, :])
                        f2t = p2.tile([128, N, BS], fp16, tag="f2t")
                        l1t = p2.tile([128, N, BS], fp16, tag="l1t")
                        if nload < 128:
                            nc.vector.memset(f2t[:], 0.0)
                            nc.vector.memset(l1t[:], 0.0)
                        if nload > 0:
                            ta = min(nload, max(0, TA - t0))
                            if ta > 0:
                                nc.sync.dma_start(
                                    out=f2t[:ta],
                                    in_=rap(a2oA, t0 * BS, [[BS, ta], [MA, N], [1, BS]]))
                            if ta < nload:
                                t0b = t0 + ta - TA
                                nc.sync.dma_start(
                                    out=f2t[ta:nload],
                                    in_=rap(a2oB, t0b * BS, [[BS, nload - ta], [MB, N], [1, BS]]))
                            nc.sync.dma_start(
                                out=l1t[:nload],
                                in_=rap(a1i, t0 * BS, [[BS, nload], [TOKR * BS, N], [1, BS]]))
                        h1 = p2h.tile([128, C], f32, tag="h1")
                        nc.vector.tensor_add(out=h1[:], in0=xt[:],
                                             in1=f2t[:].rearrange("p j c -> p (j c)"))
                        nc.vector.tensor_add(out=h1[:], in0=h1[:],
                                             in1=l1t[:].rearrange("p j c -> p (j c)"))
                        h1s.append(h1)
                        st = p2s.tile([128, 2, 6], f32, tag="st2")
                        for g in range(2):
                            nc.vector.bn_stats(out=st[:, g, :], in_=h1[:, 384 * g : 384 * (g + 1)])
                        mv = p2s.tile([128, 2], f32, tag="mv2")
                        nc.vector.bn_aggr(out=mv[:], in_=st[:])
                        rstd = p2s.tile([128, 1], f32, tag="rstd2")
                        nc.scalar.activation(out=rstd[:], in_=mv[:, 1:2], func=FT.Sqrt,
                                             bias=eps_sb[:], scale=1.0)
                        nc.vector.reciprocal(out=rstd[:], in_=rstd[:])
                        # n2w/n2b folded into fc1w/fc1b host-side
                        ln2 = p2.tile([128, C], bf16, tag="ln2")
                        nc.vector.tensor_scalar(out=ln2[:], in0=h1[:], scalar1=mv[:, 0:1],
                                                scalar2=rstd[:], op0=OP.subtract, op1=OP.mult)
                        for jb in range(2):
                            pst = ptp.tile([128, 3, 128], bf16, tag="pst")
                            for jj in range(3):
                                j = 3 * jb + jj
                                nc.tensor.transpose(pst[:, jj, :],
                                                    ln2[:, 128 * j : 128 * (j + 1)],
                                                    identb[:])
                            nc.vector.tensor_copy(out=ln2T[:, 3 * jb : 3 * jb + 3, hf, :],
                                                  in_=pst[:])
                    return T0, ln2T, h1s

                def p2_mm(T0, ln2T, h1s):
                    hmidT = p2hm.tile([128, 24, 512], fp8, tag="hmidT")
                    for l in range(24):
                        psH = php.tile([128, 512], f32, tag="psH")
                        for jp in range(3):
                            nc.tensor.matmul(
                                psH[:],
                                fc1w_sb[:, 2 * jp : 2 * jp + 2, 128 * l : 128 * (l + 1)],
                                ln2T[:, 2 * jp : 2 * jp + 2, :, :]
                                    .rearrange("p j h t -> p j (h t)"),
                                start=(jp == 0), stop=(jp == 2), perf_mode=PM)
                        nc.scalar.activation(out=hmidT[:, l, :], in_=psH[:], func=FT.Gelu,
                                             bias=B1[:, l : l + 1], scale=sM16[:, l : l + 1])
                    for hf in range(4):
                        t0 = T0 + 128 * hf
                        psO = pop.tile([128, 2, 512], f32, tag="psO")
                        for lp in range(12):
                            for h2 in range(2):
                                nc.tensor.matmul(
                                    psO[:, h2, 0:384],
                                    hmidT[:, 2 * lp : 2 * lp + 2, 128 * hf : 128 * (hf + 1)],
                                    fc2w_sb[:, 2 * lp : 2 * lp + 2, 384 * h2 : 384 * (h2 + 1)],
                                    start=(lp == 0), stop=(lp == 11), perf_mode=PM)
                        ot = p2.tile([128, C], f32, tag="ot")
                        nc.vector.scalar_tensor_tensor(
                            out=ot[:].rearrange("p (a b) -> p a b", a=2),
                            in0=psO[:, :, 0:384], scalar=1.0 / 16.0,
                            in1=h1s[hf][:].rearrange("p (a b) -> p a b", a=2),
                            op0=OP.mult, op1=OP.add)
                        nc.gpsimd.tensor_add(out=ot[:], in0=ot[:], in1=fc2b_b[:])
                        nc.sync.dma_start(out=out_p[t0 : t0 + 128, :], in_=ot[:])

                # depth-4 software pipeline, preps paired so scalar runs
                # [Sqrt x8, Gelu x48] per pair (one act-table swap per tile)
                sq = [p2_prep(i) for i in range(min(4, NT2))]
                for itp in range(0, NT2, 2):
                    p2_mm(*sq[itp])
                    if itp + 1 < NT2:
                        p2_mm(*sq[itp + 1])
                    for j in (itp + 4, itp + 5):
                        if j < NT2:
                            sq.append(p2_prep(j))

    nc.compile()
    return nc


_NC = None


def _get_nc():
    global _NC
    if _NC is None:
        _NC = _build()
    return _NC


def _dft_mats():
    w = np.arange(W); kw = np.arange(KW)
    ang = 2 * np.pi * np.outer(w, kw) / W
    fwr = (np.cos(ang) / np.sqrt(W)).astype(np.float32)
    fwi = (-np.sin(ang) / np.sqrt(W)).astype(np.float32)
    kh = np.arange(H); h = np.arange(H)
    angh = 2 * np.pi * np.outer(kh, h) / H        # [kh, h]
    fhr = np.cos(angh) / np.sqrt(H)
    fhi = -np.sin(angh) / np.sqrt(H)
    fhs = np.zeros((2 * H, 2 * H))
    fhs[:H, :H] = fhr.T; fhs[:H, H:] = fhi.T
    fhs[H:, :H] = -fhi.T; fhs[H:, H:] = fhr.T
    ci = np.cos(angh) / np.sqrt(H)                # [kh, h] for inverse
    si = np.sin(angh) / np.sqrt(H)
    ifhs = np.zeros((2 * H, 2 * H))
    ifhs[:H, :H] = ci; ifhs[:H, H:] = si
    ifhs[H:, :H] = -si; ifhs[H:, H:] = ci
    ckw = np.where(kw == 0, 1.0, 2.0)
    angi = 2 * np.pi * np.outer(kw, np.arange(W)) / W    # [kw, w]
    ifwr = (ckw[:, None] * np.cos(angi) / np.sqrt(W)).astype(np.float32)
    ifwi = (-ckw[:, None] * np.sin(angi) / np.sqrt(W)).astype(np.float32)
    return fwr, fwi, fhs.astype(np.float32), ifhs.astype(np.float32), ifwr, ifwi


def _prepare_in_maps(x, mod_embed, n1w, n1b, n2w, n2b, w1, b1, w2, b2,
                     fs_w0, fs_b0, fs_w1, fs_b1, fc1w, fc1b, fc2w, fc2b,
                     ms_w0, ms_b0, ms_w1, ms_b1):
    f = np.asarray
    x = f(x, dtype=np.float32)
    grid = x.reshape(H, W, C)
    fwr, fwi, fhs, ifhs, ifwr, ifwi = _dft_mats()
    bf = ml_dtypes.bfloat16

    in_maps = []
    for b in range(N):
        r0, r1 = HP * b, min(HP * (b + 1), H)
        xsb = np.zeros((TOKP, C), np.float32)
        xsb[: (r1 - r0) * W] = grid[r0:r1].reshape(-1, C)
        sl = slice(BS * b, BS * (b + 1))
        w2r = f(w2[0, b], np.float32); w2i = f(w2[1, b], np.float32)
        im = {
            "xs": xsb,
            "modT": np.repeat(f(mod_embed, np.float32).reshape(MODD, 1), 2, axis=1).copy(),
            "n1w": f(n1w, np.float32), "n1b": f(n1b, np.float32),
            "n2w": f(n2w, np.float32), "n2b": f(n2b, np.float32),
            "fwr": fwr.astype(np.float16), "fwi": fwi.astype(np.float16),
            "fhs": fhs.astype(np.float16), "ifhs": ifhs.astype(np.float16),
            "ifwr": ifwr.astype(np.float16), "ifwi": ifwi.astype(np.float16),
            "w1r": f(w1[0, b], np.float16).copy(),
            "w1i": f(w1[1, b], np.float16).copy(),
            "w1in": (-f(w1[1, b], np.float16)).copy(),
            "w2cr": np.concatenate([w2r, w2i], axis=1).astype(np.float16),
            "w2ci": np.concatenate([-w2i, w2r], axis=1).astype(np.float16),
            "b1r": f(b1[0, b], np.float32).reshape(BS, 1).copy(),
            "b1i": f(b1[1, b], np.float32).reshape(BS, 1).copy(),
            "b2c": np.concatenate([f(b2[0, b], np.float32), f(b2[1, b], np.float32)]),
            "fs_w0": f(fs_w0, np.float32),
            "fs_b0T": f(fs_b0, np.float32).reshape(12, 128).T.copy(),
            "fs_w1s": np.concatenate(
                [f(fs_w1, np.float32)[:, sl], f(fs_w1, np.float32)[:, C + BS * b : C + BS * (b + 1)]],
                axis=1),
            "fs_b1s": np.concatenate(
                [f(fs_b1, np.float32)[sl], f(fs_b1, np.float32)[C + BS * b : C + BS * (b + 1)]]
            ).reshape(1, -1),
            "ms_w0": f(ms_w0, np.float32),
            "ms_b0T": f(ms_b0, np.float32).reshape(48, 128).T.copy(),
            "ms_w1s": f(ms_w1, np.float32)[:, C * b : C * (b + 1)].astype(bf),
            "ms_b1s": f(ms_b1, np.float32)[C * b : C * (b + 1)].reshape(1, -1),
            "fc1w": (16.0 * f(n2w, np.float32)[:, None] * f(fc1w, np.float32)
                     ).astype(ml_dtypes.float8_e4m3),
            "fc1bT": (f(fc1b, np.float32)
                      + f(n2b, np.float32) @ f(fc1w, np.float32)
                      ).reshape(24, 128).T.copy(),
            "fc2w": (16.0 * f(fc2w, np.float32)).astype(ml_dtypes.float8_e4m3),
            "fc2b": f(fc2b, np.float32),
        }
        in_maps.append(im)
    return in_maps


def kernel(x, mod_embed, n1w, n1b, n2w, n2b, w1, b1, w2, b2,
           fs_w0, fs_b0, fs_w1, fs_b1, fc1w, fc1b, fc2w, fc2b,
           ms_w0, ms_b0, ms_w1, ms_b1):
    nc = _get_nc()
    in_maps = _prepare_in_maps(
        x, mod_embed, n1w, n1b, n2w, n2b, w1, b1, w2, b2,
        fs_w0, fs_b0, fs_w1, fs_b1, fc1w, fc1b, fc2w, fc2b,
        ms_w0, ms_b0, ms_w1, ms_b1)

    res = run_bass_kernel_spmd(nc, in_maps, core_ids=list(range(N)))
    globals()["last_results"] = res
    out = np.zeros((H, W, C), np.float32)
    for b in range(N):
        r0, r1 = HP * b, min(HP * (b + 1), H)
        out[r0:r1] = res.results[b]["out"][: (r1 - r0) * W].reshape(r1 - r0, W, C)
    return out.reshape(1, H, W, C)



# revision 46
# speedup vs baseline: 1.2063x; 1.0170x over previous
"""AFNO block kernel for 8 Trainium2 NeuronCores.

Sharding: token-shard (H rows, 23 per core padded) for LN/MLP phases;
AllToAll to channel-shard (core i = spectral block i, 96 channels) for the
2D-FFT filter, computed as matmuls against precomputed DFT matrices;
AllToAll back; small AllGather for the column-sharded 6144x6144 scale-shift
MLP weight.
"""

import os
import numpy as np
import ml_dtypes

import concourse.bass as bass
import concourse.bacc as bacc
import concourse.mybir as mybir
import concourse.tile as tile
from concourse.bass_utils import run_bass_kernel_spmd
from concourse.masks import make_identity

f32 = mybir.dt.float32
f32r = mybir.dt.float32r
bf16 = mybir.dt.bfloat16
fp16 = mybir.dt.float16
fp8 = mybir.dt.float8e4
FT = mybir.ActivationFunctionType
OP = mybir.AluOpType

H, W, C = 180, 360, 768
NB, BS, KW = 8, 96, 91
HP = 23                 # rows per shard (8*23 = 184 >= 180)
TOKR = HP * W           # 8280 real token slots per shard
NT2 = 17                # phase-2 tiles of 512
TOKP = NT2 * 512        # 8704 padded tokens per shard
MODD, LAT, LAT2 = 64, 3072, 6144
LAM = 0.01
EPS = 1e-5
N = 8


def rap(t, offset, dims):
    a = t[:] if not isinstance(t, bass.AP) else t
    return bass.AP(tensor=a.tensor, offset=a.offset + offset, ap=[list(d) for d in dims])


def _build():
    nc = bacc.Bacc("TRN2", target_bir_lowering=False, debug=False, num_devices=N)

    def P(name, shp, dt=f32):
        return nc.declare_dram_parameter(name, list(shp), dt, isOutput=False)

    xs = P("xs", [TOKP, C])
    modT = P("modT", [MODD, 2])
    n1w = P("n1w", [C]); n1b = P("n1b", [C])
    n2w = P("n2w", [C]); n2b = P("n2b", [C])
    fwr_p = P("fwr", [W, KW], fp16); fwi_p = P("fwi", [W, KW], fp16)
    fhs_p = P("fhs", [2 * H, 2 * H], fp16)
    ifhs_p = P("ifhs", [2 * H, 2 * H], fp16)
    ifwr_p = P("ifwr", [KW, W], fp16); ifwi_p = P("ifwi", [KW, W], fp16)
    w1r_p = P("w1r", [BS, BS], fp16); w1i_p = P("w1i", [BS, BS], fp16)
    w1in_p = P("w1in", [BS, BS], fp16)
    w2cr_p = P("w2cr", [BS, 2 * BS], fp16)   # [W2r | W2i]
    w2ci_p = P("w2ci", [BS, 2 * BS], fp16)   # [-W2i | W2r]
    b1r_p = P("b1r", [BS, 1]); b1i_p = P("b1i", [BS, 1])
    b2c_p = P("b2c", [2 * BS])               # concat(b2r, b2i)
    fs_w0_p = P("fs_w0", [MODD, 2 * C])
    fs_b0T_p = P("fs_b0T", [128, 12])
    fs_w1s_p = P("fs_w1s", [2 * C, 2 * BS])
    fs_b1s_p = P("fs_b1s", [1, 2 * BS])
    ms_w0_p = P("ms_w0", [MODD, LAT2])
    ms_b0T_p = P("ms_b0T", [128, 48])
    ms_w1s_p = P("ms_w1s", [LAT2, C], bf16)
    ms_b1s_p = P("ms_b1s", [1, C])
    fc1w_p = P("fc1w", [C, LAT], fp8)
    fc1bT_p = P("fc1bT", [128, 24])
    fc2w_p = P("fc2w", [LAT, C], fp8)
    fc2b_p = P("fc2b", [C])
    out_p = nc.declare_dram_parameter("out", [TOKP, C], f32, isOutput=True)
    DBG = False

    # internal DRAM
    a1i = nc.dram_tensor("a1i", [N, TOKR * BS], fp16)
    a1o = nc.dram_tensor("a1o", [N, TOKR * BS], fp16)
    MA = 12 * W * BS        # chunk A: local rows 0-11 per dest
    MB = 11 * W * BS        # chunk B: local rows 12-22
    TA = 12 * W             # tokens per dest covered by chunk A
    a2iA = nc.dram_tensor("a2iA", [N, MA], fp16)
    a2oA = nc.dram_tensor("a2oA", [N, MA], fp16)
    a2iB = nc.dram_tensor("a2iB", [N, MB], fp16)
    a2oB = nc.dram_tensor("a2oB", [N, MB], fp16)
    t1d = nc.dram_tensor("t1d", [2 * H, KW, BS], fp16)  # [hri][kw][c]
    ud = nc.dram_tensor("ud", [KW, BS, 2 * H], fp16)
    sfd = nc.dram_tensor("sfd", [1, 2 * BS], f32)
    agi = nc.dram_tensor("agi", [1, C], f32)
    ago = nc.dram_tensor("ago", [N, C], f32)

    RG = [list(range(N))]

    with tile.TileContext(nc) as tc:
        with (
            tc.tile_pool(name="const", bufs=1) as cpool,
            tc.tile_pool(name="ssb", bufs=1) as ssb,
        ):
            # ---- broadcast constants ----
            def bcast(p, n, name):
                t = cpool.tile([128, n], f32, tag=name)
                nc.sync.dma_start(out=t[:], in_=rap(p, 0, [[0, 128], [1, n]]))
                return t

            n1w_b = bcast(n1w, C, "n1w"); n1b_b = bcast(n1b, C, "n1b")
            n2w_b = bcast(n2w, C, "n2w"); n2b_b = bcast(n2b, C, "n2b")
            fc2b_b = bcast(fc2b_p, C, "fc2b")
            b2c_b = cpool.tile([128, 2, 2 * BS], f32, tag="b2c")
            for bh in range(2):
                nc.sync.dma_start(out=b2c_b[:, bh, :],
                                  in_=rap(b2c_p, 0, [[0, 128], [1, 2 * BS]]))
            eps_sb = cpool.tile([128, 1], f32, tag="eps")
            nc.vector.memset(eps_sb[:], EPS)
            nlam_sb = cpool.tile([128, 1], f32, tag="nlam")
            nc.vector.memset(nlam_sb[:], -LAM)
            ident = cpool.tile([128, 128], f32, tag="ident")
            make_identity(nc, ident[:])
            identb = cpool.tile([128, 128], bf16, tag="identb")
            nc.vector.tensor_copy(out=identb[:], in_=ident[:])


            # ---- phase 0: LN1 + scatter into A2A-1 send buffer ----
            with (
                tc.tile_pool(name="p0", bufs=6) as p0,
                tc.tile_pool(name="p0s", bufs=8) as p0s,
            ):
                for it in range(65):
                    t0 = it * 128
                    nrow = min(128, TOKR - t0)
                    xt = p0.tile([128, C], f32, tag="xt")
                    nc.sync.dma_start(out=xt[:], in_=xs[t0 : t0 + 128, :])
                    st = p0s.tile([128, 2, 6], f32, tag="st")
                    for g in range(2):
                        nc.vector.bn_stats(out=st[:, g, :], in_=xt[:, 384 * g : 384 * (g + 1)])
                    mv = p0s.tile([128, 2], f32, tag="mv")
                    nc.vector.bn_aggr(out=mv[:], in_=st[:])
                    rstd = p0s.tile([128, 1], f32, tag="rstd")
                    nc.scalar.activation(out=rstd[:], in_=mv[:, 1:2], func=FT.Sqrt,
                                         bias=eps_sb[:], scale=1.0)
                    nc.vector.reciprocal(out=rstd[:], in_=rstd[:])
                    ln = p0.tile([128, C], f32, tag="ln")
                    nc.vector.tensor_scalar(out=ln[:], in0=xt[:], scalar1=mv[:, 0:1],
                                            scalar2=rstd[:], op0=OP.subtract, op1=OP.mult)
                    aeng = nc.vector
                    aeng.tensor_mul(out=ln[:], in0=ln[:], in1=n1w_b[:])
                    lnh = p0.tile([128, C], fp16, tag="lnh")
                    aeng.tensor_add(out=lnh[:], in0=ln[:], in1=n1b_b[:])
                    nc.scalar.dma_start(
                        out=rap(a1i, t0 * BS, [[BS, nrow], [TOKR * BS, N], [1, BS]]),
                        in_=lnh[:nrow].rearrange("p (j c) -> p j c", j=N),
                    )

            nc.gpsimd.collective_compute(
                "AllToAll", OP.bypass, replica_groups=RG, ins=[a1i[:]], outs=[a1o[:]])

            # ---- scale-shift MLPs (overlap with A2A1 window) ----
            ss_ctx = tc.tile_pool(name="ssw", bufs=1)
            ssw = ss_ctx.__enter__()
            ssp_ctx = tc.tile_pool(name="ssp", bufs=1, space="PSUM")
            ssp = ssp_ctx.__enter__()
            modT_sb = ssw.tile([MODD, 2], f32r)
            nc.sync.dma_start(out=modT_sb[:], in_=modT[:].bitcast(f32r))
            fs_w0_sb = ssw.tile([MODD, 2 * C], f32r)
            nc.sync.dma_start(out=fs_w0_sb[:], in_=fs_w0_p[:].bitcast(f32r))
            fs_b0T_sb = ssw.tile([128, 12], f32)
            nc.sync.dma_start(out=fs_b0T_sb[:], in_=fs_b0T_p[:])
            e0T = ssw.tile([128, 12], f32r)
            for j in range(12):
                pt = ssp.tile([128, 2], f32, tag="ss1")
                nc.tensor.matmul(pt[:], fs_w0_sb[:, 128 * j : 128 * (j + 1)],
                                 modT_sb[:], start=True, stop=True)
                nc.scalar.activation(out=e0T[:, j : j + 1], in_=pt[:, 0:1], func=FT.Gelu,
                                     bias=fs_b0T_sb[:, j : j + 1], scale=1.0)
            fs_w1s_sb = ssw.tile([128, 12, 2 * BS], f32r)
            nc.sync.dma_start(
                out=fs_w1s_sb[:],
                in_=rap(fs_w1s_p, 0, [[2 * BS, 128], [128 * 2 * BS, 12], [1, 2 * BS]]).bitcast(f32r),
            )
            fs_b1s_sb = ssw.tile([1, 2 * BS], f32)
            nc.sync.dma_start(out=fs_b1s_sb[:], in_=fs_b1s_p[:])
            p2 = ssp.tile([1, 2 * BS], f32, tag="ss2")
            for j in range(12):
                nc.tensor.matmul(p2[:], e0T[:, j : j + 1], fs_w1s_sb[:, j, :],
                                 start=(j == 0), stop=(j == 11))
            sfo = ssw.tile([1, 2 * BS], f32)
            nc.vector.tensor_add(out=sfo[:], in0=p2[:], in1=fs_b1s_sb[:])
            nc.sync.dma_start(out=sfd[:], in_=sfo[:])
            sfT = ssw.tile([BS, 2], f32)
            nc.sync.dma_start(out=sfT[:], in_=rap(sfd, 0, [[1, BS], [BS, 2]]))
            sfv = ssb.tile([BS, 1], f32)
            nc.vector.tensor_scalar_add(out=sfv[:], in0=sfT[:, 0:1], scalar1=1.0)
            b1r_sb = ssw.tile([BS, 1], f32)
            nc.sync.dma_start(out=b1r_sb[:], in_=b1r_p[:])
            b1i_sb = ssw.tile([BS, 1], f32)
            nc.sync.dma_start(out=b1i_sb[:], in_=b1i_p[:])
            Br = ssb.tile([BS, 1], f32)
            nc.vector.tensor_mul(out=Br[:], in0=b1r_sb[:], in1=sfv[:])
            nc.vector.tensor_add(out=Br[:], in0=Br[:], in1=sfT[:, 1:2])
            Bi = ssb.tile([BS, 1], f32)
            nc.vector.tensor_mul(out=Bi[:], in0=b1i_sb[:], in1=sfv[:])
            nc.vector.tensor_add(out=Bi[:], in0=Bi[:], in1=sfT[:, 1:2])

            # ms MLP: e1T then column-sharded 6144->768, AllGather
            ms_w0_sb = ssw.tile([MODD, LAT2], f32r)
            nc.gpsimd.dma_start(out=ms_w0_sb[:], in_=ms_w0_p[:].bitcast(f32r))
            ms_b0T_sb = ssw.tile([128, 48], f32)
            nc.sync.dma_start(out=ms_b0T_sb[:], in_=ms_b0T_p[:])
            e1T = ssw.tile([128, 48], bf16)
            for j in range(48):
                pt = ssp.tile([128, 2], f32, tag="ss1")
                nc.tensor.matmul(pt[:], ms_w0_sb[:, 128 * j : 128 * (j + 1)],
                                 modT_sb[:], start=True, stop=True)
                nc.scalar.activation(out=e1T[:, j : j + 1], in_=pt[:, 0:1], func=FT.Gelu,
                                     bias=ms_b0T_sb[:, j : j + 1], scale=1.0)
            p3 = ssp.tile([1, 2, 512], f32, tag="ss3")
            with tc.tile_pool(name="msw", bufs=3) as mswp:
                for j in range(48):
                    wt = mswp.tile([128, C], bf16)
                    nc.gpsimd.dma_start(
                        out=wt[:], in_=ms_w1s_p[128 * j : 128 * (j + 1), :])
                    for h2 in range(2):
                        nc.tensor.matmul(
                            p3[:, h2, 0:384], e1T[:, j : j + 1],
                            wt[:, 384 * h2 : 384 * (h2 + 1)],
                            start=(j == 0), stop=(j == 47))
            ms_b1s_sb = ssw.tile([1, C], f32)
            nc.sync.dma_start(out=ms_b1s_sb[:], in_=ms_b1s_p[:])
            mso = ssw.tile([1, C], f32)
            nc.vector.tensor_add(out=mso[:].rearrange("p (a b) -> p a b", a=2),
                                 in0=p3[:, :, 0:384],
                                 in1=ms_b1s_sb[:].rearrange("p (a b) -> p a b", a=2))
            nc.sync.dma_start(out=agi[:], in_=mso[:])
            nc.gpsimd.collective_compute(
                "AllGather", OP.bypass, replica_groups=RG, ins=[agi[:]], outs=[ago[:]])
            sM = ssb.tile([128, 24], f32)
            nc.sync.dma_start(out=sM[:], in_=rap(ago, 0, [[1, 128], [128, 24]]))
            nc.vector.tensor_scalar_add(out=sM[:], in0=sM[:], scalar1=1.0)
            tM = ssb.tile([128, 24], f32)
            nc.sync.dma_start(out=tM[:], in_=rap(ago, LAT, [[1, 128], [128, 24]]))
            fc1bT_sb = ssw.tile([128, 24], f32)
            nc.sync.dma_start(out=fc1bT_sb[:], in_=fc1bT_p[:])
            B1 = ssb.tile([128, 24], f32)
            nc.vector.tensor_mul(out=B1[:], in0=fc1bT_sb[:], in1=sM[:])
            nc.vector.tensor_add(out=B1[:], in0=B1[:], in1=tM[:])
            sM16 = ssb.tile([128, 24], f32)
            nc.vector.tensor_scalar_mul(out=sM16[:], in0=sM[:], scalar1=1.0 / 16.0)

            ssp_ctx.__exit__(None, None, None)
            ss_ctx.__exit__(None, None, None)

            # ---- phase 1 stage A: W-DFT  (X[h,w,c] -> t1d[kw,ri,h,c]) ----
            with (
                tc.tile_pool(name="sa", bufs=1) as sa,
                tc.tile_pool(name="sac", bufs=3) as sac,
                tc.tile_pool(name="sap", bufs=2, space="PSUM") as sap,
            ):
                fw_sb = []
                for ri, p in enumerate([fwr_p, fwi_p]):
                    t = sa.tile([120, 3, KW], fp16, tag=f"fw{ri}")
                    nc.sync.dma_start(
                        out=t[:], in_=rap(p, 0, [[KW, 120], [120 * KW, 3], [1, KW]]))
                    fw_sb.append(t)
                X_sb = sa.tile([120, 3, H, BS], fp16, tag="xsb")
                for ch in range(4):
                    for k in range(3):
                        eng = nc.scalar if (ch + k) % 2 else nc.sync
                        eng.dma_start(
                            out=X_sb[:, k, 45 * ch : 45 * (ch + 1), :],
                            in_=rap(a1o, (45 * ch * W + 120 * k) * BS,
                                    [[BS, 120], [W * BS, 45], [1, BS]]))
                for hs in range(36):
                    hh0 = 5 * hs
                    cp = sac.tile([KW, 2, 5, BS], fp16, tag="cpa")
                    for ri in range(2):
                        ps = sap.tile([KW, 5, BS], f32, tag="pa")
                        for k in range(3):
                            nc.tensor.matmul(ps[:], fw_sb[ri][:, k, :],
                                             X_sb[:, k, hh0 : hh0 + 5, :],
                                             start=(k == 0), stop=(k == 2))
                        nc.vector.tensor_copy(out=cp[:, ri, :, :], in_=ps[:])
                    for ri in range(2):
                        eng = nc.scalar if ri else nc.sync
                        eng.dma_start(
                            out=rap(t1d, (ri * H + hh0) * KW * BS,
                                    [[BS, KW], [KW * BS, 5], [1, BS]]),
                            in_=cp[:, ri, :, :])

            # ---- stages B+C+D fused in SBUF, then E ----
            with tc.tile_pool(name="fb", bufs=1) as fb:
                # forward H-DFT stacked matrix chunked as 3x120 (vs 4x90):
                # two fewer serialized matmuls per kw pair in stage B
                fhs_sb = fb.tile([120, 3, 2 * H], fp16)
                nc.sync.dma_start(
                    out=fhs_sb[:],
                    in_=rap(fhs_p, 0, [[2 * H, 120], [120 * 2 * H, 3], [1, 2 * H]]))
                ifhs_sb = fb.tile([90, 4, 2 * H], fp16)
                nc.sync.dma_start(
                    out=ifhs_sb[:],
                    in_=rap(ifhs_p, 0, [[2 * H, 90], [90 * 2 * H, 4], [1, 2 * H]]))
                w1r_sb = fb.tile([BS, BS], fp16)
                nc.sync.dma_start(out=w1r_sb[:], in_=w1r_p[:])
                w1i_sb = fb.tile([BS, BS], fp16)
                nc.sync.dma_start(out=w1i_sb[:], in_=w1i_p[:])
                w1in_sb = fb.tile([BS, BS], fp16)
                nc.sync.dma_start(out=w1in_sb[:], in_=w1in_p[:])
                w2cr_sb = fb.tile([BS, 2 * BS], fp16)
                nc.sync.dma_start(out=w2cr_sb[:], in_=w2cr_p[:])
                w2ci_sb = fb.tile([BS, 2 * BS], fp16)
                nc.sync.dma_start(out=w2ci_sb[:], in_=w2ci_p[:])
                ifw_sb = []
                for ri, p in enumerate([ifwr_p, ifwi_p]):
                    t = fb.tile([KW, 3, 120], fp16, tag=f"ifw{ri}")
                    nc.sync.dma_start(
                        out=t[:], in_=rap(p, 0, [[W, KW], [120, 3], [1, 120]]))
                    ifw_sb.append(t)
                # T1 resident: [hri(120), chunk(3), kw, c] with hri = ri*H + h
                T1_sb = fb.tile([120, 3, KW, BS], fp16, tag="t1sb")
                for k in range(3):
                    eng = nc.scalar if k == 1 else nc.sync
                    eng.dma_start(
                        out=T1_sb[:, k, :, :],
                        in_=rap(t1d, 120 * k * KW * BS,
                                [[KW * BS, 120], [BS, KW], [1, BS]]))
                U_sb = fb.tile([KW, BS, 2 * H], fp16, tag="usb")

                with (
                    tc.tile_pool(name="bcw", bufs=4) as bcw,
                    tc.tile_pool(name="bct", bufs=4) as bct,
                    tc.tile_pool(name="bco", bufs=3) as bco,
                    tc.tile_pool(name="bcp1", bufs=1, space="PSUM") as bcps,
                    tc.tile_pool(name="bcp2", bufs=1, space="PSUM") as bcps2,
                    tc.tile_pool(name="bcp3", bufs=2, space="PSUM") as bcps3,
                    tc.tile_pool(name="bcp4", bufs=1, space="PSUM") as bcps4,
                ):
                    def emit_D(O2t, kw0, G):
                        psU = bcps4.tile([BS, 2, 512], f32, tag="psU")
                        for g in range(G):
                            for q in range(4):
                                ri, half = q // 2, q % 2
                                nc.tensor.matmul(
                                    psU[:, g, 0 : 2 * H],
                                    O2t[:, g, half, ri * BS : (ri + 1) * BS],
                                    ifhs_sb[:, q, :], start=(q == 0), stop=(q == 3))
                        ucp = bcw.tile([BS, 2, 2 * H], fp16, tag="ucp")
                        nc.vector.tensor_copy(out=ucp[:, :G, :], in_=psU[:, :G, 0 : 2 * H])
                        nc.sync.dma_start(
                            out=rap(ud, kw0 * BS * 2 * H,
                                    [[2 * H, BS], [BS * 2 * H, G], [1, 2 * H]]),
                            in_=ucp[:, :G, :])
                        # stream this pair back into kw-partitioned U_sb on
                        # the idle gpsimd DMA queue (hides the pre-stage-E
                        # bulk reload)
                        nc.gpsimd.dma_start(
                            out=U_sb[kw0 : kw0 + G, :, :],
                            in_=rap(ud, kw0 * BS * 2 * H,
                                    [[BS * 2 * H, G], [2 * H, BS], [1, 2 * H]]))

                    pend = None
                    for pr in range(46):
                        kw0 = 2 * pr
                        G = 2 if kw0 + 1 < KW else 1
                        psF = bcps.tile([BS, 2, 512], f32, tag="psF")
                        for g in range(G):
                            kw = kw0 + g
                            for k in range(3):
                                nc.tensor.matmul(
                                    psF[:, g, 0 : 2 * H], T1_sb[:, k, kw, :],
                                    fhs_sb[:, k, :], start=(k == 0), stop=(k == 2))
                        fsb = bcw.tile([BS, 2, 2 * H], fp16, tag="fsb")
                        nc.vector.tensor_copy(out=fsb[:, :G, :], in_=psF[:, :G, 0 : 2 * H])
                        ps1r = bcps2.tile([BS, 2, H], f32, tag="ps1r")
                        ps1i = bcps2.tile([BS, 2, H], f32, tag="ps1i")
                        nc.tensor.matmul(ps1r[:, :G, :], w1r_sb[:], fsb[:, :G, 0:H],
                                         start=True, stop=False)
                        nc.tensor.matmul(ps1r[:, :G, :], w1in_sb[:], fsb[:, :G, H : 2 * H],
                                         start=False, stop=True)
                        nc.tensor.matmul(ps1i[:, :G, :], w1i_sb[:], fsb[:, :G, 0:H],
                                         start=True, stop=False)
                        nc.tensor.matmul(ps1i[:, :G, :], w1r_sb[:], fsb[:, :G, H : 2 * H],
                                         start=False, stop=True)
                        o1r = bcw.tile([BS, 2, H], fp16, tag="o1r")
                        o1i = bcw.tile([BS, 2, H], fp16, tag="o1i")
                        nc.scalar.activation(out=o1r[:, :G, :], in_=ps1r[:, :G, :],
                                             func=FT.Relu, bias=Br[:], scale=sfv[:])
                        nc.scalar.activation(out=o1i[:, :G, :], in_=ps1i[:, :G, :],
                                             func=FT.Relu, bias=Bi[:], scale=sfv[:])
                        o1rf = o1r[:].rearrange("p g k -> p (g k)")
                        o1if = o1i[:].rearrange("p g k -> p (g k)")
                        O2t = bco.tile([90, 2, 2, 2 * BS], fp16, tag="o2sb")
                        for g in range(G):
                            ps2 = bcps3.tile([90, 2, 2 * BS], f32, tag="ps2")
                            for half in range(2):
                                sl = slice(90 * (2 * g + half), 90 * (2 * g + half + 1))
                                nc.tensor.matmul(ps2[:, half, :], o1rf[:, sl], w2cr_sb[:],
                                                 start=True, stop=False)
                                nc.tensor.matmul(ps2[:, half, :], o1if[:, sl], w2ci_sb[:],
                                                 start=False, stop=True)
                            tmp = bct.tile([90, 2, 2 * BS], f32, tag="tmp")
                            nc.vector.tensor_add(out=tmp[:], in0=ps2[:], in1=b2c_b[:90])
                            r1 = bct.tile([90, 2, 2 * BS], f32, tag="r1")
                            nc.vector.tensor_scalar(out=r1[:], in0=tmp[:], scalar1=-LAM,
                                                    scalar2=LAM, op0=OP.max, op1=OP.min)
                            nc.vector.tensor_sub(out=O2t[:, g, :, :],
                                                 in0=tmp[:], in1=r1[:])
                        # stage D (inverse H-DFT) pipelined one iteration
                        # behind B/C so the softshrink chain never stalls
                        # the in-order tensor queue
                        if pend is not None:
                            emit_D(*pend)
                        pend = (O2t, kw0, G)
                    emit_D(*pend)

                # stage E (inverse W-DFT); U_sb streamed in by emit_D above
                with (
                    tc.tile_pool(name="sec", bufs=4) as sec,
                    tc.tile_pool(name="sep", bufs=2, space="PSUM") as sep,
                ):
                    # chunk A = local rows 0-11 per dest, chunk B = rows 12-22;
                    # A2A for chunk A overlaps stage-E compute of chunk B.
                    for part in range(2):
                        rts = [(0, 4), (4, 4), (8, 4)] if part == 0 else \
                              [(12, 4), (16, 4), (20, 3)]
                        for d in range(N):
                            for roff, nr in rts:
                                h0 = HP * d + roff
                                nr = min(nr, H - h0)
                                if nr <= 0:
                                    continue
                                for wk in range(3):
                                    psE = sep.tile([120, 4, BS], f32, tag="psE")
                                    for ri in range(2):
                                        nc.tensor.matmul(
                                            psE[:, :nr, :], ifw_sb[ri][:, wk, :],
                                            U_sb[:, :, ri * H + h0 : ri * H + h0 + nr]
                                                .rearrange("p c h -> p h c"),
                                            start=(ri == 0), stop=(ri == 1))
                                    ecp = sec.tile([120, 4, BS], fp16, tag="ecp")
                                    nc.vector.tensor_copy(out=ecp[:, :nr, :],
                                                          in_=psE[:, :nr, :])
                                    if part == 0:
                                        dst, off = a2iA, d * MA + (roff * W + wk * 120) * BS
                                    else:
                                        dst, off = a2iB, d * MB + ((roff - 12) * W + wk * 120) * BS
                                    nc.scalar.dma_start(
                                        out=rap(dst, off, [[BS, 120], [W * BS, nr], [1, BS]]),
                                        in_=ecp[:, :nr, :])
                        if part == 0:
                            nc.gpsimd.collective_compute(
                                "AllToAll", OP.bypass, replica_groups=RG,
                                ins=[a2iA[:]], outs=[a2oA[:]])

            nc.gpsimd.collective_compute(
                "AllToAll", OP.bypass, replica_groups=RG, ins=[a2iB[:]], outs=[a2oB[:]])

            # ---- phase 2: h1 = F2 + ln1x + x; LN2; modulated MLP; + h1 ----
            with (
                tc.tile_pool(name="p2w", bufs=1) as p2w,
                tc.tile_pool(name="p2", bufs=4) as p2,
                tc.tile_pool(name="p2h", bufs=16) as p2h,
                tc.tile_pool(name="p2hm", bufs=2) as p2hm,
                tc.tile_pool(name="p2s", bufs=4) as p2s,
                tc.tile_pool(name="p2m", bufs=4) as p2m,
                tc.tile_pool(name="ptp", bufs=2, space="PSUM") as ptp,
                tc.tile_pool(name="php", bufs=2, space="PSUM") as php,
                tc.tile_pool(name="pop", bufs=2, space="PSUM") as pop,
            ):
                PM = mybir.MatmulPerfMode.DoubleRow
                fc1w_sb = p2w.tile([128, 6, LAT], fp8)
                nc.sync.dma_start(
                    out=fc1w_sb[:], in_=rap(fc1w_p, 0, [[LAT, 128], [128 * LAT, 6], [1, LAT]]))
                fc2w_sb = p2w.tile([128, 24, C], fp8)
                nc.sync.dma_start(
                    out=fc2w_sb[:], in_=rap(fc2w_p, 0, [[C, 128], [128 * C, 24], [1, C]]))

                def p2_prep(it):
                    T0 = it * 512
                    ln2T = p2m.tile([128, 6, 4, 128], fp8, tag="ln2T")
                    h1s = []
                    for hf in range(4):
                        t0 = T0 + 128 * hf
                        nload = max(0, min(128, TOKR - t0))
                        xt = p2.tile([128, C], f32, tag="xt2")
                        nc.sync.dma_start(out=xt[:], in_=xs[t0 : t0 + 128, :])
                        f2t = p2.tile([128, N, BS], fp16, tag="f2t")
                        l1t = p2.tile([128, N, BS], fp16, tag="l1t")
                        if nload < 128:
                            nc.vector.memset(f2t[:], 0.0)
                            nc.vector.memset(l1t[:], 0.0)
                        if nload > 0:
                            ta = min(nload, max(0, TA - t0))
                            if ta > 0:
                                nc.sync.dma_start(
                                    out=f2t[:ta],
                                    in_=rap(a2oA, t0 * BS, [[BS, ta], [MA, N], [1, BS]]))
                            if ta < nload:
                                t0b = t0 + ta - TA
                                nc.sync.dma_start(
                                    out=f2t[ta:nload],
                                    in_=rap(a2oB, t0b * BS, [[BS, nload - ta], [MB, N], [1, BS]]))
                            nc.sync.dma_start(
                                out=l1t[:nload],
                                in_=rap(a1i, t0 * BS, [[BS, nload], [TOKR * BS, N], [1, BS]]))
                        h1 = p2h.tile([128, C], f32, tag="h1")
                        nc.vector.tensor_add(out=h1[:], in0=xt[:],
                                             in1=f2t[:].rearrange("p j c -> p (j c)"))
                        nc.vector.tensor_add(out=h1[:], in0=h1[:],
                                             in1=l1t[:].rearrange("p j c -> p (j c)"))
                        h1s.append(h1)
                        st = p2s.tile([128, 2, 6], f32, tag="st2")
                        for g in range(2):
                            nc.vector.bn_stats(out=st[:, g, :], in_=h1[:, 384 * g : 384 * (g + 1)])
                        mv = p2s.tile([128, 2], f32, tag="mv2")
                        nc.vector.bn_aggr(out=mv[:], in_=st[:])
                        rstd = p2s.tile([128, 1], f32, tag="rstd2")
                        nc.scalar.activation(out=rstd[:], in_=mv[:, 1:2], func=FT.Sqrt,
                                             bias=eps_sb[:], scale=1.0)
                        nc.vector.reciprocal(out=rstd[:], in_=rstd[:])
                        # n2w/n2b folded into fc1w/fc1b host-side
                        ln2 = p2.tile([128, C], bf16, tag="ln2")
                        nc.vector.tensor_scalar(out=ln2[:], in0=h1[:], scalar1=mv[:, 0:1],
                                                scalar2=rstd[:], op0=OP.subtract, op1=OP.mult)
                        for jb in range(2):
                            pst = ptp.tile([128, 3, 128], bf16, tag="pst")
                            for jj in range(3):
                                j = 3 * jb + jj
                                nc.tensor.transpose(pst[:, jj, :],
                                                    ln2[:, 128 * j : 128 * (j + 1)],
                                                    identb[:])
                            nc.vector.tensor_copy(out=ln2T[:, 3 * jb : 3 * jb + 3, hf, :],
                                                  in_=pst[:])
                    return T0, ln2T, h1s

                def p2_mm(T0, ln2T, h1s):
                    hmidT = p2hm.tile([128, 24, 512], fp8, tag="hmidT")
                    for l in range(24):
                        psH = php.tile([128, 512], f32, tag="psH")
                        for jp in range(3):
                            nc.tensor.matmul(
                                psH[:],
                                fc1w_sb[:, 2 * jp : 2 * jp + 2, 128 * l : 128 * (l + 1)],
                                ln2T[:, 2 * jp : 2 * jp + 2, :, :]
                                    .rearrange("p j h t -> p j (h t)"),
                                start=(jp == 0), stop=(jp == 2), perf_mode=PM)
                        nc.scalar.activation(out=hmidT[:, l, :], in_=psH[:], func=FT.Gelu,
                                             bias=B1[:, l : l + 1], scale=sM16[:, l : l + 1])
                    for hf in range(4):
                        t0 = T0 + 128 * hf
                        psO = pop.tile([128, 2, 512], f32, tag="psO")
                        for lp in range(12):
                            for h2 in range(2):
                                nc.tensor.matmul(
                                    psO[:, h2, 0:384],
                                    hmidT[:, 2 * lp : 2 * lp + 2, 128 * hf : 128 * (hf + 1)],
                                    fc2w_sb[:, 2 * lp : 2 * lp + 2, 384 * h2 : 384 * (h2 + 1)],
                                    start=(lp == 0), stop=(lp == 11), perf_mode=PM)
                        ot = p2.tile([128, C], f32, tag="ot")
                        nc.vector.scalar_tensor_tensor(
                            out=ot[:].rearrange("p (a b) -> p a b", a=2),
                            in0=psO[:, :, 0:384], scalar=1.0 / 16.0,
                            in1=h1s[hf][:].rearrange("p (a b) -> p a b", a=2),
                            op0=OP.mult, op1=OP.add)
                        nc.gpsimd.tensor_add(out=ot[:], in0=ot[:], in1=fc2b_b[:])
                        nc.sync.dma_start(out=out_p[t0 : t0 + 128, :], in_=ot[:])

                # depth-4 software pipeline, preps paired so scalar runs
                # [Sqrt x8, Gelu x48] per pair (one act-table swap per tile)
                sq = [p2_prep(i) for i in range(min(4, NT2))]
                for itp in range(0, NT2, 2):
                    p2_mm(*sq[itp])
                    if itp + 1 < NT2:
                        p2_mm(*sq[itp + 1])
                    for j in (itp + 4, itp + 5):
                        if j < NT2:
                            sq.append(p2_prep(j))

    nc.compile()
    return nc


_NC = None


def _get_nc():
    global _NC
    if _NC is None:
        _NC = _build()
    return _NC


def _dft_mats():
    w = np.arange(W); kw = np.arange(KW)
    ang = 2 * np.pi * np.outer(w, kw) / W
    fwr = (np.cos(ang) / np.sqrt(W)).astype(np.float32)
    fwi = (-np.sin(ang) / np.sqrt(W)).astype(np.float32)
    kh = np.arange(H); h = np.arange(H)
    angh = 2 * np.pi * np.outer(kh, h) / H        # [kh, h]
    fhr = np.cos(angh) / np.sqrt(H)
    fhi = -np.sin(angh) / np.sqrt(H)
    fhs = np.zeros((2 * H, 2 * H))
    fhs[:H, :H] = fhr.T; fhs[:H, H:] = fhi.T
    fhs[H:, :H] = -fhi.T; fhs[H:, H:] = fhr.T
    ci = np.cos(angh) / np.sqrt(H)                # [kh, h] for inverse
    si = np.sin(angh) / np.sqrt(H)
    ifhs = np.zeros((2 * H, 2 * H))
    ifhs[:H, :H] = ci; ifhs[:H, H:] = si
    ifhs[H:, :H] = -si; ifhs[H:, H:] = ci
    ckw = np.where(kw == 0, 1.0, 2.0)
    angi = 2 * np.pi * np.outer(kw, np.arange(W)) / W    # [kw, w]
    ifwr = (ckw[:, None] * np.cos(angi) / np.sqrt(W)).astype(np.float32)
    ifwi = (-ckw[:, None] * np.sin(angi) / np.sqrt(W)).astype(np.float32)
    return fwr, fwi, fhs.astype(np.float32), ifhs.astype(np.float32), ifwr, ifwi


def _prepare_in_maps(x, mod_embed, n1w, n1b, n2w, n2b, w1, b1, w2, b2,
                     fs_w0, fs_b0, fs_w1, fs_b1, fc1w, fc1b, fc2w, fc2b,
                     ms_w0, ms_b0, ms_w1, ms_b1):
    f = np.asarray
    x = f(x, dtype=np.float32)
    grid = x.reshape(H, W, C)
    fwr, fwi, fhs, ifhs, ifwr, ifwi = _dft_mats()
    bf = ml_dtypes.bfloat16

    in_maps = []
    for b in range(N):
        r0, r1 = HP * b, min(HP * (b + 1), H)
        xsb = np.zeros((TOKP, C), np.float32)
        xsb[: (r1 - r0) * W] = grid[r0:r1].reshape(-1, C)
        sl = slice(BS * b, BS * (b + 1))
        w2r = f(w2[0, b], np.float32); w2i = f(w2[1, b], np.float32)
        im = {
            "xs": xsb,
            "modT": np.repeat(f(mod_embed, np.float32).reshape(MODD, 1), 2, axis=1).copy(),
            "n1w": f(n1w, np.float32), "n1b": f(n1b, np.float32),
            "n2w": f(n2w, np.float32), "n2b": f(n2b, np.float32),
            "fwr": fwr.astype(np.float16), "fwi": fwi.astype(np.float16),
            "fhs": fhs.astype(np.float16), "ifhs": ifhs.astype(np.float16),
            "ifwr": ifwr.astype(np.float16), "ifwi": ifwi.astype(np.float16),
            "w1r": f(w1[0, b], np.float16).copy(),
            "w1i": f(w1[1, b], np.float16).copy(),
            "w1in": (-f(w1[1, b], np.float16)).copy(),
            "w2cr": np.concatenate([w2r, w2i], axis=1).astype(np.float16),
            "w2ci": np.concatenate([-w2i, w2r], axis=1).astype(np.float16),
            "b1r": f(b1[0, b], np.float32).reshape(BS, 1).copy(),
            "b1i": f(b1[1, b], np.float32).reshape(BS, 1).copy(),
            "b2c": np.concatenate([f(b2[0, b], np.float32), f(b2[1, b], np.float32)]),
            "fs_w0": f(fs_w0, np.float32),
            "fs_b0T": f(fs_b0, np.float32).reshape(12, 128).T.copy(),
            "fs_w1s": np.concatenate(
                [f(fs_w1, np.float32)[:, sl], f(fs_w1, np.float32)[:, C + BS * b : C + BS * (b + 1)]],
                axis=1),
            "fs_b1s": np.concatenate(
                [f(fs_b1, np.float32)[sl], f(fs_b1, np.float32)[C + BS * b : C + BS * (b + 1)]]
            ).reshape(1, -1),
            "ms_w0": f(ms_w0, np.float32),
            "ms_b0T": f(ms_b0, np.float32).reshape(48, 128).T.copy(),
            "ms_w1s": f(ms_w1, np.float32)[:, C * b : C * (b + 1)].astype(bf),
            "ms_b1s": f(ms_b1, np.float32)[C * b : C * (b + 1)].reshape(1, -1),
            "fc1w": (16.0 * f(n2w, np.float32)[:, None] * f(fc1w, np.float32)
                     ).astype(ml_dtypes.float8_e4m3),
            "fc1bT": (f(fc1b, np.float32)
                      + f(n2b, np.float32) @ f(fc1w, np.float32)
                      ).reshape(24, 128).T.copy(),
            "fc2w": (16.0 * f(fc2w, np.float32)).astype(ml_dtypes.float8_e4m3),
            "fc2b": f(fc2b, np.float32),
        }
        in_maps.append(im)
    return in_maps


def kernel(x, mod_embed, n1w, n1b, n2w, n2b, w1, b1, w2, b2,
           fs_w0, fs_b0, fs_w1, fs_b1, fc1w, fc1b, fc2w, fc2b,
           ms_w0, ms_b0, ms_w1, ms_b1):
    nc = _get_nc()
    in_maps = _prepare_in_maps(
        x, mod_embed, n1w, n1b, n2w, n2b, w1, b1, w2, b2,
        fs_w0, fs_b0, fs_w1, fs_b1, fc1w, fc1b, fc2w, fc2b,
        ms_w0, ms_b0, ms_w1, ms_b1)

    res = run_bass_kernel_spmd(nc, in_maps, core_ids=list(range(N)))
    globals()["last_results"] = res
    out = np.zeros((H, W, C), np.float32)
    for b in range(N):
        r0, r1 = HP * b, min(HP * (b + 1), H)
        out[r0:r1] = res.results[b]["out"][: (r1 - r0) * W].reshape(r1 - r0, W, C)
    return out.reshape(1, H, W, C)



# revision 53
# speedup vs baseline: 1.2271x; 1.0172x over previous
"""AFNO block kernel for 8 Trainium2 NeuronCores.

Sharding: token-shard (H rows, 23 per core padded) for LN/MLP phases;
AllToAll to channel-shard (core i = spectral block i, 96 channels) for the
2D-FFT filter, computed as matmuls against precomputed DFT matrices;
AllToAll back; small AllGather for the column-sharded 6144x6144 scale-shift
MLP weight.
"""

import os
import numpy as np
import ml_dtypes

import concourse.bass as bass
import concourse.bacc as bacc
import concourse.mybir as mybir
import concourse.tile as tile
from concourse.bass_utils import run_bass_kernel_spmd
from concourse.masks import make_identity

f32 = mybir.dt.float32
f32r = mybir.dt.float32r
bf16 = mybir.dt.bfloat16
fp16 = mybir.dt.float16
fp8 = mybir.dt.float8e4
FT = mybir.ActivationFunctionType
OP = mybir.AluOpType

H, W, C = 180, 360, 768
NB, BS, KW = 8, 96, 91
HP = 23                 # rows per shard (8*23 = 184 >= 180)
TOKR = HP * W           # 8280 real token slots per shard
NT2 = 17                # phase-2 tiles of 512
TOKP = NT2 * 512        # 8704 padded tokens per shard
MODD, LAT, LAT2 = 64, 3072, 6144
LAM = 0.01
EPS = 1e-5
N = 8


def rap(t, offset, dims):
    a = t[:] if not isinstance(t, bass.AP) else t
    return bass.AP(tensor=a.tensor, offset=a.offset + offset, ap=[list(d) for d in dims])


def _build():
    nc = bacc.Bacc("TRN2", target_bir_lowering=False, debug=False, num_devices=N)

    def P(name, shp, dt=f32):
        return nc.declare_dram_parameter(name, list(shp), dt, isOutput=False)

    xs = P("xs", [TOKP, C])
    modT = P("modT", [MODD, 2])
    n1w = P("n1w", [C]); n1b = P("n1b", [C])
    n2w = P("n2w", [C]); n2b = P("n2b", [C])
    fwr_p = P("fwr", [W, KW], fp16); fwi_p = P("fwi", [W, KW], fp16)
    fhs_p = P("fhs", [2 * H, 2 * H], fp16)
    ifhs_p = P("ifhs", [2 * H, 2 * H], fp16)
    ifwr_p = P("ifwr", [KW, W], fp16); ifwi_p = P("ifwi", [KW, W], fp16)
    w1r_p = P("w1r", [BS, BS], fp16); w1i_p = P("w1i", [BS, BS], fp16)
    w1in_p = P("w1in", [BS, BS], fp16)
    w2cr_p = P("w2cr", [BS, 2 * BS], fp16)   # [W2r | W2i]
    w2ci_p = P("w2ci", [BS, 2 * BS], fp16)   # [-W2i | W2r]
    b1r_p = P("b1r", [BS, 1]); b1i_p = P("b1i", [BS, 1])
    dcr_p = P("dcr", [BS, 1]); dci_p = P("dci", [BS, 1])
    b2c_p = P("b2c", [2 * BS])               # concat(b2r, b2i)
    fs_w0_p = P("fs_w0", [MODD, 2 * C])
    fs_b0T_p = P("fs_b0T", [128, 12])
    fs_w1s_p = P("fs_w1s", [2 * C, 2 * BS])
    fs_b1s_p = P("fs_b1s", [1, 2 * BS])
    ms_w0_p = P("ms_w0", [MODD, LAT2])
    ms_b0T_p = P("ms_b0T", [128, 48])
    ms_w1s_p = P("ms_w1s", [LAT2, C], bf16)
    ms_b1s_p = P("ms_b1s", [1, C])
    fc1w_p = P("fc1w", [C, LAT], fp8)
    fc1bT_p = P("fc1bT", [128, 24])
    fc2w_p = P("fc2w", [LAT, C], fp8)
    fc2b_p = P("fc2b", [C])
    out_p = nc.declare_dram_parameter("out", [TOKP, C], f32, isOutput=True)
    DBG = False

    # internal DRAM
    a1i = nc.dram_tensor("a1i", [N, TOKR * BS], fp16)
    a1o = nc.dram_tensor("a1o", [N, TOKR * BS], fp16)
    MA = 12 * W * BS        # chunk A: local rows 0-11 per dest
    MB = 11 * W * BS        # chunk B: local rows 12-22
    TA = 12 * W             # tokens per dest covered by chunk A
    a2iA = nc.dram_tensor("a2iA", [N, MA], fp16)
    a2oA = nc.dram_tensor("a2oA", [N, MA], fp16)
    a2iB = nc.dram_tensor("a2iB", [N, MB], fp16)
    a2oB = nc.dram_tensor("a2oB", [N, MB], fp16)
    t1d = nc.dram_tensor("t1d", [2 * H, KW, BS], fp16)  # [hri][kw][c]
    ud = nc.dram_tensor("ud", [KW, BS, 2 * H], fp16)
    sfd = nc.dram_tensor("sfd", [1, 2 * BS], f32)
    agi = nc.dram_tensor("agi", [1, C], f32)
    ago = nc.dram_tensor("ago", [N, C], f32)

    RG = [list(range(N))]

    with tile.TileContext(nc) as tc:
        with (
            tc.tile_pool(name="const", bufs=1) as cpool,
            tc.tile_pool(name="ssb", bufs=1) as ssb,
        ):
            # ---- broadcast constants ----
            def bcast(p, n, name):
                t = cpool.tile([128, n], f32, tag=name)
                nc.sync.dma_start(out=t[:], in_=rap(p, 0, [[0, 128], [1, n]]))
                return t

            n1w_b = bcast(n1w, C, "n1w"); n1b_b = bcast(n1b, C, "n1b")
            n2w_b = bcast(n2w, C, "n2w"); n2b_b = bcast(n2b, C, "n2b")
            fc2b_b = bcast(fc2b_p, C, "fc2b")
            b2c_b = cpool.tile([128, 2, 2 * BS], f32, tag="b2c")
            for bh in range(2):
                nc.sync.dma_start(out=b2c_b[:, bh, :],
                                  in_=rap(b2c_p, 0, [[0, 128], [1, 2 * BS]]))
            eps_sb = cpool.tile([128, 1], f32, tag="eps")
            nc.vector.memset(eps_sb[:], EPS)
            nlam_sb = cpool.tile([128, 1], f32, tag="nlam")
            nc.vector.memset(nlam_sb[:], -LAM)
            ident = cpool.tile([128, 128], f32, tag="ident")
            make_identity(nc, ident[:])
            identb = cpool.tile([128, 128], bf16, tag="identb")
            nc.vector.tensor_copy(out=identb[:], in_=ident[:])


            # ---- phase 0: LN1 + scatter into A2A-1 send buffer ----
            with (
                tc.tile_pool(name="p0", bufs=6) as p0,
                tc.tile_pool(name="p0s", bufs=8) as p0s,
            ):
                for it in range(65):
                    t0 = it * 128
                    nrow = min(128, TOKR - t0)
                    xt = p0.tile([128, C], f32, tag="xt")
                    nc.sync.dma_start(out=xt[:], in_=xs[t0 : t0 + 128, :])
                    st = p0s.tile([128, 2, 6], f32, tag="st")
                    for g in range(2):
                        nc.vector.bn_stats(out=st[:, g, :], in_=xt[:, 384 * g : 384 * (g + 1)])
                    mv = p0s.tile([128, 2], f32, tag="mv")
                    nc.vector.bn_aggr(out=mv[:], in_=st[:])
                    rstd = p0s.tile([128, 1], f32, tag="rstd")
                    nc.scalar.activation(out=rstd[:], in_=mv[:, 1:2], func=FT.Sqrt,
                                         bias=eps_sb[:], scale=1.0)
                    nc.vector.reciprocal(out=rstd[:], in_=rstd[:])
                    # n1w/n1b folded into w1 + DC correction (see dcr/dci);
                    # a1i carries the unaffined normalized x
                    lnh = p0.tile([128, C], fp16, tag="lnh")
                    nc.vector.tensor_scalar(out=lnh[:], in0=xt[:], scalar1=mv[:, 0:1],
                                            scalar2=rstd[:], op0=OP.subtract, op1=OP.mult)
                    nc.scalar.dma_start(
                        out=rap(a1i, t0 * BS, [[BS, nrow], [TOKR * BS, N], [1, BS]]),
                        in_=lnh[:nrow].rearrange("p (j c) -> p j c", j=N),
                    )

            nc.gpsimd.collective_compute(
                "AllToAll", OP.bypass, replica_groups=RG, ins=[a1i[:]], outs=[a1o[:]])

            # ---- scale-shift MLPs (overlap with A2A1 window) ----
            ss_ctx = tc.tile_pool(name="ssw", bufs=1)
            ssw = ss_ctx.__enter__()
            ssp_ctx = tc.tile_pool(name="ssp", bufs=1, space="PSUM")
            ssp = ssp_ctx.__enter__()
            modT_sb = ssw.tile([MODD, 2], f32r)
            nc.sync.dma_start(out=modT_sb[:], in_=modT[:].bitcast(f32r))
            fs_w0_sb = ssw.tile([MODD, 2 * C], f32r)
            nc.sync.dma_start(out=fs_w0_sb[:], in_=fs_w0_p[:].bitcast(f32r))
            fs_b0T_sb = ssw.tile([128, 12], f32)
            nc.sync.dma_start(out=fs_b0T_sb[:], in_=fs_b0T_p[:])
            e0T = ssw.tile([128, 12], f32r)
            for j in range(12):
                pt = ssp.tile([128, 2], f32, tag="ss1")
                nc.tensor.matmul(pt[:], fs_w0_sb[:, 128 * j : 128 * (j + 1)],
                                 modT_sb[:], start=True, stop=True)
                nc.scalar.activation(out=e0T[:, j : j + 1], in_=pt[:, 0:1], func=FT.Gelu,
                                     bias=fs_b0T_sb[:, j : j + 1], scale=1.0)
            fs_w1s_sb = ssw.tile([128, 12, 2 * BS], f32r)
            nc.sync.dma_start(
                out=fs_w1s_sb[:],
                in_=rap(fs_w1s_p, 0, [[2 * BS, 128], [128 * 2 * BS, 12], [1, 2 * BS]]).bitcast(f32r),
            )
            fs_b1s_sb = ssw.tile([1, 2 * BS], f32)
            nc.sync.dma_start(out=fs_b1s_sb[:], in_=fs_b1s_p[:])
            p2 = ssp.tile([1, 2 * BS], f32, tag="ss2")
            for j in range(12):
                nc.tensor.matmul(p2[:], e0T[:, j : j + 1], fs_w1s_sb[:, j, :],
                                 start=(j == 0), stop=(j == 11))
            sfo = ssw.tile([1, 2 * BS], f32)
            nc.vector.tensor_add(out=sfo[:], in0=p2[:], in1=fs_b1s_sb[:])
            nc.sync.dma_start(out=sfd[:], in_=sfo[:])
            sfT = ssw.tile([BS, 2], f32)
            nc.sync.dma_start(out=sfT[:], in_=rap(sfd, 0, [[1, BS], [BS, 2]]))
            sfv = ssb.tile([BS, 1], f32)
            nc.vector.tensor_scalar_add(out=sfv[:], in0=sfT[:, 0:1], scalar1=1.0)
            b1r_sb = ssw.tile([BS, 1], f32)
            nc.sync.dma_start(out=b1r_sb[:], in_=b1r_p[:])
            b1i_sb = ssw.tile([BS, 1], f32)
            nc.sync.dma_start(out=b1i_sb[:], in_=b1i_p[:])
            Br = ssb.tile([BS, 1], f32)
            nc.vector.tensor_mul(out=Br[:], in0=b1r_sb[:], in1=sfv[:])
            nc.vector.tensor_add(out=Br[:], in0=Br[:], in1=sfT[:, 1:2])
            Bi = ssb.tile([BS, 1], f32)
            nc.vector.tensor_mul(out=Bi[:], in0=b1i_sb[:], in1=sfv[:])
            nc.vector.tensor_add(out=Bi[:], in0=Bi[:], in1=sfT[:, 1:2])

            # ms MLP: e1T then column-sharded 6144->768, AllGather
            ms_w0_sb = ssw.tile([MODD, LAT2], f32r)
            nc.gpsimd.dma_start(out=ms_w0_sb[:], in_=ms_w0_p[:].bitcast(f32r))
            ms_b0T_sb = ssw.tile([128, 48], f32)
            nc.sync.dma_start(out=ms_b0T_sb[:], in_=ms_b0T_p[:])
            e1T = ssw.tile([128, 48], bf16)
            for j in range(48):
                pt = ssp.tile([128, 2], f32, tag="ss1")
                nc.tensor.matmul(pt[:], ms_w0_sb[:, 128 * j : 128 * (j + 1)],
                                 modT_sb[:], start=True, stop=True)
                nc.scalar.activation(out=e1T[:, j : j + 1], in_=pt[:, 0:1], func=FT.Gelu,
                                     bias=ms_b0T_sb[:, j : j + 1], scale=1.0)
            p3 = ssp.tile([1, 2, 512], f32, tag="ss3")
            with tc.tile_pool(name="msw", bufs=3) as mswp:
                for j in range(48):
                    wt = mswp.tile([128, C], bf16)
                    nc.gpsimd.dma_start(
                        out=wt[:], in_=ms_w1s_p[128 * j : 128 * (j + 1), :])
                    for h2 in range(2):
                        nc.tensor.matmul(
                            p3[:, h2, 0:384], e1T[:, j : j + 1],
                            wt[:, 384 * h2 : 384 * (h2 + 1)],
                            start=(j == 0), stop=(j == 47))
            ms_b1s_sb = ssw.tile([1, C], f32)
            nc.sync.dma_start(out=ms_b1s_sb[:], in_=ms_b1s_p[:])
            mso = ssw.tile([1, C], f32)
            nc.vector.tensor_add(out=mso[:].rearrange("p (a b) -> p a b", a=2),
                                 in0=p3[:, :, 0:384],
                                 in1=ms_b1s_sb[:].rearrange("p (a b) -> p a b", a=2))
            nc.sync.dma_start(out=agi[:], in_=mso[:])
            nc.gpsimd.collective_compute(
                "AllGather", OP.bypass, replica_groups=RG, ins=[agi[:]], outs=[ago[:]])
            sM = ssb.tile([128, 24], f32)
            nc.sync.dma_start(out=sM[:], in_=rap(ago, 0, [[1, 128], [128, 24]]))
            nc.vector.tensor_scalar_add(out=sM[:], in0=sM[:], scalar1=1.0)
            tM = ssb.tile([128, 24], f32)
            nc.sync.dma_start(out=tM[:], in_=rap(ago, LAT, [[1, 128], [128, 24]]))
            fc1bT_sb = ssw.tile([128, 24], f32)
            nc.sync.dma_start(out=fc1bT_sb[:], in_=fc1bT_p[:])
            B1 = ssb.tile([128, 24], f32)
            nc.vector.tensor_mul(out=B1[:], in0=fc1bT_sb[:], in1=sM[:])
            nc.vector.tensor_add(out=B1[:], in0=B1[:], in1=tM[:])
            sM16 = ssb.tile([128, 24], f32)
            nc.vector.tensor_scalar_mul(out=sM16[:], in0=sM[:], scalar1=1.0 / 16.0)

            ssp_ctx.__exit__(None, None, None)
            ss_ctx.__exit__(None, None, None)

            # ---- phase 1 stage A: W-DFT  (X[h,w,c] -> t1d[kw,ri,h,c]) ----
            with (
                tc.tile_pool(name="sa", bufs=1) as sa,
                tc.tile_pool(name="sac", bufs=3) as sac,
                tc.tile_pool(name="sap", bufs=2, space="PSUM") as sap,
            ):
                fw_sb = []
                for ri, p in enumerate([fwr_p, fwi_p]):
                    t = sa.tile([120, 3, KW], fp16, tag=f"fw{ri}")
                    nc.sync.dma_start(
                        out=t[:], in_=rap(p, 0, [[KW, 120], [120 * KW, 3], [1, KW]]))
                    fw_sb.append(t)
                X_sb = sa.tile([120, 3, H, BS], fp16, tag="xsb")
                for ch in range(4):
                    for k in range(3):
                        eng = nc.scalar if (ch + k) % 2 else nc.sync
                        eng.dma_start(
                            out=X_sb[:, k, 45 * ch : 45 * (ch + 1), :],
                            in_=rap(a1o, (45 * ch * W + 120 * k) * BS,
                                    [[BS, 120], [W * BS, 45], [1, BS]]))
                for hs in range(36):
                    hh0 = 5 * hs
                    cp = sac.tile([KW, 2, 5, BS], fp16, tag="cpa")
                    for ri in range(2):
                        ps = sap.tile([KW, 5, BS], f32, tag="pa")
                        for k in range(3):
                            nc.tensor.matmul(ps[:], fw_sb[ri][:, k, :],
                                             X_sb[:, k, hh0 : hh0 + 5, :],
                                             start=(k == 0), stop=(k == 2))
                        nc.vector.tensor_copy(out=cp[:, ri, :, :], in_=ps[:])
                    for ri in range(2):
                        eng = nc.scalar if ri else nc.sync
                        eng.dma_start(
                            out=rap(t1d, (ri * H + hh0) * KW * BS,
                                    [[BS, KW], [KW * BS, 5], [1, BS]]),
                            in_=cp[:, ri, :, :])

            # ---- stages B+C+D fused in SBUF, then E ----
            with tc.tile_pool(name="fb", bufs=1) as fb:
                # forward H-DFT stacked matrix chunked as 3x120 (vs 4x90):
                # two fewer serialized matmuls per kw pair in stage B
                fhs_sb = fb.tile([120, 3, 2 * H], fp16)
                nc.sync.dma_start(
                    out=fhs_sb[:],
                    in_=rap(fhs_p, 0, [[2 * H, 120], [120 * 2 * H, 3], [1, 2 * H]]))
                ifhs_sb = fb.tile([90, 4, 2 * H], fp16)
                nc.sync.dma_start(
                    out=ifhs_sb[:],
                    in_=rap(ifhs_p, 0, [[2 * H, 90], [90 * 2 * H, 4], [1, 2 * H]]))
                w1r_sb = fb.tile([BS, BS], fp16)
                nc.sync.dma_start(out=w1r_sb[:], in_=w1r_p[:])
                w1i_sb = fb.tile([BS, BS], fp16)
                nc.sync.dma_start(out=w1i_sb[:], in_=w1i_p[:])
                w1in_sb = fb.tile([BS, BS], fp16)
                nc.sync.dma_start(out=w1in_sb[:], in_=w1in_p[:])
                w2cr_sb = fb.tile([BS, 2 * BS], fp16)
                nc.sync.dma_start(out=w2cr_sb[:], in_=w2cr_p[:])
                w2ci_sb = fb.tile([BS, 2 * BS], fp16)
                nc.sync.dma_start(out=w2ci_sb[:], in_=w2ci_p[:])
                dcr_sb = fb.tile([BS, 1], f32)
                nc.sync.dma_start(out=dcr_sb[:], in_=dcr_p[:])
                dci_sb = fb.tile([BS, 1], f32)
                nc.sync.dma_start(out=dci_sb[:], in_=dci_p[:])
                ifw_sb = []
                for ri, p in enumerate([ifwr_p, ifwi_p]):
                    t = fb.tile([KW, 3, 120], fp16, tag=f"ifw{ri}")
                    nc.sync.dma_start(
                        out=t[:], in_=rap(p, 0, [[W, KW], [120, 3], [1, 120]]))
                    ifw_sb.append(t)
                # T1 resident: [hri(120), chunk(3), kw, c] with hri = ri*H + h
                T1_sb = fb.tile([120, 3, KW, BS], fp16, tag="t1sb")
                for k in range(3):
                    eng = nc.scalar if k == 1 else nc.sync
                    eng.dma_start(
                        out=T1_sb[:, k, :, :],
                        in_=rap(t1d, 120 * k * KW * BS,
                                [[KW * BS, 120], [BS, KW], [1, BS]]))
                U_sb = fb.tile([KW, BS, 2 * H], fp16, tag="usb")

                with (
                    tc.tile_pool(name="bcw", bufs=4) as bcw,
                    tc.tile_pool(name="bct", bufs=4) as bct,
                    tc.tile_pool(name="bco", bufs=3) as bco,
                    tc.tile_pool(name="bcp1", bufs=1, space="PSUM") as bcps,
                    tc.tile_pool(name="bcp2", bufs=1, space="PSUM") as bcps2,
                    tc.tile_pool(name="bcp3", bufs=2, space="PSUM") as bcps3,
                    tc.tile_pool(name="bcp4", bufs=1, space="PSUM") as bcps4,
                ):
                    def emit_D(O2t, kw0, G):
                        psU = bcps4.tile([BS, 2, 512], f32, tag="psU")
                        for g in range(G):
                            for q in range(4):
                                ri, half = q // 2, q % 2
                                nc.tensor.matmul(
                                    psU[:, g, 0 : 2 * H],
                                    O2t[:, g, half, ri * BS : (ri + 1) * BS],
                                    ifhs_sb[:, q, :], start=(q == 0), stop=(q == 3))
                        ucp = bcw.tile([BS, 2, 2 * H], fp16, tag="ucp")
                        nc.vector.tensor_copy(out=ucp[:, :G, :], in_=psU[:, :G, 0 : 2 * H])
                        nc.sync.dma_start(
                            out=rap(ud, kw0 * BS * 2 * H,
                                    [[2 * H, BS], [BS * 2 * H, G], [1, 2 * H]]),
                            in_=ucp[:, :G, :])
                        # stream this pair back into kw-partitioned U_sb on
                        # the idle gpsimd DMA queue (hides the pre-stage-E
                        # bulk reload)
                        nc.gpsimd.dma_start(
                            out=U_sb[kw0 : kw0 + G, :, :],
                            in_=rap(ud, kw0 * BS * 2 * H,
                                    [[BS * 2 * H, G], [2 * H, BS], [1, 2 * H]]))

                    pend = None
                    for pr in range(46):
                        kw0 = 2 * pr
                        G = 2 if kw0 + 1 < KW else 1
                        psF = bcps.tile([BS, 2, 512], f32, tag="psF")
                        for g in range(G):
                            kw = kw0 + g
                            for k in range(3):
                                nc.tensor.matmul(
                                    psF[:, g, 0 : 2 * H], T1_sb[:, k, kw, :],
                                    fhs_sb[:, k, :], start=(k == 0), stop=(k == 2))
                        fsb = bcw.tile([BS, 2, 2 * H], fp16, tag="fsb")
                        nc.vector.tensor_copy(out=fsb[:, :G, :], in_=psF[:, :G, 0 : 2 * H])
                        ps1r = bcps2.tile([BS, 2, H], f32, tag="ps1r")
                        ps1i = bcps2.tile([BS, 2, H], f32, tag="ps1i")
                        nc.tensor.matmul(ps1r[:, :G, :], w1r_sb[:], fsb[:, :G, 0:H],
                                         start=True, stop=False)
                        nc.tensor.matmul(ps1r[:, :G, :], w1in_sb[:], fsb[:, :G, H : 2 * H],
                                         start=False, stop=True)
                        nc.tensor.matmul(ps1i[:, :G, :], w1i_sb[:], fsb[:, :G, 0:H],
                                         start=True, stop=False)
                        nc.tensor.matmul(ps1i[:, :G, :], w1r_sb[:], fsb[:, :G, H : 2 * H],
                                         start=False, stop=True)
                        if pr == 0:
                            # n1b's DC-bin contribution (kh=0, kw=0 only)
                            nc.vector.tensor_add(out=ps1r[:, 0, 0:1],
                                                 in0=ps1r[:, 0, 0:1], in1=dcr_sb[:])
                            nc.vector.tensor_add(out=ps1i[:, 0, 0:1],
                                                 in0=ps1i[:, 0, 0:1], in1=dci_sb[:])
                        o1r = bcw.tile([BS, 2, H], fp16, tag="o1r")
                        o1i = bcw.tile([BS, 2, H], fp16, tag="o1i")
                        nc.scalar.activation(out=o1r[:, :G, :], in_=ps1r[:, :G, :],
                                             func=FT.Relu, bias=Br[:], scale=sfv[:])
                        nc.scalar.activation(out=o1i[:, :G, :], in_=ps1i[:, :G, :],
                                             func=FT.Relu, bias=Bi[:], scale=sfv[:])
                        o1rf = o1r[:].rearrange("p g k -> p (g k)")
                        o1if = o1i[:].rearrange("p g k -> p (g k)")
                        O2t = bco.tile([90, 2, 2, 2 * BS], fp16, tag="o2sb")
                        for g in range(G):
                            ps2 = bcps3.tile([90, 2, 2 * BS], f32, tag="ps2")
                            for half in range(2):
                                sl = slice(90 * (2 * g + half), 90 * (2 * g + half + 1))
                                nc.tensor.matmul(ps2[:, half, :], o1rf[:, sl], w2cr_sb[:],
                                                 start=True, stop=False)
                                nc.tensor.matmul(ps2[:, half, :], o1if[:, sl], w2ci_sb[:],
                                                 start=False, stop=True)
                            tmp = bct.tile([90, 2, 2 * BS], f32, tag="tmp")
                            nc.vector.tensor_add(out=tmp[:], in0=ps2[:], in1=b2c_b[:90])
                            r1 = bct.tile([90, 2, 2 * BS], f32, tag="r1")
                            nc.vector.tensor_scalar(out=r1[:], in0=tmp[:], scalar1=-LAM,
                                                    scalar2=LAM, op0=OP.max, op1=OP.min)
                            nc.vector.tensor_sub(out=O2t[:, g, :, :],
                                                 in0=tmp[:], in1=r1[:])
                        # stage D (inverse H-DFT) pipelined one iteration
                        # behind B/C so the softshrink chain never stalls
                        # the in-order tensor queue
                        if pend is not None:
                            emit_D(*pend)
                        pend = (O2t, kw0, G)
                    emit_D(*pend)

                # stage E (inverse W-DFT); U_sb streamed in by emit_D above
                with (
                    tc.tile_pool(name="sec", bufs=4) as sec,
                    tc.tile_pool(name="sep", bufs=2, space="PSUM") as sep,
                ):
                    # chunk A = local rows 0-11 per dest, chunk B = rows 12-22;
                    # A2A for chunk A overlaps stage-E compute of chunk B.
                    for part in range(2):
                        rts = [(0, 4), (4, 4), (8, 4)] if part == 0 else \
                              [(12, 4), (16, 4), (20, 3)]
                        for d in range(N):
                            for roff, nr in rts:
                                h0 = HP * d + roff
                                nr = min(nr, H - h0)
                                if nr <= 0:
                                    continue
                                for wk in range(3):
                                    psE = sep.tile([120, 4, BS], f32, tag="psE")
                                    for ri in range(2):
                                        nc.tensor.matmul(
                                            psE[:, :nr, :], ifw_sb[ri][:, wk, :],
                                            U_sb[:, :, ri * H + h0 : ri * H + h0 + nr]
                                                .rearrange("p c h -> p h c"),
                                            start=(ri == 0), stop=(ri == 1))
                                    ecp = sec.tile([120, 4, BS], fp16, tag="ecp")
                                    nc.vector.tensor_copy(out=ecp[:, :nr, :],
                                                          in_=psE[:, :nr, :])
                                    if part == 0:
                                        dst, off = a2iA, d * MA + (roff * W + wk * 120) * BS
                                    else:
                                        dst, off = a2iB, d * MB + ((roff - 12) * W + wk * 120) * BS
                                    nc.scalar.dma_start(
                                        out=rap(dst, off, [[BS, 120], [W * BS, nr], [1, BS]]),
                                        in_=ecp[:, :nr, :])
                        if part == 0:
                            nc.gpsimd.collective_compute(
                                "AllToAll", OP.bypass, replica_groups=RG,
                                ins=[a2iA[:]], outs=[a2oA[:]])

            nc.gpsimd.collective_compute(
                "AllToAll", OP.bypass, replica_groups=RG, ins=[a2iB[:]], outs=[a2oB[:]])

            # ---- phase 2: h1 = F2 + ln1x + x; LN2; modulated MLP; + h1 ----
            with (
                tc.tile_pool(name="p2w", bufs=1) as p2w,
                tc.tile_pool(name="p2", bufs=4) as p2,
                tc.tile_pool(name="p2h", bufs=16) as p2h,
                tc.tile_pool(name="p2hm", bufs=2) as p2hm,
                tc.tile_pool(name="p2s", bufs=4) as p2s,
                tc.tile_pool(name="p2m", bufs=4) as p2m,
                tc.tile_pool(name="ptp", bufs=2, space="PSUM") as ptp,
                tc.tile_pool(name="php", bufs=2, space="PSUM") as php,
                tc.tile_pool(name="pop", bufs=2, space="PSUM") as pop,
            ):
                PM = mybir.MatmulPerfMode.DoubleRow
                fc1w_sb = p2w.tile([128, 6, LAT], fp8)
                nc.sync.dma_start(
                    out=fc1w_sb[:], in_=rap(fc1w_p, 0, [[LAT, 128], [128 * LAT, 6], [1, LAT]]))
                fc2w_sb = p2w.tile([128, 24, C], fp8)
                nc.sync.dma_start(
                    out=fc2w_sb[:], in_=rap(fc2w_p, 0, [[C, 128], [128 * C, 24], [1, C]]))

                def p2_prep(it):
                    T0 = it * 512
                    ln2T = p2m.tile([128, 6, 4, 128], fp8, tag="ln2T")
                    h1s = []
                    for hf in range(4):
                        t0 = T0 + 128 * hf
                        nload = max(0, min(128, TOKR - t0))
                        xt = p2.tile([128, C], f32, tag="xt2")
                        nc.sync.dma_start(out=xt[:], in_=xs[t0 : t0 + 128, :])
                        f2t = p2.tile([128, N, BS], fp16, tag="f2t")
                        l1t = p2.tile([128, N, BS], fp16, tag="l1t")
                        if nload < 128:
                            nc.vector.memset(f2t[:], 0.0)
                            nc.vector.memset(l1t[:], 0.0)
                        if nload > 0:
                            ta = min(nload, max(0, TA - t0))
                            if ta > 0:
                                nc.sync.dma_start(
                                    out=f2t[:ta],
                                    in_=rap(a2oA, t0 * BS, [[BS, ta], [MA, N], [1, BS]]))
                            if ta < nload:
                                t0b = t0 + ta - TA
                                nc.sync.dma_start(
                                    out=f2t[ta:nload],
                                    in_=rap(a2oB, t0b * BS, [[BS, nload - ta], [MB, N], [1, BS]]))
                            nc.sync.dma_start(
                                out=l1t[:nload],
                                in_=rap(a1i, t0 * BS, [[BS, nload], [TOKR * BS, N], [1, BS]]))
                        # reconstruct affined ln1 (= z*n1w + n1b) on idle gpsimd
                        lw = p2.tile([128, C], f32, tag="lw")
                        nc.gpsimd.tensor_mul(
                            out=lw[:], in0=l1t[:].rearrange("p j c -> p (j c)"),
                            in1=n1w_b[:])
                        nc.gpsimd.tensor_add(out=lw[:], in0=lw[:], in1=n1b_b[:])
                        h1 = p2h.tile([128, C], f32, tag="h1")
                        nc.vector.tensor_add(out=h1[:], in0=xt[:],
                                             in1=f2t[:].rearrange("p j c -> p (j c)"))
                        nc.vector.tensor_add(out=h1[:], in0=h1[:], in1=lw[:])
                        h1s.append(h1)
                        st = p2s.tile([128, 2, 6], f32, tag="st2")
                        for g in range(2):
                            nc.vector.bn_stats(out=st[:, g, :], in_=h1[:, 384 * g : 384 * (g + 1)])
                        mv = p2s.tile([128, 2], f32, tag="mv2")
                        nc.vector.bn_aggr(out=mv[:], in_=st[:])
                        rstd = p2s.tile([128, 1], f32, tag="rstd2")
                        nc.scalar.activation(out=rstd[:], in_=mv[:, 1:2], func=FT.Sqrt,
                                             bias=eps_sb[:], scale=1.0)
                        nc.vector.reciprocal(out=rstd[:], in_=rstd[:])
                        # n2w/n2b folded into fc1w/fc1b host-side
                        ln2 = p2.tile([128, C], bf16, tag="ln2")
                        nc.vector.tensor_scalar(out=ln2[:], in0=h1[:], scalar1=mv[:, 0:1],
                                                scalar2=rstd[:], op0=OP.subtract, op1=OP.mult)
                        for jb in range(2):
                            pst = ptp.tile([128, 3, 128], bf16, tag="pst")
                            for jj in range(3):
                                j = 3 * jb + jj
                                nc.tensor.transpose(pst[:, jj, :],
                                                    ln2[:, 128 * j : 128 * (j + 1)],
                                                    identb[:])
                            nc.vector.tensor_copy(out=ln2T[:, 3 * jb : 3 * jb + 3, hf, :],
                                                  in_=pst[:])
                    return T0, ln2T, h1s

                def p2_mm(T0, ln2T, h1s):
                    hmidT = p2hm.tile([128, 24, 512], fp8, tag="hmidT")
                    for l in range(24):
                        psH = php.tile([128, 512], f32, tag="psH")
                        for jp in range(3):
                            nc.tensor.matmul(
                                psH[:],
                                fc1w_sb[:, 2 * jp : 2 * jp + 2, 128 * l : 128 * (l + 1)],
                                ln2T[:, 2 * jp : 2 * jp + 2, :, :]
                                    .rearrange("p j h t -> p j (h t)"),
                                start=(jp == 0), stop=(jp == 2), perf_mode=PM)
                        nc.scalar.activation(out=hmidT[:, l, :], in_=psH[:], func=FT.Gelu,
                                             bias=B1[:, l : l + 1], scale=sM16[:, l : l + 1])
                    for hf in range(4):
                        t0 = T0 + 128 * hf
                        psO = pop.tile([128, 2, 512], f32, tag="psO")
                        for lp in range(12):
                            for h2 in range(2):
                                nc.tensor.matmul(
                                    psO[:, h2, 0:384],
                                    hmidT[:, 2 * lp : 2 * lp + 2, 128 * hf : 128 * (hf + 1)],
                                    fc2w_sb[:, 2 * lp : 2 * lp + 2, 384 * h2 : 384 * (h2 + 1)],
                                    start=(lp == 0), stop=(lp == 11), perf_mode=PM)
                        ot = p2.tile([128, C], f32, tag="ot")
                        nc.vector.scalar_tensor_tensor(
                            out=ot[:].rearrange("p (a b) -> p a b", a=2),
                            in0=psO[:, :, 0:384], scalar=1.0 / 16.0,
                            in1=h1s[hf][:].rearrange("p (a b) -> p a b", a=2),
                            op0=OP.mult, op1=OP.add)
                        nc.gpsimd.tensor_add(out=ot[:], in0=ot[:], in1=fc2b_b[:])
                        nc.sync.dma_start(out=out_p[t0 : t0 + 128, :], in_=ot[:])

                # depth-4 software pipeline, preps paired so scalar runs
                # [Sqrt x8, Gelu x48] per pair (one act-table swap per tile)
                sq = [p2_prep(i) for i in range(min(4, NT2))]
                for itp in range(0, NT2, 2):
                    p2_mm(*sq[itp])
                    if itp + 1 < NT2:
                        p2_mm(*sq[itp + 1])
                    for j in (itp + 4, itp + 5):
                        if j < NT2:
                            sq.append(p2_prep(j))

    nc.compile()
    return nc


_NC = None


def _get_nc():
    global _NC
    if _NC is None:
        _NC = _build()
    return _NC


def _dft_mats():
    w = np.arange(W); kw = np.arange(KW)
    ang = 2 * np.pi * np.outer(w, kw) / W
    fwr = (np.cos(ang) / np.sqrt(W)).astype(np.float32)
    fwi = (-np.sin(ang) / np.sqrt(W)).astype(np.float32)
    kh = np.arange(H); h = np.arange(H)
    angh = 2 * np.pi * np.outer(kh, h) / H        # [kh, h]
    fhr = np.cos(angh) / np.sqrt(H)
    fhi = -np.sin(angh) / np.sqrt(H)
    fhs = np.zeros((2 * H, 2 * H))
    fhs[:H, :H] = fhr.T; fhs[:H, H:] = fhi.T
    fhs[H:, :H] = -fhi.T; fhs[H:, H:] = fhr.T
    ci = np.cos(angh) / np.sqrt(H)                # [kh, h] for inverse
    si = np.sin(angh) / np.sqrt(H)
    ifhs = np.zeros((2 * H, 2 * H))
    ifhs[:H, :H] = ci; ifhs[:H, H:] = si
    ifhs[H:, :H] = -si; ifhs[H:, H:] = ci
    ckw = np.where(kw == 0, 1.0, 2.0)
    angi = 2 * np.pi * np.outer(kw, np.arange(W)) / W    # [kw, w]
    ifwr = (ckw[:, None] * np.cos(angi) / np.sqrt(W)).astype(np.float32)
    ifwi = (-ckw[:, None] * np.sin(angi) / np.sqrt(W)).astype(np.float32)
    return fwr, fwi, fhs.astype(np.float32), ifhs.astype(np.float32), ifwr, ifwi


def _prepare_in_maps(x, mod_embed, n1w, n1b, n2w, n2b, w1, b1, w2, b2,
                     fs_w0, fs_b0, fs_w1, fs_b1, fc1w, fc1b, fc2w, fc2b,
                     ms_w0, ms_b0, ms_w1, ms_b1):
    f = np.asarray
    x = f(x, dtype=np.float32)
    grid = x.reshape(H, W, C)
    fwr, fwi, fhs, ifhs, ifwr, ifwi = _dft_mats()
    bf = ml_dtypes.bfloat16

    in_maps = []
    for b in range(N):
        r0, r1 = HP * b, min(HP * (b + 1), H)
        xsb = np.zeros((TOKP, C), np.float32)
        xsb[: (r1 - r0) * W] = grid[r0:r1].reshape(-1, C)
        sl = slice(BS * b, BS * (b + 1))
        w2r = f(w2[0, b], np.float32); w2i = f(w2[1, b], np.float32)
        im = {
            "xs": xsb,
            "modT": np.repeat(f(mod_embed, np.float32).reshape(MODD, 1), 2, axis=1).copy(),
            "n1w": f(n1w, np.float32), "n1b": f(n1b, np.float32),
            "n2w": f(n2w, np.float32), "n2b": f(n2b, np.float32),
            "fwr": fwr.astype(np.float16), "fwi": fwi.astype(np.float16),
            "fhs": fhs.astype(np.float16), "ifhs": ifhs.astype(np.float16),
            "ifwr": ifwr.astype(np.float16), "ifwi": ifwi.astype(np.float16),
            "w1r": (f(n1w, np.float32)[sl][:, None]
                    * f(w1[0, b], np.float32)).astype(np.float16),
            "w1i": (f(n1w, np.float32)[sl][:, None]
                    * f(w1[1, b], np.float32)).astype(np.float16),
            "w1in": (-f(n1w, np.float32)[sl][:, None]
                     * f(w1[1, b], np.float32)).astype(np.float16),
            "dcr": (np.sqrt(H * W) * (f(n1b, np.float32)[sl]
                    @ f(w1[0, b], np.float32))).reshape(BS, 1),
            "dci": (np.sqrt(H * W) * (f(n1b, np.float32)[sl]
                    @ f(w1[1, b], np.float32))).reshape(BS, 1),
            "w2cr": np.concatenate([w2r, w2i], axis=1).astype(np.float16),
            "w2ci": np.concatenate([-w2i, w2r], axis=1).astype(np.float16),
            "b1r": f(b1[0, b], np.float32).reshape(BS, 1).copy(),
            "b1i": f(b1[1, b], np.float32).reshape(BS, 1).copy(),
            "b2c": np.concatenate([f(b2[0, b], np.float32), f(b2[1, b], np.float32)]),
            "fs_w0": f(fs_w0, np.float32),
            "fs_b0T": f(fs_b0, np.float32).reshape(12, 128).T.copy(),
            "fs_w1s": np.concatenate(
                [f(fs_w1, np.float32)[:, sl], f(fs_w1, np.float32)[:, C + BS * b : C + BS * (b + 1)]],
                axis=1),
            "fs_b1s": np.concatenate(
                [f(fs_b1, np.float32)[sl], f(fs_b1, np.float32)[C + BS * b : C + BS * (b + 1)]]
            ).reshape(1, -1),
            "ms_w0": f(ms_w0, np.float32),
            "ms_b0T": f(ms_b0, np.float32).reshape(48, 128).T.copy(),
            "ms_w1s": f(ms_w1, np.float32)[:, C * b : C * (b + 1)].astype(bf),
            "ms_b1s": f(ms_b1, np.float32)[C * b : C * (b + 1)].reshape(1, -1),
            "fc1w": (16.0 * f(n2w, np.float32)[:, None] * f(fc1w, np.float32)
                     ).astype(ml_dtypes.float8_e4m3),
            "fc1bT": (f(fc1b, np.float32)
                      + f(n2b, np.float32) @ f(fc1w, np.float32)
                      ).reshape(24, 128).T.copy(),
            "fc2w": (16.0 * f(fc2w, np.float32)).astype(ml_dtypes.float8_e4m3),
            "fc2b": f(fc2b, np.float32),
        }
        in_maps.append(im)
    return in_maps


def kernel(x, mod_embed, n1w, n1b, n2w, n2b, w1, b1, w2, b2,
           fs_w0, fs_b0, fs_w1, fs_b1, fc1w, fc1b, fc2w, fc2b,
           ms_w0, ms_b0, ms_w1, ms_b1):
    nc = _get_nc()
    in_maps = _prepare_in_maps(
        x, mod_embed, n1w, n1b, n2w, n2b, w1, b1, w2, b2,
        fs_w0, fs_b0, fs_w1, fs_b1, fc1w, fc1b, fc2w, fc2b,
        ms_w0, ms_b0, ms_w1, ms_b1)

    res = run_bass_kernel_spmd(nc, in_maps, core_ids=list(range(N)))
    globals()["last_results"] = res
    out = np.zeros((H, W, C), np.float32)
    for b in range(N):
        r0, r1 = HP * b, min(HP * (b + 1), H)
        out[r0:r1] = res.results[b]["out"][: (r1 - r0) * W].reshape(r1 - r0, W, C)
    return out.reshape(1, H, W, C)

